# revision 4
# baseline (speedup 1.0000x reference)
"""Causal single-head attention (batch=8, ctx=2048, d=1024) on 8 trn2 cores.

Strategy: pure data-parallel over batch — core b computes attention for
batch element b with no cross-core communication.

Per-core pipeline:
  phase 1: Q^T, K^T (o-major) and V (s-major) projections accumulated in
           PSUM (fp32), consumed per 512-column s-group of x^T.
  phase 2: flash-style causal attention per 128-row q-block:
           S = Q^T.T @ K^T, additive causal mask on the diagonal
           128x128 sub-tile, one-pass softmax (per-tile exp(s - m_tile)
           on ACT with row-sum accumulators, exp(m_tile - m_all)
           correction folded into P), P transposed per tile on the PE,
           O = P @ V accumulated in PSUM, deferred normalization by the
           reciprocal row sum, DMA out (fp32); two-block software
           pipeline so PV of one block hides the next one's softmax.

MODE selects the matmul input dtype:
  "fp32": all matmul inputs fp32 (4 cyc/row); x^T built on-device via PE
          transposes; Q^T/K^T/V staged through DRAM scratch (SBUF can't
          hold x^T + all three in fp32).  ~1.25ms, rel err ~6e-6.
  "fp16": matmul inputs fp16 (1 cyc/row), fp32 PSUM accumulation and
          softmax; x^T and the weights are pre-cast/pre-transposed on the
          host (pure layout prep, bit-identical to a DVE cast) and
          everything stays resident in SBUF.  ~332us, rel err ~5e-4.
"""

import sys

sys.path.insert(0, "/opt/trn_rl_repo")

import numpy as np

import concourse.bass as bass
import concourse.mybir as mybir
import concourse.tile as tile
from concourse.masks import make_identity
from concourse.vector_clock import ScopedClock

MODE = "fp8"

BATCH = 8
CTX = 2048
D_IN = 1024
D_OUT = 1024
N_CORES = 8
P = 128
F32 = mybir.dt.float32
F16 = mybir.dt.float16
F8 = mybir.dt.float8e4
DR = mybir.MatmulPerfMode.DoubleRow
NEG_BIG = -1.0e30
R16 = 5  # q/seq blocks 0..R16-1 use the fp16 path; the rest fp8 DoubleRow
RS = R16 * P

# ---------------------------------------------------------------------------
# Workarounds for the walrus build on this stack: it rejects any instruction
# carrying more than ONE sync wait.  (1) Patch the TileContext tail drain to
# spread its waits over preceding sync-engine nops; (2) post-pass that hoists
# extra waits from any instruction onto same-engine nops inserted right
# before it (sequencers execute per-engine streams in order, so this is
# semantics-preserving).
# ---------------------------------------------------------------------------


def _patched_drain_and_barrier(self, tick_clock, wait_clock):
    nc = self.nc
    nops = [nc.sync.nop(nofuse=True) for _ in range(27)]
    drain_inst = nc.sync.drain()
    wait_clock.add_sem_waits(
        drain_inst.ins, ScopedClock({None: tick_clock.global_clock})
    )
    si = drain_inst.ins.sync_info
    if si is not None and si.on_wait is not None and len(si.on_wait) > 1:
        waits = list(si.on_wait)
        si.on_wait = waits[:1]
        rest = waits[1:]
        for i, nop in enumerate(nops):
            chunk = rest[i : i + 1]
            if not chunk:
                break
            nsi = nop.ins.sync_info
            if nsi is None:
                nop.ins.sync_info = mybir.SyncInfo(on_wait=chunk, on_update=[])
            else:
                nsi.on_wait = chunk

    nc.all_engine_barrier()
    assert self.sems is not None
    popped = nc._tile_sem_poison_stack.pop()
    assert popped is self._sem_poison
    nc.clear_and_free_semaphores(list(self.sems.allocated().values()))
    nc.all_engine_barrier()


tile.TileContext._drain_and_barrier = _patched_drain_and_barrier


def _split_multi_waits(nc):
    n_split = 0
    for f in nc.m.functions:
        for bb in f.blocks:
            il = bb.instructions
            if not any(
                inst.sync_info is not None
                and inst.sync_info.on_wait
                and len(inst.sync_info.on_wait) > 1
                for inst in il
            ):
                continue
            new = []
            for inst in il:
                si = inst.sync_info
                if si is not None and si.on_wait and len(si.on_wait) > 1:
                    waits = list(si.on_wait)
                    for w in waits[:-1]:
                        nop = mybir.InstNoOp(
                            name=nc.get_next_instruction_name(), ins=[], outs=[]
                        )
                        nop.engine = inst.engine
                        nop.sync_info = mybir.SyncInfo(on_wait=[w], on_update=[])
                        new.append(nop)
                        n_split += 1
                    si.on_wait = [waits[-1]]
                new.append(inst)
            il[:] = new
    return n_split


# ---------------------------------------------------------------------------
# Program builders
# ---------------------------------------------------------------------------

IC = D_IN // P  # 8 input-dim chunks
OC = D_OUT // P  # 8 output-dim chunks
ST = CTX // P  # 16 seq chunks
QB = CTX // P  # 16 q blocks
MMW = 512  # moving width (psum bank = 512 fp32)


def _declare_io(nc):
    x_d = nc.declare_dram_parameter("x", [CTX, D_IN], F32, isOutput=False)
    wq_d = nc.declare_dram_parameter("Wq", [D_IN, D_OUT], F32, isOutput=False)
    wk_d = nc.declare_dram_parameter("Wk", [D_IN, D_OUT], F32, isOutput=False)
    wv_d = nc.declare_dram_parameter("Wv", [D_IN, D_OUT], F32, isOutput=False)
    negmask_d = nc.declare_dram_parameter("negmask", [P, P], F32, isOutput=False)
    out_d = nc.declare_dram_parameter("out", [CTX, D_OUT], F32, isOutput=True)
    return x_d, wq_d, wk_d, wv_d, negmask_d, out_d


def _attention_phase(nc, tc, consts_ident, negmask, qt_src, kt_sb, v_sb, out_d, dt):
    """qt_src(qb) -> [P, OC, P] tile of Q^T for that block.

    One-pass softmax: each score tile gets exp(s - m_tile) immediately
    (ACT, off the PE critical path); after the block's tiles are done the
    per-tile correction exp(m_tile - m_all) is folded into the 16-bit P
    tiles and the row-sum."""
    with (
        tc.tile_pool(name="pexp", bufs=3) as p_pool,
        tc.tile_pool(name="pexp32", bufs=3) as p32_pool,
        tc.tile_pool(name="ptr", bufs=2) as pt_pool,
        tc.tile_pool(name="red", bufs=3) as red_pool,
        tc.tile_pool(name="ob", bufs=3) as o_pool,
        tc.tile_pool(name="ps_s", bufs=4, space="PSUM") as ps_s,
        tc.tile_pool(name="ps_o", bufs=2, space="PSUM") as ps_o,
        tc.tile_pool(name="ps_pt", bufs=2, space="PSUM") as ps_pt,
    ):

        def emit_scores(qb):
            L = (qb + 1) * P
            ktiles = [(k0, min(MMW, L - k0)) for k0 in range(0, L, MMW)]
            nkt = len(ktiles)

            qt_b = qt_src(qb)

            red = red_pool.tile([P, 4 * nkt + 3], F32, tag="red")
            mx = red[:, 0:nkt]
            negm = red[:, nkt : 2 * nkt]
            sm = red[:, 2 * nkt : 3 * nkt]
            scl = red[:, 3 * nkt : 4 * nkt]
            negm_all = red[:, 4 * nkt : 4 * nkt + 1]
            rsum = red[:, 4 * nkt + 1 : 4 * nkt + 2]
            rinv = red[:, 4 * nkt + 2 : 4 * nkt + 3]

            p_sb = p_pool.tile([P, CTX], dt, tag="p")
            p32_sb = p32_pool.tile([P, CTX], F32, tag="p32")
            for idx, (k0, w) in enumerate(ktiles):
                ps = ps_s.tile([P, MMW], F32, tag="ps_s")
                for oc in range(OC):
                    nc.tensor.matmul(
                        ps[:, :w],
                        qt_b[:, oc, :],
                        kt_sb[:, oc, k0 : k0 + w],
                        start=(oc == 0),
                        stop=(oc == OC - 1),
                    )
                if idx == nkt - 1:
                    nc.vector.tensor_add(
                        ps[:, w - P : w], ps[:, w - P : w], negmask[:]
                    )
                nc.vector.reduce_max(
                    mx[:, idx : idx + 1], ps[:, :w], axis=mybir.AxisListType.X
                )
                nc.scalar.mul(
                    negm[:, idx : idx + 1], mx[:, idx : idx + 1], -0.03125
                )
                # exp(s - m_tile) immediately; row-sums into sm[idx]
                nc.scalar.activation(
                    p32_sb[:, k0 : k0 + w],
                    ps[:, :w],
                    mybir.ActivationFunctionType.Exp,
                    bias=negm[:, idx : idx + 1],
                    scale=0.03125,
                    accum_out=sm[:, idx : idx + 1],
                )
            # combine: negm_all = min_idx(-m_idx/32) = -m_all/32;
            # scl_idx = exp((m_idx - m_all)/32)
            nc.vector.tensor_reduce(
                negm_all[:], negm[:], axis=mybir.AxisListType.X,
                op=mybir.AluOpType.min,
            )
            nc.scalar.activation(
                scl[:],
                mx[:],
                mybir.ActivationFunctionType.Exp,
                bias=negm_all[:, 0:1],
                scale=0.03125,
            )
            nc.vector.tensor_mul(sm[:], sm[:], scl[:])
            nc.vector.reduce_sum(rsum[:], sm[:], axis=mybir.AxisListType.X)
            nc.vector.reciprocal(rinv[:], rsum[:])
            for idx, (k0, w) in enumerate(ktiles):
                nc.vector.tensor_scalar_mul(
                    p_sb[:, k0 : k0 + w],
                    p32_sb[:, k0 : k0 + w],
                    scl[:, idx : idx + 1],
                )
            return {"qb": qb, "p_sb": p_sb, "rinv": rinv}

        def emit_pv(stt):
            qb = stt["qb"]
            p_sb = stt["p_sb"]
            rinv = stt["rinv"]
            L = (qb + 1) * P
            pt_sb = pt_pool.tile([P, L], dt, tag="pt")
            for c0 in range(0, qb + 1, 4):
                cn = min(4, qb + 1 - c0)
                ptp = ps_pt.tile([P, 4 * P], dt, tag="ptp")
                for j in range(cn):
                    kc = c0 + j
                    nc.tensor.transpose(
                        ptp[:, j * P : (j + 1) * P],
                        p_sb[:, kc * P : (kc + 1) * P],
                        consts_ident[:],
                    )
                nc.vector.tensor_copy(
                    pt_sb[:, c0 * P : c0 * P + cn * P], ptp[:, : cn * P]
                )

            o_sb = o_pool.tile([P, D_OUT], F32, tag="o")
            for o0 in range(0, D_OUT, MMW):
                pso = ps_o.tile([P, MMW], F32, tag="ps_o")
                for kc in range(qb + 1):
                    nc.tensor.matmul(
                        pso[:],
                        pt_sb[:, kc * P : (kc + 1) * P],
                        v_sb[:, kc, o0 : o0 + MMW],
                        start=(kc == 0),
                        stop=(kc == qb),
                    )
                nc.vector.tensor_scalar_mul(
                    o_sb[:, o0 : o0 + MMW], pso[:], rinv[:, 0:1]
                )
            nc.sync.dma_start(out_d[qb * P : (qb + 1) * P, :], o_sb[:])

        # two-block software pipeline: PV of the previous block hides the
        # softmax latency of the current one.  The four smallest blocks run
        # first (their PV is too short to hide a softmax), then largest-
        # first, so the exposed tail block still has a few-us PV.
        order = [3, 2, 1, 0] + list(range(QB - 1, 3, -1))
        prev = None
        for qb in order:
            stt = emit_scores(qb)
            if prev is not None:
                emit_pv(prev)
            prev = stt
        emit_pv(prev)


def build_program_fp32():
    nc = bass.Bass()
    x_d, wq_d, wk_d, wv_d, negmask_d, out_d = _declare_io(nc)
    qt_d = nc.dram_tensor("qt_scratch", [D_OUT, CTX], F32)
    kt_d = nc.dram_tensor("kt_scratch", [D_OUT, CTX], F32)
    v_d = nc.dram_tensor("v_scratch", [CTX, D_OUT], F32)

    with tile.TileContext(nc) as tc:
        with tc.tile_pool(name="consts", bufs=1) as consts:
            ident = consts.tile([P, P], F32)
            make_identity(nc, ident[:])
            negmask = consts.tile([P, P], F32)
            nc.sync.dma_start(negmask[:], negmask_d[:])

            with (
                tc.tile_pool(name="xt", bufs=1) as xt_pool,
                tc.tile_pool(name="w", bufs=2) as w_pool,
                tc.tile_pool(name="xs", bufs=3) as xs_pool,
                tc.tile_pool(name="stage", bufs=4) as stage_pool,
                tc.tile_pool(name="ps_proj", bufs=4, space="PSUM") as ps_proj,
                tc.tile_pool(name="ps_tr", bufs=4, space="PSUM") as ps_tr,
            ):
                xt = xt_pool.tile([P, IC, CTX], F32)
                for st in range(ST):
                    xs = xs_pool.tile([P, D_IN], F32, tag="xs")
                    nc.sync.dma_start(xs[:], x_d[st * P : (st + 1) * P, :])
                    for ic in range(IC):
                        pt = ps_tr.tile([P, P], F32, tag="pt")
                        nc.tensor.transpose(
                            pt[:], xs[:, ic * P : (ic + 1) * P], ident[:]
                        )
                        nc.vector.tensor_copy(xt[:, ic, st * P : (st + 1) * P], pt[:])

                for w_d, dst in ((wq_d, qt_d), (wk_d, kt_d)):
                    w_sb = w_pool.tile([P, IC, D_OUT], F32, tag="w")
                    nc.sync.dma_start(
                        w_sb[:], w_d[:].rearrange("(c p) o -> p c o", p=P)
                    )
                    for s0 in range(0, CTX, MMW):
                        for oc in range(OC):
                            ps = ps_proj.tile([P, MMW], F32, tag="ps")
                            for ic in range(IC):
                                nc.tensor.matmul(
                                    ps[:],
                                    w_sb[:, ic, oc * P : (oc + 1) * P],
                                    xt[:, ic, s0 : s0 + MMW],
                                    start=(ic == 0),
                                    stop=(ic == IC - 1),
                                )
                            sg = stage_pool.tile([P, MMW], F32, tag="sg")
                            nc.vector.tensor_copy(sg[:], ps[:])
                            nc.sync.dma_start(
                                dst[oc * P : (oc + 1) * P, s0 : s0 + MMW], sg[:]
                            )

                wv_sb = w_pool.tile([P, IC, D_OUT], F32, tag="w")
                nc.sync.dma_start(
                    wv_sb[:], wv_d[:].rearrange("(c p) o -> p c o", p=P)
                )
                for st in range(ST):
                    for o0 in range(0, D_OUT, MMW):
                        ps = ps_proj.tile([P, MMW], F32, tag="ps")
                        for ic in range(IC):
                            nc.tensor.matmul(
                                ps[:],
                                xt[:, ic, st * P : (st + 1) * P],
                                wv_sb[:, ic, o0 : o0 + MMW],
                                start=(ic == 0),
                                stop=(ic == IC - 1),
                            )
                        sg = stage_pool.tile([P, MMW], F32, tag="sg")
                        nc.vector.tensor_copy(sg[:], ps[:])
                        nc.sync.dma_start(
                            v_d[st * P : (st + 1) * P, o0 : o0 + MMW], sg[:]
                        )

            with (
                tc.tile_pool(name="kt", bufs=1) as kt_pool,
                tc.tile_pool(name="v", bufs=1) as v_pool,
                tc.tile_pool(name="qtb", bufs=2) as qtb_pool,
            ):
                kt_sb = kt_pool.tile([P, OC, CTX], F32)
                for c in range(OC):
                    nc.sync.dma_start(kt_sb[:, c, :], kt_d[c * P : (c + 1) * P, :])
                v_sb = v_pool.tile([P, ST, D_OUT], F32)
                for c in range(ST):
                    nc.sync.dma_start(v_sb[:, c, :], v_d[c * P : (c + 1) * P, :])

                def qt_src(qb):
                    qt_b = qtb_pool.tile([P, OC, P], F32, tag="qtb")
                    for oc in range(OC):
                        nc.sync.dma_start(
                            qt_b[:, oc, :],
                            qt_d[oc * P : (oc + 1) * P, qb * P : (qb + 1) * P],
                        )
                    return qt_b

                _attention_phase(
                    nc, tc, ident, negmask, qt_src, kt_sb, v_sb, out_d, F32
                )

    _split_multi_waits(nc)
    return nc


def build_program_fp16():
    """fp16 build: x^T and the weights are pre-cast/pre-transposed to fp16 on
    the HOST (pure layout prep; identical round-to-nearest as a DVE cast), so
    the device only does matmuls, softmax and the P transposes."""
    nc = bass.Bass()
    xt_d = nc.declare_dram_parameter("xT16", [D_IN, CTX], F16, isOutput=False)
    wq_d = nc.declare_dram_parameter("Wq16", [D_IN, D_OUT], F16, isOutput=False)
    wk_d = nc.declare_dram_parameter("Wk16", [D_IN, D_OUT], F16, isOutput=False)
    wv_d = nc.declare_dram_parameter("Wv16", [D_IN, D_OUT], F16, isOutput=False)
    negmask_d = nc.declare_dram_parameter("negmask", [P, P], F32, isOutput=False)
    out_d = nc.declare_dram_parameter("out", [CTX, D_OUT], F32, isOutput=True)

    with tile.TileContext(nc) as tc:
        with tc.tile_pool(name="consts", bufs=1) as consts:
            ident16 = consts.tile([P, P], F16)
            make_identity(nc, ident16[:])
            negmask = consts.tile([P, P], F32)
            nc.sync.dma_start(negmask[:], negmask_d[:])

            with (
                tc.tile_pool(name="qt", bufs=1) as qt_pool,
                tc.tile_pool(name="kt", bufs=1) as kt_pool,
                tc.tile_pool(name="v", bufs=1) as v_pool,
            ):
                qt_sb = qt_pool.tile([P, OC, CTX], F16)
                kt_sb = kt_pool.tile([P, OC, CTX], F16)
                v_sb = v_pool.tile([P, ST, D_OUT], F16)

                with (
                    tc.tile_pool(name="xt", bufs=1) as xt_pool,
                    tc.tile_pool(name="w", bufs=1) as w_pool,
                    tc.tile_pool(name="ps_proj", bufs=8, space="PSUM") as ps_proj,
                ):
                    # x^T arrives per (i-chunk, 512-col s-group); group-0
                    # chunks are queued BEFORE the weight loads so the first
                    # projection group only waits for ~1MB of x^T + 2MB of Wq.
                    xt = xt_pool.tile([P, IC, CTX], F16)
                    SG = MMW // P  # stripes per s-group

                    def load_xt_group(g):
                        s0 = g * MMW
                        for ic in range(IC):
                            nc.sync.dma_start(
                                xt[:, ic, s0 : s0 + MMW],
                                xt_d[ic * P : (ic + 1) * P, s0 : s0 + MMW],
                            )

                    wq_sb = w_pool.tile([P, IC, D_OUT], F16, tag="wq")
                    nc.sync.dma_start(wq_sb[:, 0, :], wq_d[0:P, :])
                    load_xt_group(0)
                    wk_sb = w_pool.tile([P, IC, D_OUT], F16, tag="wk")
                    wv_sb = w_pool.tile([P, IC, D_OUT], F16, tag="wv")
                    for ic in range(1, IC):
                        nc.sync.dma_start(
                            wq_sb[:, ic, :], wq_d[ic * P : (ic + 1) * P, :]
                        )
                    for w_d, w_sb in ((wk_d, wk_sb), (wv_d, wv_sb)):
                        for ic in range(IC):
                            nc.sync.dma_start(
                                w_sb[:, ic, :], w_d[ic * P : (ic + 1) * P, :]
                            )

                    for g in range(ST // SG):
                        s0 = g * MMW
                        if g > 0:
                            load_xt_group(g)
                        for dst, w_sb in ((qt_sb, wq_sb), (kt_sb, wk_sb)):
                            for oc in range(OC):
                                ps = ps_proj.tile([P, MMW], F32, tag="ps")
                                for ic in range(IC):
                                    nc.tensor.matmul(
                                        ps[:],
                                        w_sb[:, ic, oc * P : (oc + 1) * P],
                                        xt[:, ic, s0 : s0 + MMW],
                                        start=(ic == 0),
                                        stop=(ic == IC - 1),
                                    )
                                nc.vector.tensor_copy(
                                    dst[:, oc, s0 : s0 + MMW], ps[:]
                                )
                        for st in range(g * SG, (g + 1) * SG):
                            for o0 in range(0, D_OUT, MMW):
                                ps = ps_proj.tile([P, MMW], F32, tag="ps")
                                for ic in range(IC):
                                    nc.tensor.matmul(
                                        ps[:],
                                        xt[:, ic, st * P : (st + 1) * P],
                                        wv_sb[:, ic, o0 : o0 + MMW],
                                        start=(ic == 0),
                                        stop=(ic == IC - 1),
                                    )
                                nc.vector.tensor_copy(
                                    v_sb[:, st, o0 : o0 + MMW], ps[:]
                                )

                def qt_src(qb):
                    return qt_sb[:, :, qb * P : (qb + 1) * P]

                _attention_phase(
                    nc, tc, ident16, negmask, qt_src, kt_sb, v_sb, out_d, F16
                )

    _split_multi_waits(nc)
    return nc


def _attention_phase_hybrid(
    nc, tc, ident16, negmask, qt16, kt16, v16, qt8, kt8, v8, out_d
):
    """Causal attention with a per-q-block dtype split: blocks < R16 run the
    fp16 path (qt16/kt16/v16), blocks >= R16 run fp8 DoubleRow matmuls
    (qt8/kt8/v8, 0.5 cyc/row).  Softmax is fp32 either way; P is built fp16,
    transposed fp16 on the PE, and cast to fp8 on the PSUM->SBUF copy for the
    fp8 blocks."""
    with (
        tc.tile_pool(name="pexp", bufs=3) as p_pool,
        tc.tile_pool(name="pexp32", bufs=3) as p32_pool,
        tc.tile_pool(name="ptr16", bufs=2) as pt16_pool,
        tc.tile_pool(name="ptr8", bufs=2) as pt8_pool,
        tc.tile_pool(name="red", bufs=3) as red_pool,
        tc.tile_pool(name="ob", bufs=3) as o_pool,
        tc.tile_pool(name="ps_s", bufs=4, space="PSUM") as ps_s,
        tc.tile_pool(name="ps_o", bufs=2, space="PSUM") as ps_o,
        tc.tile_pool(name="ps_pt", bufs=2, space="PSUM") as ps_pt,
    ):

        def emit_scores(qb):
            L = (qb + 1) * P
            ktiles = [(k0, min(MMW, L - k0)) for k0 in range(0, L, MMW)]
            nkt = len(ktiles)

            red = red_pool.tile([P, 4 * nkt + 3], F32, tag="red")
            mx = red[:, 0:nkt]
            negm = red[:, nkt : 2 * nkt]
            sm = red[:, 2 * nkt : 3 * nkt]
            scl = red[:, 3 * nkt : 4 * nkt]
            negm_all = red[:, 4 * nkt : 4 * nkt + 1]
            rsum = red[:, 4 * nkt + 1 : 4 * nkt + 2]
            rinv = red[:, 4 * nkt + 2 : 4 * nkt + 3]

            p_sb = p_pool.tile([P, CTX], F16, tag="p")
            p32_sb = p32_pool.tile([P, CTX], F32, tag="p32")
            for idx, (k0, w) in enumerate(ktiles):
                ps = ps_s.tile([P, MMW], F32, tag="ps_s")
                if qb < R16:
                    for oc in range(OC):
                        nc.tensor.matmul(
                            ps[:, :w],
                            qt16[:, oc, qb * P : (qb + 1) * P],
                            kt16[:, oc, k0 : k0 + w],
                            start=(oc == 0),
                            stop=(oc == OC - 1),
                        )
                else:
                    for c in range(OC // 2):
                        nc.tensor.matmul(
                            ps[:, :w],
                            qt8[:, 2 * c : 2 * c + 2, qb * P : (qb + 1) * P],
                            kt8[:, 2 * c : 2 * c + 2, k0 : k0 + w],
                            start=(c == 0),
                            stop=(c == OC // 2 - 1),
                            perf_mode=DR,
                        )
                if idx == nkt - 1:
                    nc.vector.tensor_add(
                        ps[:, w - P : w], ps[:, w - P : w], negmask[:]
                    )
                nc.vector.reduce_max(
                    mx[:, idx : idx + 1], ps[:, :w], axis=mybir.AxisListType.X
                )
                nc.scalar.mul(
                    negm[:, idx : idx + 1], mx[:, idx : idx + 1], -0.03125
                )
                nc.scalar.activation(
                    p32_sb[:, k0 : k0 + w],
                    ps[:, :w],
                    mybir.ActivationFunctionType.Exp,
                    bias=negm[:, idx : idx + 1],
                    scale=0.03125,
                    accum_out=sm[:, idx : idx + 1],
                )
            nc.vector.tensor_reduce(
                negm_all[:], negm[:], axis=mybir.AxisListType.X,
                op=mybir.AluOpType.min,
            )
            nc.scalar.activation(
                scl[:],
                mx[:],
                mybir.ActivationFunctionType.Exp,
                bias=negm_all[:, 0:1],
                scale=0.03125,
            )
            nc.vector.tensor_mul(sm[:], sm[:], scl[:])
            nc.vector.reduce_sum(rsum[:], sm[:], axis=mybir.AxisListType.X)
            nc.vector.reciprocal(rinv[:], rsum[:])
            for idx, (k0, w) in enumerate(ktiles):
                nc.vector.tensor_scalar_mul(
                    p_sb[:, k0 : k0 + w],
                    p32_sb[:, k0 : k0 + w],
                    scl[:, idx : idx + 1],
                )
            return {"qb": qb, "p_sb": p_sb, "rinv": rinv}

        def emit_pv(stt):
            qb = stt["qb"]
            p_sb = stt["p_sb"]
            rinv = stt["rinv"]
            nch = qb + 1
            fp8 = qb >= R16
            if fp8:
                pt_sb = pt8_pool.tile([P, ST, P], F8, tag="pt8")
            else:
                pt_sb = pt16_pool.tile([P, R16, P], F16, tag="pt16")
            for c0 in range(0, nch, 4):
                cn = min(4, nch - c0)
                ptp = ps_pt.tile([P, 4 * P], F16, tag="ptp")
                for j in range(cn):
                    kc = c0 + j
                    nc.tensor.transpose(
                        ptp[:, j * P : (j + 1) * P],
                        p_sb[:, kc * P : (kc + 1) * P],
                        ident16[:],
                    )
                nc.vector.tensor_copy(
                    pt_sb[:, c0 : c0 + cn, :], ptp[:, : cn * P]
                )

            o_sb = o_pool.tile([P, D_OUT], F32, tag="o")
            for o0 in range(0, D_OUT, MMW):
                pso = ps_o.tile([P, MMW], F32, tag="ps_o")
                if fp8:
                    npair = nch // 2
                    for c in range(npair):
                        nc.tensor.matmul(
                            pso[:],
                            pt_sb[:, 2 * c : 2 * c + 2, :],
                            v8[:, 2 * c : 2 * c + 2, o0 : o0 + MMW],
                            start=(c == 0),
                            stop=(c == npair - 1 and nch % 2 == 0),
                            perf_mode=DR,
                        )
                    if nch % 2 == 1:
                        nc.tensor.matmul(
                            pso[:],
                            pt_sb[:, nch - 1, :],
                            v8[:, nch - 1, o0 : o0 + MMW],
                            start=False,
                            stop=True,
                        )
                else:
                    for kc in range(nch):
                        nc.tensor.matmul(
                            pso[:],
                            pt_sb[:, kc, :],
                            v16[:, kc, o0 : o0 + MMW],
                            start=(kc == 0),
                            stop=(kc == nch - 1),
                        )
                nc.vector.tensor_scalar_mul(
                    o_sb[:, o0 : o0 + MMW], pso[:], rinv[:, 0:1]
                )
            nc.sync.dma_start(out_d[qb * P : (qb + 1) * P, :], o_sb[:])

        order = [3, 2, 1, 0] + list(range(QB - 1, 3, -1))
        prev = None
        for qb in order:
            stt = emit_scores(qb)
            if prev is not None:
                emit_pv(prev)
            prev = stt
        emit_pv(prev)


def build_program_fp8():
    """Hybrid fp16/fp8 build.  Rows < RS go through the fp16 pipeline
    (projections and attention), rows >= RS through fp8 DoubleRow matmuls
    (2x PE throughput).  Early K/V are cast fp16->fp8 on the DVE so late
    blocks can consume them in DoubleRow mode.  Host pre-casts x^T and the
    weights to fp16 and fp8 (pure dtype/layout prep, same round-to-nearest
    as a DVE cast)."""
    nc = bass.Bass()
    xt16_d = nc.declare_dram_parameter("xT16pre", [D_IN, RS], F16, isOutput=False)
    xt8_d = nc.declare_dram_parameter("xT8post", [D_IN, CTX - RS], F8, isOutput=False)
    wq16_d = nc.declare_dram_parameter("Wq16", [D_IN, D_OUT], F16, isOutput=False)
    wk16_d = nc.declare_dram_parameter("Wk16", [D_IN, D_OUT], F16, isOutput=False)
    wv16_d = nc.declare_dram_parameter("Wv16", [D_IN, D_OUT], F16, isOutput=False)
    wq8_d = nc.declare_dram_parameter("Wq8", [D_IN, D_OUT], F8, isOutput=False)
    wk8_d = nc.declare_dram_parameter("Wk8", [D_IN, D_OUT], F8, isOutput=False)
    wv8_d = nc.declare_dram_parameter("Wv8", [D_IN, D_OUT], F8, isOutput=False)
    negmask_d = nc.declare_dram_parameter("negmask", [P, P], F32, isOutput=False)
    out_d = nc.declare_dram_parameter("out", [CTX, D_OUT], F32, isOutput=True)

    with tile.TileContext(nc) as tc:
        with tc.tile_pool(name="consts", bufs=1) as consts:
            ident16 = consts.tile([P, P], F16)
            make_identity(nc, ident16[:])
            negmask = consts.tile([P, P], F32)
            nc.sync.dma_start(negmask[:], negmask_d[:])

            with (
                tc.tile_pool(name="qt16", bufs=1) as qt16_pool,
                tc.tile_pool(name="kt16", bufs=1) as kt16_pool,
                tc.tile_pool(name="v16", bufs=1) as v16_pool,
                tc.tile_pool(name="qt8", bufs=1) as qt8_pool,
                tc.tile_pool(name="kt8", bufs=1) as kt8_pool,
                tc.tile_pool(name="v8", bufs=1) as v8_pool,
                tc.tile_pool(name="w8", bufs=1) as w8_pool,
                tc.tile_pool(name="xt8", bufs=1) as xt8_pool,
            ):
                qt16 = qt16_pool.tile([P, OC, RS], F16)
                kt16 = kt16_pool.tile([P, OC, RS], F16)
                v16 = v16_pool.tile([P, R16, D_OUT], F16)
                qt8 = qt8_pool.tile([P, OC, CTX], F8)
                kt8 = kt8_pool.tile([P, OC, CTX], F8)
                v8 = v8_pool.tile([P, ST, D_OUT], F8)
                wq8 = w8_pool.tile([P, IC, D_OUT], F8, tag="wq8")
                wk8 = w8_pool.tile([P, IC, D_OUT], F8, tag="wk8")
                wv8 = w8_pool.tile([P, IC, D_OUT], F8, tag="wv8")
                xt8 = xt8_pool.tile([P, IC, CTX], F8)

                # ---- fp16 projections for rows < RS ----
                with (
                    tc.tile_pool(name="w16", bufs=1) as w16_pool,
                    tc.tile_pool(name="xt16", bufs=1) as xt16_pool,
                    tc.tile_pool(name="ps_p16", bufs=8, space="PSUM") as ps_p16,
                ):
                    wq16 = w16_pool.tile([P, IC, D_OUT], F16, tag="wq16")
                    wk16 = w16_pool.tile([P, IC, D_OUT], F16, tag="wk16")
                    wv16 = w16_pool.tile([P, IC, D_OUT], F16, tag="wv16")
                    xt16 = xt16_pool.tile([P, IC, RS], F16)
                    # DMA order: what the first matmuls need comes first;
                    # the fp8-phase tensors stream in behind.
                    for ic in range(IC):
                        nc.sync.dma_start(
                            xt16[:, ic, :], xt16_d[ic * P : (ic + 1) * P, :]
                        )
                    for w_d, w_sb in (
                        (wq16_d, wq16),
                        (wk16_d, wk16),
                        (wv16_d, wv16),
                    ):
                        for ic in range(IC):
                            nc.sync.dma_start(
                                w_sb[:, ic, :], w_d[ic * P : (ic + 1) * P, :]
                            )
                    for w_d, w_sb in ((wq8_d, wq8), (wk8_d, wk8), (wv8_d, wv8)):
                        for ic in range(IC):
                            nc.sync.dma_start(
                                w_sb[:, ic, :], w_d[ic * P : (ic + 1) * P, :]
                            )
                    for ic in range(IC):
                        nc.sync.dma_start(
                            xt8[:, ic, RS:CTX], xt8_d[ic * P : (ic + 1) * P, :]
                        )

                    for dst, w_sb in ((qt16, wq16), (kt16, wk16)):
                        for s0, w in ((0, MMW), (MMW, RS - MMW)):
                            for oc in range(OC):
                                ps = ps_p16.tile([P, MMW], F32, tag="ps")
                                for ic in range(IC):
                                    nc.tensor.matmul(
                                        ps[:, :w],
                                        w_sb[:, ic, oc * P : (oc + 1) * P],
                                        xt16[:, ic, s0 : s0 + w],
                                        start=(ic == 0),
                                        stop=(ic == IC - 1),
                                    )
                                nc.vector.tensor_copy(
                                    dst[:, oc, s0 : s0 + w], ps[:, :w]
                                )
                    for st in range(R16):
                        for o0 in range(0, D_OUT, MMW):
                            ps = ps_p16.tile([P, MMW], F32, tag="ps")
                            for ic in range(IC):
                                nc.tensor.matmul(
                                    ps[:],
                                    xt16[:, ic, st * P : (st + 1) * P],
                                    wv16[:, ic, o0 : o0 + MMW],
                                    start=(ic == 0),
                                    stop=(ic == IC - 1),
                                )
                            nc.vector.tensor_copy(
                                v16[:, st, o0 : o0 + MMW], ps[:]
                            )
                    # early K/V cast to fp8 for the late fp8 blocks
                    for oc in range(OC):
                        nc.vector.tensor_copy(kt8[:, oc, 0:RS], kt16[:, oc, :])
                    for st in range(R16):
                        nc.vector.tensor_copy(v8[:, st, :], v16[:, st, :])

                # ---- fp8 DoubleRow projections for rows >= RS ----
                with tc.tile_pool(name="ps_p8", bufs=8, space="PSUM") as ps_p8:
                    sgroups = []
                    s0 = RS
                    while s0 < CTX:
                        w = min(MMW, CTX - s0)
                        sgroups.append((s0, w))
                        s0 += w
                    for s0, w in sgroups:
                        for dst, w_sb in ((qt8, wq8), (kt8, wk8)):
                            for oc in range(OC):
                                ps = ps_p8.tile([P, MMW], F32, tag="ps")
                                for c in range(IC // 2):
                                    nc.tensor.matmul(
                                        ps[:, :w],
                                        w_sb[
                                            :,
                                            2 * c : 2 * c + 2,
                                            oc * P : (oc + 1) * P,
                                        ],
                                        xt8[:, 2 * c : 2 * c + 2, s0 : s0 + w],
                                        start=(c == 0),
                                        stop=(c == IC // 2 - 1),
                                        perf_mode=DR,
                                    )
                                nc.vector.tensor_copy(
                                    dst[:, oc, s0 : s0 + w], ps[:, :w]
                                )
                        for st in range(s0 // P, (s0 + w) // P):
                            for o0 in range(0, D_OUT, MMW):
                                ps = ps_p8.tile([P, MMW], F32, tag="ps")
                                for c in range(IC // 2):
                                    nc.tensor.matmul(
                                        ps[:],
                                        xt8[
                                            :,
                                            2 * c : 2 * c + 2,
                                            st * P : (st + 1) * P,
                                        ],
                                        wv8[:, 2 * c : 2 * c + 2, o0 : o0 + MMW],
                                        start=(c == 0),
                                        stop=(c == IC // 2 - 1),
                                        perf_mode=DR,
                                    )
                                nc.vector.tensor_copy(
                                    v8[:, st, o0 : o0 + MMW], ps[:]
                                )

                _attention_phase_hybrid(
                    nc, tc, ident16, negmask, qt16, kt16, v16, qt8, kt8, v8,
                    out_d,
                )

    _split_multi_waits(nc)
    return nc


_program_cache = {}


def build_program(mode=None):
    mode = mode or MODE
    if mode == "fp32":
        return build_program_fp32()
    elif mode == "fp16":
        return build_program_fp16()
    elif mode == "fp8":
        return build_program_fp8()
    raise ValueError(mode)


def make_in_maps(x, Wq, Wk, Wv):
    x = np.ascontiguousarray(np.asarray(x), dtype=np.float32)
    Wq = np.ascontiguousarray(np.asarray(Wq), dtype=np.float32)
    Wk = np.ascontiguousarray(np.asarray(Wk), dtype=np.float32)
    Wv = np.ascontiguousarray(np.asarray(Wv), dtype=np.float32)

    iu = np.triu(np.ones((P, P), dtype=np.float32), k=1)
    negmask = (iu * NEG_BIG).astype(np.float32)

    if MODE == "fp8":
        import ml_dtypes

        F8NP = ml_dtypes.float8_e4m3
        xT = np.transpose(x, (0, 2, 1))  # [b, d_in, ctx]
        xT16pre = np.ascontiguousarray(xT[:, :, :RS].astype(np.float16))
        xT8post = np.ascontiguousarray(xT[:, :, RS:].astype(F8NP))
        wq16 = np.ascontiguousarray(Wq.astype(np.float16))
        wk16 = np.ascontiguousarray(Wk.astype(np.float16))
        wv16 = np.ascontiguousarray(Wv.astype(np.float16))
        wq8 = np.ascontiguousarray(Wq.astype(F8NP))
        wk8 = np.ascontiguousarray(Wk.astype(F8NP))
        wv8 = np.ascontiguousarray(Wv.astype(F8NP))
        in_maps = [
            {
                "xT16pre": xT16pre[b],
                "xT8post": xT8post[b],
                "Wq16": wq16,
                "Wk16": wk16,
                "Wv16": wv16,
                "Wq8": wq8,
                "Wk8": wk8,
                "Wv8": wv8,
                "negmask": negmask,
            }
            for b in range(BATCH)
        ]
    elif MODE == "fp16":
        # host-side layout prep: fp16 round-to-nearest (same as a DVE cast)
        # and the x transpose the device would otherwise do on the PE
        xT16 = np.ascontiguousarray(
            np.transpose(x.astype(np.float16), (0, 2, 1))
        )
        wq16 = np.ascontiguousarray(Wq.astype(np.float16))
        wk16 = np.ascontiguousarray(Wk.astype(np.float16))
        wv16 = np.ascontiguousarray(Wv.astype(np.float16))
        in_maps = [
            {
                "xT16": xT16[b],
                "Wq16": wq16,
                "Wk16": wk16,
                "Wv16": wv16,
                "negmask": negmask,
            }
            for b in range(BATCH)
        ]
    else:
        in_maps = [
            {"x": x[b], "Wq": Wq, "Wk": Wk, "Wv": Wv, "negmask": negmask}
            for b in range(BATCH)
        ]
    return in_maps


def kernel(x, Wq, Wk, Wv):
    from concourse.bass_utils import run_bass_kernel_spmd

    if MODE not in _program_cache:
        _program_cache[MODE] = build_program(MODE)
    nc = _program_cache[MODE]

    in_maps = make_in_maps(x, Wq, Wk, Wv)
    res = run_bass_kernel_spmd(nc, in_maps, list(range(N_CORES)))
    return np.stack([res.results[b]["out"] for b in range(BATCH)], axis=0)



# revision 8
# speedup vs baseline: 1.0201x; 1.0201x over previous
"""Causal single-head attention (batch=8, ctx=2048, d=1024) on 8 trn2 cores.

Strategy: pure data-parallel over batch — core b computes attention for
batch element b with no cross-core communication.

Per-core pipeline:
  phase 1: Q^T, K^T (o-major) and V (s-major) projections accumulated in
           PSUM (fp32), consumed per 512-column s-group of x^T.
  phase 2: flash-style causal attention per 128-row q-block:
           S = Q^T.T @ K^T, additive causal mask on the diagonal
           128x128 sub-tile, one-pass softmax (per-tile exp(s - m_tile)
           on ACT with row-sum accumulators, exp(m_tile - m_all)
           correction folded into P), P transposed per tile on the PE,
           O = P @ V accumulated in PSUM, deferred normalization by the
           reciprocal row sum, DMA out (fp32); two-block software
           pipeline so PV of one block hides the next one's softmax.

MODE selects the matmul input dtype:
  "fp32": all matmul inputs fp32 (4 cyc/row); x^T built on-device via PE
          transposes; Q^T/K^T/V staged through DRAM scratch (SBUF can't
          hold x^T + all three in fp32).  ~1.25ms, rel err ~6e-6.
  "fp16": matmul inputs fp16 (1 cyc/row), fp32 PSUM accumulation and
          softmax; x^T and the weights are pre-cast/pre-transposed on the
          host (pure layout prep, bit-identical to a DVE cast) and
          everything stays resident in SBUF.  ~332us, rel err ~5e-4.
"""

import sys

sys.path.insert(0, "/opt/trn_rl_repo")

import numpy as np

import concourse.bass as bass
import concourse.mybir as mybir
import concourse.tile as tile
from concourse.masks import make_identity
from concourse.vector_clock import ScopedClock

MODE = "fp8"

BATCH = 8
CTX = 2048
D_IN = 1024
D_OUT = 1024
N_CORES = 8
P = 128
F32 = mybir.dt.float32
F16 = mybir.dt.float16
F8 = mybir.dt.float8e4
DR = mybir.MatmulPerfMode.DoubleRow
NEG_BIG = -1.0e30
R16 = 5  # q/seq blocks 0..R16-1 use the fp16 path; the rest fp8 DoubleRow
RS = R16 * P
# e4m3's normal range starts at 2^-6; the weights (std 1/32) and softmax
# probs sit mostly below it, where quantization is coarse (and the PE
# appears to flush subnormals).  Scale W by 32 on the host (so q,k,v land
# in PSUM pre-scaled by 32) and P by 64 on the device; the inverse scales
# fold into the exp logit scale and the output normalization for free.
WSCALE = 32.0  # host: W8 = e4m3(W * 32) -> q,k,v arrive x32
PSCALE = 64.0  # device: P8 = e4m3(P * 64)
SC16 = 0.03125  # logit scale for the fp16 path: 1/sqrt(1024)
SC8 = 0.03125 / (WSCALE * WSCALE)  # fp8 path: logits arrive x1024

# ---------------------------------------------------------------------------
# Workarounds for the walrus build on this stack: it rejects any instruction
# carrying more than ONE sync wait.  (1) Patch the TileContext tail drain to
# spread its waits over preceding sync-engine nops; (2) post-pass that hoists
# extra waits from any instruction onto same-engine nops inserted right
# before it (sequencers execute per-engine streams in order, so this is
# semantics-preserving).
# ---------------------------------------------------------------------------


def _patched_drain_and_barrier(self, tick_clock, wait_clock):
    nc = self.nc
    nops = [nc.sync.nop(nofuse=True) for _ in range(27)]
    drain_inst = nc.sync.drain()
    wait_clock.add_sem_waits(
        drain_inst.ins, ScopedClock({None: tick_clock.global_clock})
    )
    si = drain_inst.ins.sync_info
    if si is not None and si.on_wait is not None and len(si.on_wait) > 1:
        waits = list(si.on_wait)
        si.on_wait = waits[:1]
        rest = waits[1:]
        for i, nop in enumerate(nops):
            chunk = rest[i : i + 1]
            if not chunk:
                break
            nsi = nop.ins.sync_info
            if nsi is None:
                nop.ins.sync_info = mybir.SyncInfo(on_wait=chunk, on_update=[])
            else:
                nsi.on_wait = chunk

    nc.all_engine_barrier()
    assert self.sems is not None
    popped = nc._tile_sem_poison_stack.pop()
    assert popped is self._sem_poison
    nc.clear_and_free_semaphores(list(self.sems.allocated().values()))
    nc.all_engine_barrier()


tile.TileContext._drain_and_barrier = _patched_drain_and_barrier


def _split_multi_waits(nc):
    n_split = 0
    for f in nc.m.functions:
        for bb in f.blocks:
            il = bb.instructions
            if not any(
                inst.sync_info is not None
                and inst.sync_info.on_wait
                and len(inst.sync_info.on_wait) > 1
                for inst in il
            ):
                continue
            new = []
            for inst in il:
                si = inst.sync_info
                if si is not None and si.on_wait and len(si.on_wait) > 1:
                    waits = list(si.on_wait)
                    for w in waits[:-1]:
                        nop = mybir.InstNoOp(
                            name=nc.get_next_instruction_name(), ins=[], outs=[]
                        )
                        nop.engine = inst.engine
                        nop.sync_info = mybir.SyncInfo(on_wait=[w], on_update=[])
                        new.append(nop)
                        n_split += 1
                    si.on_wait = [waits[-1]]
                new.append(inst)
            il[:] = new
    return n_split


# ---------------------------------------------------------------------------
# Program builders
# ---------------------------------------------------------------------------

IC = D_IN // P  # 8 input-dim chunks
OC = D_OUT // P  # 8 output-dim chunks
ST = CTX // P  # 16 seq chunks
QB = CTX // P  # 16 q blocks
MMW = 512  # moving width (psum bank = 512 fp32)


def _declare_io(nc):
    x_d = nc.declare_dram_parameter("x", [CTX, D_IN], F32, isOutput=False)
    wq_d = nc.declare_dram_parameter("Wq", [D_IN, D_OUT], F32, isOutput=False)
    wk_d = nc.declare_dram_parameter("Wk", [D_IN, D_OUT], F32, isOutput=False)
    wv_d = nc.declare_dram_parameter("Wv", [D_IN, D_OUT], F32, isOutput=False)
    negmask_d = nc.declare_dram_parameter("negmask", [P, P], F32, isOutput=False)
    out_d = nc.declare_dram_parameter("out", [CTX, D_OUT], F32, isOutput=True)
    return x_d, wq_d, wk_d, wv_d, negmask_d, out_d


def _attention_phase(nc, tc, consts_ident, negmask, qt_src, kt_sb, v_sb, out_d, dt):
    """qt_src(qb) -> [P, OC, P] tile of Q^T for that block.

    One-pass softmax: each score tile gets exp(s - m_tile) immediately
    (ACT, off the PE critical path); after the block's tiles are done the
    per-tile correction exp(m_tile - m_all) is folded into the 16-bit P
    tiles and the row-sum."""
    with (
        tc.tile_pool(name="pexp", bufs=3) as p_pool,
        tc.tile_pool(name="pexp32", bufs=3) as p32_pool,
        tc.tile_pool(name="ptr", bufs=2) as pt_pool,
        tc.tile_pool(name="red", bufs=3) as red_pool,
        tc.tile_pool(name="ob", bufs=3) as o_pool,
        tc.tile_pool(name="ps_s", bufs=4, space="PSUM") as ps_s,
        tc.tile_pool(name="ps_o", bufs=2, space="PSUM") as ps_o,
        tc.tile_pool(name="ps_pt", bufs=2, space="PSUM") as ps_pt,
    ):

        def emit_scores(qb):
            L = (qb + 1) * P
            ktiles = [(k0, min(MMW, L - k0)) for k0 in range(0, L, MMW)]
            nkt = len(ktiles)

            qt_b = qt_src(qb)

            red = red_pool.tile([P, 4 * nkt + 3], F32, tag="red")
            mx = red[:, 0:nkt]
            negm = red[:, nkt : 2 * nkt]
            sm = red[:, 2 * nkt : 3 * nkt]
            scl = red[:, 3 * nkt : 4 * nkt]
            negm_all = red[:, 4 * nkt : 4 * nkt + 1]
            rsum = red[:, 4 * nkt + 1 : 4 * nkt + 2]
            rinv = red[:, 4 * nkt + 2 : 4 * nkt + 3]

            p_sb = p_pool.tile([P, CTX], dt, tag="p")
            p32_sb = p32_pool.tile([P, CTX], F32, tag="p32")
            for idx, (k0, w) in enumerate(ktiles):
                ps = ps_s.tile([P, MMW], F32, tag="ps_s")
                for oc in range(OC):
                    nc.tensor.matmul(
                        ps[:, :w],
                        qt_b[:, oc, :],
                        kt_sb[:, oc, k0 : k0 + w],
                        start=(oc == 0),
                        stop=(oc == OC - 1),
                    )
                if idx == nkt - 1:
                    nc.vector.tensor_add(
                        ps[:, w - P : w], ps[:, w - P : w], negmask[:]
                    )
                nc.vector.reduce_max(
                    mx[:, idx : idx + 1], ps[:, :w], axis=mybir.AxisListType.X
                )
                nc.scalar.mul(
                    negm[:, idx : idx + 1], mx[:, idx : idx + 1], -0.03125
                )
                # exp(s - m_tile) immediately; row-sums into sm[idx]
                nc.scalar.activation(
                    p32_sb[:, k0 : k0 + w],
                    ps[:, :w],
                    mybir.ActivationFunctionType.Exp,
                    bias=negm[:, idx : idx + 1],
                    scale=0.03125,
                    accum_out=sm[:, idx : idx + 1],
                )
            # combine: negm_all = min_idx(-m_idx/32) = -m_all/32;
            # scl_idx = exp((m_idx - m_all)/32)
            nc.vector.tensor_reduce(
                negm_all[:], negm[:], axis=mybir.AxisListType.X,
                op=mybir.AluOpType.min,
            )
            nc.scalar.activation(
                scl[:],
                mx[:],
                mybir.ActivationFunctionType.Exp,
                bias=negm_all[:, 0:1],
                scale=0.03125,
            )
            nc.vector.tensor_mul(sm[:], sm[:], scl[:])
            nc.vector.reduce_sum(rsum[:], sm[:], axis=mybir.AxisListType.X)
            nc.vector.reciprocal(rinv[:], rsum[:])
            for idx, (k0, w) in enumerate(ktiles):
                nc.vector.tensor_scalar_mul(
                    p_sb[:, k0 : k0 + w],
                    p32_sb[:, k0 : k0 + w],
                    scl[:, idx : idx + 1],
                )
            return {"qb": qb, "p_sb": p_sb, "rinv": rinv}

        def emit_pv(stt):
            qb = stt["qb"]
            p_sb = stt["p_sb"]
            rinv = stt["rinv"]
            L = (qb + 1) * P
            pt_sb = pt_pool.tile([P, L], dt, tag="pt")
            for c0 in range(0, qb + 1, 4):
                cn = min(4, qb + 1 - c0)
                ptp = ps_pt.tile([P, 4 * P], dt, tag="ptp")
                for j in range(cn):
                    kc = c0 + j
                    nc.tensor.transpose(
                        ptp[:, j * P : (j + 1) * P],
                        p_sb[:, kc * P : (kc + 1) * P],
                        consts_ident[:],
                    )
                nc.vector.tensor_copy(
                    pt_sb[:, c0 * P : c0 * P + cn * P], ptp[:, : cn * P]
                )

            o_sb = o_pool.tile([P, D_OUT], F32, tag="o")
            for o0 in range(0, D_OUT, MMW):
                pso = ps_o.tile([P, MMW], F32, tag="ps_o")
                for kc in range(qb + 1):
                    nc.tensor.matmul(
                        pso[:],
                        pt_sb[:, kc * P : (kc + 1) * P],
                        v_sb[:, kc, o0 : o0 + MMW],
                        start=(kc == 0),
                        stop=(kc == qb),
                    )
                nc.vector.tensor_scalar_mul(
                    o_sb[:, o0 : o0 + MMW], pso[:], rinv[:, 0:1]
                )
            nc.sync.dma_start(out_d[qb * P : (qb + 1) * P, :], o_sb[:])

        # two-block software pipeline: PV of the previous block hides the
        # softmax latency of the current one.  The four smallest blocks run
        # first (their PV is too short to hide a softmax), then largest-
        # first, so the exposed tail block still has a few-us PV.
        order = [3, 2, 1, 0] + list(range(QB - 1, 3, -1))
        prev = None
        for qb in order:
            stt = emit_scores(qb)
            if prev is not None:
                emit_pv(prev)
            prev = stt
        emit_pv(prev)


def build_program_fp32():
    nc = bass.Bass()
    x_d, wq_d, wk_d, wv_d, negmask_d, out_d = _declare_io(nc)
    qt_d = nc.dram_tensor("qt_scratch", [D_OUT, CTX], F32)
    kt_d = nc.dram_tensor("kt_scratch", [D_OUT, CTX], F32)
    v_d = nc.dram_tensor("v_scratch", [CTX, D_OUT], F32)

    with tile.TileContext(nc) as tc:
        with tc.tile_pool(name="consts", bufs=1) as consts:
            ident = consts.tile([P, P], F32)
            make_identity(nc, ident[:])
            negmask = consts.tile([P, P], F32)
            nc.sync.dma_start(negmask[:], negmask_d[:])

            with (
                tc.tile_pool(name="xt", bufs=1) as xt_pool,
                tc.tile_pool(name="w", bufs=2) as w_pool,
                tc.tile_pool(name="xs", bufs=3) as xs_pool,
                tc.tile_pool(name="stage", bufs=4) as stage_pool,
                tc.tile_pool(name="ps_proj", bufs=4, space="PSUM") as ps_proj,
                tc.tile_pool(name="ps_tr", bufs=4, space="PSUM") as ps_tr,
            ):
                xt = xt_pool.tile([P, IC, CTX], F32)
                for st in range(ST):
                    xs = xs_pool.tile([P, D_IN], F32, tag="xs")
                    nc.sync.dma_start(xs[:], x_d[st * P : (st + 1) * P, :])
                    for ic in range(IC):
                        pt = ps_tr.tile([P, P], F32, tag="pt")
                        nc.tensor.transpose(
                            pt[:], xs[:, ic * P : (ic + 1) * P], ident[:]
                        )
                        nc.vector.tensor_copy(xt[:, ic, st * P : (st + 1) * P], pt[:])

                for w_d, dst in ((wq_d, qt_d), (wk_d, kt_d)):
                    w_sb = w_pool.tile([P, IC, D_OUT], F32, tag="w")
                    nc.sync.dma_start(
                        w_sb[:], w_d[:].rearrange("(c p) o -> p c o", p=P)
                    )
                    for s0 in range(0, CTX, MMW):
                        for oc in range(OC):
                            ps = ps_proj.tile([P, MMW], F32, tag="ps")
                            for ic in range(IC):
                                nc.tensor.matmul(
                                    ps[:],
                                    w_sb[:, ic, oc * P : (oc + 1) * P],
                                    xt[:, ic, s0 : s0 + MMW],
                                    start=(ic == 0),
                                    stop=(ic == IC - 1),
                                )
                            sg = stage_pool.tile([P, MMW], F32, tag="sg")
                            nc.vector.tensor_copy(sg[:], ps[:])
                            nc.sync.dma_start(
                                dst[oc * P : (oc + 1) * P, s0 : s0 + MMW], sg[:]
                            )

                wv_sb = w_pool.tile([P, IC, D_OUT], F32, tag="w")
                nc.sync.dma_start(
                    wv_sb[:], wv_d[:].rearrange("(c p) o -> p c o", p=P)
                )
                for st in range(ST):
                    for o0 in range(0, D_OUT, MMW):
                        ps = ps_proj.tile([P, MMW], F32, tag="ps")
                        for ic in range(IC):
                            nc.tensor.matmul(
                                ps[:],
                                xt[:, ic, st * P : (st + 1) * P],
                                wv_sb[:, ic, o0 : o0 + MMW],
                                start=(ic == 0),
                                stop=(ic == IC - 1),
                            )
                        sg = stage_pool.tile([P, MMW], F32, tag="sg")
                        nc.vector.tensor_copy(sg[:], ps[:])
                        nc.sync.dma_start(
                            v_d[st * P : (st + 1) * P, o0 : o0 + MMW], sg[:]
                        )

            with (
                tc.tile_pool(name="kt", bufs=1) as kt_pool,
                tc.tile_pool(name="v", bufs=1) as v_pool,
                tc.tile_pool(name="qtb", bufs=2) as qtb_pool,
            ):
                kt_sb = kt_pool.tile([P, OC, CTX], F32)
                for c in range(OC):
                    nc.sync.dma_start(kt_sb[:, c, :], kt_d[c * P : (c + 1) * P, :])
                v_sb = v_pool.tile([P, ST, D_OUT], F32)
                for c in range(ST):
                    nc.sync.dma_start(v_sb[:, c, :], v_d[c * P : (c + 1) * P, :])

                def qt_src(qb):
                    qt_b = qtb_pool.tile([P, OC, P], F32, tag="qtb")
                    for oc in range(OC):
                        nc.sync.dma_start(
                            qt_b[:, oc, :],
                            qt_d[oc * P : (oc + 1) * P, qb * P : (qb + 1) * P],
                        )
                    return qt_b

                _attention_phase(
                    nc, tc, ident, negmask, qt_src, kt_sb, v_sb, out_d, F32
                )

    _split_multi_waits(nc)
    return nc


def build_program_fp16():
    """fp16 build: x^T and the weights are pre-cast/pre-transposed to fp16 on
    the HOST (pure layout prep; identical round-to-nearest as a DVE cast), so
    the device only does matmuls, softmax and the P transposes."""
    nc = bass.Bass()
    xt_d = nc.declare_dram_parameter("xT16", [D_IN, CTX], F16, isOutput=False)
    wq_d = nc.declare_dram_parameter("Wq16", [D_IN, D_OUT], F16, isOutput=False)
    wk_d = nc.declare_dram_parameter("Wk16", [D_IN, D_OUT], F16, isOutput=False)
    wv_d = nc.declare_dram_parameter("Wv16", [D_IN, D_OUT], F16, isOutput=False)
    negmask_d = nc.declare_dram_parameter("negmask", [P, P], F32, isOutput=False)
    out_d = nc.declare_dram_parameter("out", [CTX, D_OUT], F32, isOutput=True)

    with tile.TileContext(nc) as tc:
        with tc.tile_pool(name="consts", bufs=1) as consts:
            ident16 = consts.tile([P, P], F16)
            make_identity(nc, ident16[:])
            negmask = consts.tile([P, P], F32)
            nc.sync.dma_start(negmask[:], negmask_d[:])

            with (
                tc.tile_pool(name="qt", bufs=1) as qt_pool,
                tc.tile_pool(name="kt", bufs=1) as kt_pool,
                tc.tile_pool(name="v", bufs=1) as v_pool,
            ):
                qt_sb = qt_pool.tile([P, OC, CTX], F16)
                kt_sb = kt_pool.tile([P, OC, CTX], F16)
                v_sb = v_pool.tile([P, ST, D_OUT], F16)

                with (
                    tc.tile_pool(name="xt", bufs=1) as xt_pool,
                    tc.tile_pool(name="w", bufs=1) as w_pool,
                    tc.tile_pool(name="ps_proj", bufs=8, space="PSUM") as ps_proj,
                ):
                    # x^T arrives per (i-chunk, 512-col s-group); group-0
                    # chunks are queued BEFORE the weight loads so the first
                    # projection group only waits for ~1MB of x^T + 2MB of Wq.
                    xt = xt_pool.tile([P, IC, CTX], F16)
                    SG = MMW // P  # stripes per s-group

                    def load_xt_group(g):
                        s0 = g * MMW
                        for ic in range(IC):
                            nc.sync.dma_start(
                                xt[:, ic, s0 : s0 + MMW],
                                xt_d[ic * P : (ic + 1) * P, s0 : s0 + MMW],
                            )

                    wq_sb = w_pool.tile([P, IC, D_OUT], F16, tag="wq")
                    nc.sync.dma_start(wq_sb[:, 0, :], wq_d[0:P, :])
                    load_xt_group(0)
                    wk_sb = w_pool.tile([P, IC, D_OUT], F16, tag="wk")
                    wv_sb = w_pool.tile([P, IC, D_OUT], F16, tag="wv")
                    for ic in range(1, IC):
                        nc.sync.dma_start(
                            wq_sb[:, ic, :], wq_d[ic * P : (ic + 1) * P, :]
                        )
                    for w_d, w_sb in ((wk_d, wk_sb), (wv_d, wv_sb)):
                        for ic in range(IC):
                            nc.sync.dma_start(
                                w_sb[:, ic, :], w_d[ic * P : (ic + 1) * P, :]
                            )

                    for g in range(ST // SG):
                        s0 = g * MMW
                        if g > 0:
                            load_xt_group(g)
                        for dst, w_sb in ((qt_sb, wq_sb), (kt_sb, wk_sb)):
                            for oc in range(OC):
                                ps = ps_proj.tile([P, MMW], F32, tag="ps")
                                for ic in range(IC):
                                    nc.tensor.matmul(
                                        ps[:],
                                        w_sb[:, ic, oc * P : (oc + 1) * P],
                                        xt[:, ic, s0 : s0 + MMW],
                                        start=(ic == 0),
                                        stop=(ic == IC - 1),
                                    )
                                nc.vector.tensor_copy(
                                    dst[:, oc, s0 : s0 + MMW], ps[:]
                                )
                        for st in range(g * SG, (g + 1) * SG):
                            for o0 in range(0, D_OUT, MMW):
                                ps = ps_proj.tile([P, MMW], F32, tag="ps")
                                for ic in range(IC):
                                    nc.tensor.matmul(
                                        ps[:],
                                        xt[:, ic, st * P : (st + 1) * P],
                                        wv_sb[:, ic, o0 : o0 + MMW],
                                        start=(ic == 0),
                                        stop=(ic == IC - 1),
                                    )
                                nc.vector.tensor_copy(
                                    v_sb[:, st, o0 : o0 + MMW], ps[:]
                                )

                def qt_src(qb):
                    return qt_sb[:, :, qb * P : (qb + 1) * P]

                _attention_phase(
                    nc, tc, ident16, negmask, qt_src, kt_sb, v_sb, out_d, F16
                )

    _split_multi_waits(nc)
    return nc


def _attention_phase_hybrid(
    nc, tc, ident16, negmask, qt16, kt16, v16, qt8, kt8, v8, out_d
):
    """Causal attention with a per-q-block dtype split: blocks < R16 run the
    fp16 path (qt16/kt16/v16), blocks >= R16 run fp8 DoubleRow matmuls
    (qt8/kt8/v8, 0.5 cyc/row).  Softmax is fp32 either way; P is built fp16,
    transposed fp16 on the PE, and cast to fp8 on the PSUM->SBUF copy for the
    fp8 blocks."""
    with (
        tc.tile_pool(name="pexp", bufs=3) as p_pool,
        tc.tile_pool(name="pexp32", bufs=3) as p32_pool,
        tc.tile_pool(name="ptr16", bufs=2) as pt16_pool,
        tc.tile_pool(name="ptr8", bufs=2) as pt8_pool,
        tc.tile_pool(name="red", bufs=3) as red_pool,
        tc.tile_pool(name="ob", bufs=3) as o_pool,
        tc.tile_pool(name="ps_s", bufs=4, space="PSUM") as ps_s,
        tc.tile_pool(name="ps_o", bufs=2, space="PSUM") as ps_o,
        tc.tile_pool(name="ps_pt", bufs=2, space="PSUM") as ps_pt,
    ):

        def emit_scores(qb):
            L = (qb + 1) * P
            ktiles = [(k0, min(MMW, L - k0)) for k0 in range(0, L, MMW)]
            nkt = len(ktiles)
            fp8 = qb >= R16
            sc = SC8 if fp8 else SC16

            red = red_pool.tile([P, 4 * nkt + 3], F32, tag="red")
            mx = red[:, 0:nkt]
            negm = red[:, nkt : 2 * nkt]
            sm = red[:, 2 * nkt : 3 * nkt]
            scl = red[:, 3 * nkt : 4 * nkt]
            negm_all = red[:, 4 * nkt : 4 * nkt + 1]
            rsum = red[:, 4 * nkt + 1 : 4 * nkt + 2]
            rinv = red[:, 4 * nkt + 2 : 4 * nkt + 3]

            p_sb = p_pool.tile([P, CTX], F16, tag="p")
            p32_sb = p32_pool.tile([P, CTX], F32, tag="p32")
            for idx, (k0, w) in enumerate(ktiles):
                ps = ps_s.tile([P, MMW], F32, tag="ps_s")
                if not fp8:
                    for oc in range(OC):
                        nc.tensor.matmul(
                            ps[:, :w],
                            qt16[:, oc, qb * P : (qb + 1) * P],
                            kt16[:, oc, k0 : k0 + w],
                            start=(oc == 0),
                            stop=(oc == OC - 1),
                        )
                else:
                    for c in range(OC // 2):
                        nc.tensor.matmul(
                            ps[:, :w],
                            qt8[:, 2 * c : 2 * c + 2, qb * P : (qb + 1) * P],
                            kt8[:, 2 * c : 2 * c + 2, k0 : k0 + w],
                            start=(c == 0),
                            stop=(c == OC // 2 - 1),
                            perf_mode=DR,
                        )
                if idx == nkt - 1:
                    nc.vector.tensor_add(
                        ps[:, w - P : w], ps[:, w - P : w], negmask[:]
                    )
                nc.vector.reduce_max(
                    mx[:, idx : idx + 1], ps[:, :w], axis=mybir.AxisListType.X
                )
                nc.scalar.mul(
                    negm[:, idx : idx + 1], mx[:, idx : idx + 1], -sc
                )
                nc.scalar.activation(
                    p32_sb[:, k0 : k0 + w],
                    ps[:, :w],
                    mybir.ActivationFunctionType.Exp,
                    bias=negm[:, idx : idx + 1],
                    scale=sc,
                    accum_out=sm[:, idx : idx + 1],
                )
            nc.vector.tensor_reduce(
                negm_all[:], negm[:], axis=mybir.AxisListType.X,
                op=mybir.AluOpType.min,
            )
            nc.scalar.activation(
                scl[:],
                mx[:],
                mybir.ActivationFunctionType.Exp,
                bias=negm_all[:, 0:1],
                scale=sc,
            )
            if fp8:
                # P is built x64 so its mass clears e4m3's subnormal range;
                # the row sum picks up the same factor, and with v arriving
                # x32 the output normalization needs a further 1/32.
                nc.scalar.mul(scl[:], scl[:], PSCALE)
            nc.vector.tensor_mul(sm[:], sm[:], scl[:])
            nc.vector.reduce_sum(rsum[:], sm[:], axis=mybir.AxisListType.X)
            nc.vector.reciprocal(rinv[:], rsum[:])
            if fp8:
                nc.scalar.mul(rinv[:], rinv[:], 1.0 / WSCALE)
            for idx, (k0, w) in enumerate(ktiles):
                nc.vector.tensor_scalar_mul(
                    p_sb[:, k0 : k0 + w],
                    p32_sb[:, k0 : k0 + w],
                    scl[:, idx : idx + 1],
                )
            return {"qb": qb, "p_sb": p_sb, "rinv": rinv}

        def emit_pv(stt):
            qb = stt["qb"]
            p_sb = stt["p_sb"]
            rinv = stt["rinv"]
            nch = qb + 1
            fp8 = qb >= R16
            if fp8:
                pt_sb = pt8_pool.tile([P, ST, P], F8, tag="pt8")
            else:
                pt_sb = pt16_pool.tile([P, R16, P], F16, tag="pt16")
            for c0 in range(0, nch, 4):
                cn = min(4, nch - c0)
                ptp = ps_pt.tile([P, 4 * P], F16, tag="ptp")
                for j in range(cn):
                    kc = c0 + j
                    nc.tensor.transpose(
                        ptp[:, j * P : (j + 1) * P],
                        p_sb[:, kc * P : (kc + 1) * P],
                        ident16[:],
                    )
                nc.vector.tensor_copy(
                    pt_sb[:, c0 : c0 + cn, :], ptp[:, : cn * P]
                )

            o_sb = o_pool.tile([P, D_OUT], F32, tag="o")
            for o0 in range(0, D_OUT, MMW):
                pso = ps_o.tile([P, MMW], F32, tag="ps_o")
                if fp8:
                    npair = nch // 2
                    for c in range(npair):
                        nc.tensor.matmul(
                            pso[:],
                            pt_sb[:, 2 * c : 2 * c + 2, :],
                            v8[:, 2 * c : 2 * c + 2, o0 : o0 + MMW],
                            start=(c == 0),
                            stop=(c == npair - 1 and nch % 2 == 0),
                            perf_mode=DR,
                        )
                    if nch % 2 == 1:
                        nc.tensor.matmul(
                            pso[:],
                            pt_sb[:, nch - 1, :],
                            v8[:, nch - 1, o0 : o0 + MMW],
                            start=False,
                            stop=True,
                        )
                else:
                    for kc in range(nch):
                        nc.tensor.matmul(
                            pso[:],
                            pt_sb[:, kc, :],
                            v16[:, kc, o0 : o0 + MMW],
                            start=(kc == 0),
                            stop=(kc == nch - 1),
                        )
                nc.vector.tensor_scalar_mul(
                    o_sb[:, o0 : o0 + MMW], pso[:], rinv[:, 0:1]
                )
            nc.sync.dma_start(out_d[qb * P : (qb + 1) * P, :], o_sb[:])

        order = [3, 2, 1, 0] + list(range(QB - 1, 3, -1))
        prev = None
        for qb in order:
            stt = emit_scores(qb)
            if prev is not None:
                emit_pv(prev)
            prev = stt
        emit_pv(prev)


def build_program_fp8():
    """Hybrid fp16/fp8 build.  Rows < RS go through the fp16 pipeline
    (projections and attention), rows >= RS through fp8 DoubleRow matmuls
    (2x PE throughput).  Early K/V are cast fp16->fp8 on the DVE so late
    blocks can consume them in DoubleRow mode.  Host pre-casts x^T and the
    weights to fp16 and fp8 (pure dtype/layout prep, same round-to-nearest
    as a DVE cast)."""
    nc = bass.Bass()
    xt16_d = nc.declare_dram_parameter("xT16pre", [D_IN, RS], F16, isOutput=False)
    xt8_d = nc.declare_dram_parameter("xT8post", [D_IN, CTX - RS], F8, isOutput=False)
    wq16_d = nc.declare_dram_parameter("Wq16", [D_IN, D_OUT], F16, isOutput=False)
    wk16_d = nc.declare_dram_parameter("Wk16", [D_IN, D_OUT], F16, isOutput=False)
    wv16_d = nc.declare_dram_parameter("Wv16", [D_IN, D_OUT], F16, isOutput=False)
    wq8_d = nc.declare_dram_parameter("Wq8", [D_IN, D_OUT], F8, isOutput=False)
    wk8_d = nc.declare_dram_parameter("Wk8", [D_IN, D_OUT], F8, isOutput=False)
    wv8_d = nc.declare_dram_parameter("Wv8", [D_IN, D_OUT], F8, isOutput=False)
    negmask_d = nc.declare_dram_parameter("negmask", [P, P], F32, isOutput=False)
    out_d = nc.declare_dram_parameter("out", [CTX, D_OUT], F32, isOutput=True)

    with tile.TileContext(nc) as tc:
        with tc.tile_pool(name="consts", bufs=1) as consts:
            ident16 = consts.tile([P, P], F16)
            make_identity(nc, ident16[:])
            negmask = consts.tile([P, P], F32)
            nc.sync.dma_start(negmask[:], negmask_d[:])

            with (
                tc.tile_pool(name="qt16", bufs=1) as qt16_pool,
                tc.tile_pool(name="kt16", bufs=1) as kt16_pool,
                tc.tile_pool(name="v16", bufs=1) as v16_pool,
                tc.tile_pool(name="qt8", bufs=1) as qt8_pool,
                tc.tile_pool(name="kt8", bufs=1) as kt8_pool,
                tc.tile_pool(name="v8", bufs=1) as v8_pool,
                tc.tile_pool(name="w8", bufs=1) as w8_pool,
                tc.tile_pool(name="xt8", bufs=1) as xt8_pool,
            ):
                qt16 = qt16_pool.tile([P, OC, RS], F16)
                kt16 = kt16_pool.tile([P, OC, RS], F16)
                v16 = v16_pool.tile([P, R16, D_OUT], F16)
                qt8 = qt8_pool.tile([P, OC, CTX], F8)
                kt8 = kt8_pool.tile([P, OC, CTX], F8)
                v8 = v8_pool.tile([P, ST, D_OUT], F8)
                wq8 = w8_pool.tile([P, IC, D_OUT], F8, tag="wq8")
                wk8 = w8_pool.tile([P, IC, D_OUT], F8, tag="wk8")
                wv8 = w8_pool.tile([P, IC, D_OUT], F8, tag="wv8")
                xt8 = xt8_pool.tile([P, IC, CTX], F8)

                # ---- fp16 projections for rows < RS ----
                with (
                    tc.tile_pool(name="w16", bufs=1) as w16_pool,
                    tc.tile_pool(name="xt16", bufs=1) as xt16_pool,
                    tc.tile_pool(name="ps_p16", bufs=8, space="PSUM") as ps_p16,
                ):
                    wq16 = w16_pool.tile([P, IC, D_OUT], F16, tag="wq16")
                    wk16 = w16_pool.tile([P, IC, D_OUT], F16, tag="wk16")
                    wv16 = w16_pool.tile([P, IC, D_OUT], F16, tag="wv16")
                    xt16 = xt16_pool.tile([P, IC, RS], F16)
                    # DMA order: what the first matmuls need comes first;
                    # the fp8-phase tensors stream in behind.
                    for ic in range(IC):
                        nc.sync.dma_start(
                            xt16[:, ic, :], xt16_d[ic * P : (ic + 1) * P, :]
                        )
                    for w_d, w_sb in (
                        (wq16_d, wq16),
                        (wk16_d, wk16),
                        (wv16_d, wv16),
                    ):
                        for ic in range(IC):
                            nc.sync.dma_start(
                                w_sb[:, ic, :], w_d[ic * P : (ic + 1) * P, :]
                            )
                    for w_d, w_sb in ((wq8_d, wq8), (wk8_d, wk8), (wv8_d, wv8)):
                        for ic in range(IC):
                            nc.sync.dma_start(
                                w_sb[:, ic, :], w_d[ic * P : (ic + 1) * P, :]
                            )
                    for ic in range(IC):
                        nc.sync.dma_start(
                            xt8[:, ic, RS:CTX], xt8_d[ic * P : (ic + 1) * P, :]
                        )

                    for dst, w_sb in ((qt16, wq16), (kt16, wk16)):
                        for s0, w in ((0, MMW), (MMW, RS - MMW)):
                            for oc in range(OC):
                                ps = ps_p16.tile([P, MMW], F32, tag="ps")
                                for ic in range(IC):
                                    nc.tensor.matmul(
                                        ps[:, :w],
                                        w_sb[:, ic, oc * P : (oc + 1) * P],
                                        xt16[:, ic, s0 : s0 + w],
                                        start=(ic == 0),
                                        stop=(ic == IC - 1),
                                    )
                                nc.vector.tensor_copy(
                                    dst[:, oc, s0 : s0 + w], ps[:, :w]
                                )
                    for st in range(R16):
                        for o0 in range(0, D_OUT, MMW):
                            ps = ps_p16.tile([P, MMW], F32, tag="ps")
                            for ic in range(IC):
                                nc.tensor.matmul(
                                    ps[:],
                                    xt16[:, ic, st * P : (st + 1) * P],
                                    wv16[:, ic, o0 : o0 + MMW],
                                    start=(ic == 0),
                                    stop=(ic == IC - 1),
                                )
                            nc.vector.tensor_copy(
                                v16[:, st, o0 : o0 + MMW], ps[:]
                            )
                    # early K/V cast to fp8 (x32, matching the scaled fp8
                    # projections) for the late fp8 blocks
                    for oc in range(OC):
                        nc.scalar.mul(kt8[:, oc, 0:RS], kt16[:, oc, :], WSCALE)
                    for st in range(R16):
                        nc.scalar.mul(v8[:, st, :], v16[:, st, :], WSCALE)

                # ---- fp8 DoubleRow projections for rows >= RS ----
                with tc.tile_pool(name="ps_p8", bufs=8, space="PSUM") as ps_p8:
                    sgroups = []
                    s0 = RS
                    while s0 < CTX:
                        w = min(MMW, CTX - s0)
                        sgroups.append((s0, w))
                        s0 += w
                    for s0, w in sgroups:
                        for dst, w_sb in ((qt8, wq8), (kt8, wk8)):
                            for oc in range(OC):
                                ps = ps_p8.tile([P, MMW], F32, tag="ps")
                                for c in range(IC // 2):
                                    nc.tensor.matmul(
                                        ps[:, :w],
                                        w_sb[
                                            :,
                                            2 * c : 2 * c + 2,
                                            oc * P : (oc + 1) * P,
                                        ],
                                        xt8[:, 2 * c : 2 * c + 2, s0 : s0 + w],
                                        start=(c == 0),
                                        stop=(c == IC // 2 - 1),
                                        perf_mode=DR,
                                    )
                                nc.vector.tensor_copy(
                                    dst[:, oc, s0 : s0 + w], ps[:, :w]
                                )
                        for st in range(s0 // P, (s0 + w) // P):
                            for o0 in range(0, D_OUT, MMW):
                                ps = ps_p8.tile([P, MMW], F32, tag="ps")
                                for c in range(IC // 2):
                                    nc.tensor.matmul(
                                        ps[:],
                                        xt8[
                                            :,
                                            2 * c : 2 * c + 2,
                                            st * P : (st + 1) * P,
                                        ],
                                        wv8[:, 2 * c : 2 * c + 2, o0 : o0 + MMW],
                                        start=(c == 0),
                                        stop=(c == IC // 2 - 1),
                                        perf_mode=DR,
                                    )
                                nc.vector.tensor_copy(
                                    v8[:, st, o0 : o0 + MMW], ps[:]
                                )

                _attention_phase_hybrid(
                    nc, tc, ident16, negmask, qt16, kt16, v16, qt8, kt8, v8,
                    out_d,
                )

    _split_multi_waits(nc)
    return nc


_program_cache = {}


def build_program(mode=None):
    mode = mode or MODE
    if mode == "fp32":
        return build_program_fp32()
    elif mode == "fp16":
        return build_program_fp16()
    elif mode == "fp8":
        return build_program_fp8()
    raise ValueError(mode)


def make_in_maps(x, Wq, Wk, Wv):
    x = np.ascontiguousarray(np.asarray(x), dtype=np.float32)
    Wq = np.ascontiguousarray(np.asarray(Wq), dtype=np.float32)
    Wk = np.ascontiguousarray(np.asarray(Wk), dtype=np.float32)
    Wv = np.ascontiguousarray(np.asarray(Wv), dtype=np.float32)

    iu = np.triu(np.ones((P, P), dtype=np.float32), k=1)
    negmask = (iu * NEG_BIG).astype(np.float32)

    if MODE == "fp8":
        import ml_dtypes

        F8NP = ml_dtypes.float8_e4m3
        xT = np.transpose(x, (0, 2, 1))  # [b, d_in, ctx]
        xT16pre = np.ascontiguousarray(xT[:, :, :RS].astype(np.float16))
        xT8post = np.ascontiguousarray(xT[:, :, RS:].astype(F8NP))
        wq16 = np.ascontiguousarray(Wq.astype(np.float16))
        wk16 = np.ascontiguousarray(Wk.astype(np.float16))
        wv16 = np.ascontiguousarray(Wv.astype(np.float16))
        ws = np.float32(WSCALE)
        wq8 = np.ascontiguousarray((Wq * ws).astype(F8NP))
        wk8 = np.ascontiguousarray((Wk * ws).astype(F8NP))
        wv8 = np.ascontiguousarray((Wv * ws).astype(F8NP))
        in_maps = [
            {
                "xT16pre": xT16pre[b],
                "xT8post": xT8post[b],
                "Wq16": wq16,
                "Wk16": wk16,
                "Wv16": wv16,
                "Wq8": wq8,
                "Wk8": wk8,
                "Wv8": wv8,
                "negmask": negmask,
            }
            for b in range(BATCH)
        ]
    elif MODE == "fp16":
        # host-side layout prep: fp16 round-to-nearest (same as a DVE cast)
        # and the x transpose the device would otherwise do on the PE
        xT16 = np.ascontiguousarray(
            np.transpose(x.astype(np.float16), (0, 2, 1))
        )
        wq16 = np.ascontiguousarray(Wq.astype(np.float16))
        wk16 = np.ascontiguousarray(Wk.astype(np.float16))
        wv16 = np.ascontiguousarray(Wv.astype(np.float16))
        in_maps = [
            {
                "xT16": xT16[b],
                "Wq16": wq16,
                "Wk16": wk16,
                "Wv16": wv16,
                "negmask": negmask,
            }
            for b in range(BATCH)
        ]
    else:
        in_maps = [
            {"x": x[b], "Wq": Wq, "Wk": Wk, "Wv": Wv, "negmask": negmask}
            for b in range(BATCH)
        ]
    return in_maps


def kernel(x, Wq, Wk, Wv):
    from concourse.bass_utils import run_bass_kernel_spmd

    if MODE not in _program_cache:
        _program_cache[MODE] = build_program(MODE)
    nc = _program_cache[MODE]

    in_maps = make_in_maps(x, Wq, Wk, Wv)
    res = run_bass_kernel_spmd(nc, in_maps, list(range(N_CORES)))
    return np.stack([res.results[b]["out"] for b in range(BATCH)], axis=0)



# revision 17
# speedup vs baseline: 1.0551x; 1.0343x over previous
"""Causal single-head attention (batch=8, ctx=2048, d=1024) on 8 trn2 cores.

Strategy: pure data-parallel over batch — core b computes attention for
batch element b with no cross-core communication.

Per-core pipeline:
  phase 1: Q^T, K^T (o-major) and V (s-major) projections accumulated in
           PSUM (fp32), consumed per 512-column s-group of x^T.
  phase 2: flash-style causal attention per 128-row q-block:
           S = Q^T.T @ K^T, additive causal mask on the diagonal
           128x128 sub-tile, one-pass softmax (per-tile exp(s - m_tile)
           on ACT with row-sum accumulators, exp(m_tile - m_all)
           correction folded into P), P transposed per tile on the PE,
           O = P @ V accumulated in PSUM, deferred normalization by the
           reciprocal row sum, DMA out (fp32); two-block software
           pipeline so PV of one block hides the next one's softmax.

MODE selects the matmul input dtype:
  "fp32": all matmul inputs fp32 (4 cyc/row); x^T built on-device via PE
          transposes; Q^T/K^T/V staged through DRAM scratch (SBUF can't
          hold x^T + all three in fp32).  ~1.25ms, rel err ~6e-6.
  "fp16": matmul inputs fp16 (1 cyc/row), fp32 PSUM accumulation and
          softmax; x^T and the weights are pre-cast/pre-transposed on the
          host (pure layout prep, bit-identical to a DVE cast) and
          everything stays resident in SBUF.  ~332us, rel err ~5e-4.
"""

import sys

sys.path.insert(0, "/opt/trn_rl_repo")

import numpy as np

import concourse.bass as bass
import concourse.mybir as mybir
import concourse.tile as tile
from concourse.masks import make_identity
from concourse.vector_clock import ScopedClock

MODE = "fp8"

BATCH = 8
CTX = 2048
D_IN = 1024
D_OUT = 1024
N_CORES = 8
P = 128
F32 = mybir.dt.float32
F16 = mybir.dt.float16
F8 = mybir.dt.float8e4
DR = mybir.MatmulPerfMode.DoubleRow
NEG_BIG = -1.0e30
R16 = 5  # q/seq blocks 0..R16-1 use the fp16 path; the rest fp8 DoubleRow
RS = R16 * P
# e4m3's normal range starts at 2^-6; the weights (std 1/32) and softmax
# probs sit mostly below it, where quantization is coarse (and the PE
# appears to flush subnormals).  Scale W by 32 on the host (so q,k,v land
# in PSUM pre-scaled by 32) and P by 64 on the device; the inverse scales
# fold into the exp logit scale and the output normalization for free.
WSCALE = 32.0  # host: W8 = e4m3(W * 32) -> q,k,v arrive x32
PSCALE = 64.0  # device: P8 = e4m3(P * 64)
SC16 = 0.03125  # logit scale for the fp16 path: 1/sqrt(1024)
SC8 = 0.03125 / (WSCALE * WSCALE)  # fp8 path: logits arrive x1024
DEBUG_DUMP = False  # extra DRAM outputs for per-stage error attribution
DEBUG_QB = 6

# ---------------------------------------------------------------------------
# Workarounds for the walrus build on this stack: it rejects any instruction
# carrying more than ONE sync wait.  (1) Patch the TileContext tail drain to
# spread its waits over preceding sync-engine nops; (2) post-pass that hoists
# extra waits from any instruction onto same-engine nops inserted right
# before it (sequencers execute per-engine streams in order, so this is
# semantics-preserving).
# ---------------------------------------------------------------------------


def _patched_drain_and_barrier(self, tick_clock, wait_clock):
    nc = self.nc
    nops = [nc.sync.nop(nofuse=True) for _ in range(27)]
    drain_inst = nc.sync.drain()
    wait_clock.add_sem_waits(
        drain_inst.ins, ScopedClock({None: tick_clock.global_clock})
    )
    si = drain_inst.ins.sync_info
    if si is not None and si.on_wait is not None and len(si.on_wait) > 1:
        waits = list(si.on_wait)
        si.on_wait = waits[:1]
        rest = waits[1:]
        for i, nop in enumerate(nops):
            chunk = rest[i : i + 1]
            if not chunk:
                break
            nsi = nop.ins.sync_info
            if nsi is None:
                nop.ins.sync_info = mybir.SyncInfo(on_wait=chunk, on_update=[])
            else:
                nsi.on_wait = chunk

    nc.all_engine_barrier()
    assert self.sems is not None
    popped = nc._tile_sem_poison_stack.pop()
    assert popped is self._sem_poison
    nc.clear_and_free_semaphores(list(self.sems.allocated().values()))
    nc.all_engine_barrier()


tile.TileContext._drain_and_barrier = _patched_drain_and_barrier


def _split_multi_waits(nc):
    n_split = 0
    for f in nc.m.functions:
        for bb in f.blocks:
            il = bb.instructions
            if not any(
                inst.sync_info is not None
                and inst.sync_info.on_wait
                and len(inst.sync_info.on_wait) > 1
                for inst in il
            ):
                continue
            new = []
            for inst in il:
                si = inst.sync_info
                if si is not None and si.on_wait and len(si.on_wait) > 1:
                    waits = list(si.on_wait)
                    for w in waits[:-1]:
                        nop = mybir.InstNoOp(
                            name=nc.get_next_instruction_name(), ins=[], outs=[]
                        )
                        nop.engine = inst.engine
                        nop.sync_info = mybir.SyncInfo(on_wait=[w], on_update=[])
                        new.append(nop)
                        n_split += 1
                    si.on_wait = [waits[-1]]
                new.append(inst)
            il[:] = new
    return n_split


# ---------------------------------------------------------------------------
# Program builders
# ---------------------------------------------------------------------------

IC = D_IN // P  # 8 input-dim chunks
OC = D_OUT // P  # 8 output-dim chunks
ST = CTX // P  # 16 seq chunks
QB = CTX // P  # 16 q blocks
MMW = 512  # moving width (psum bank = 512 fp32)


def _declare_io(nc):
    x_d = nc.declare_dram_parameter("x", [CTX, D_IN], F32, isOutput=False)
    wq_d = nc.declare_dram_parameter("Wq", [D_IN, D_OUT], F32, isOutput=False)
    wk_d = nc.declare_dram_parameter("Wk", [D_IN, D_OUT], F32, isOutput=False)
    wv_d = nc.declare_dram_parameter("Wv", [D_IN, D_OUT], F32, isOutput=False)
    negmask_d = nc.declare_dram_parameter("negmask", [P, P], F32, isOutput=False)
    out_d = nc.declare_dram_parameter("out", [CTX, D_OUT], F32, isOutput=True)
    return x_d, wq_d, wk_d, wv_d, negmask_d, out_d


def _attention_phase(nc, tc, consts_ident, negmask, qt_src, kt_sb, v_sb, out_d, dt):
    """qt_src(qb) -> [P, OC, P] tile of Q^T for that block.

    One-pass softmax: each score tile gets exp(s - m_tile) immediately
    (ACT, off the PE critical path); after the block's tiles are done the
    per-tile correction exp(m_tile - m_all) is folded into the 16-bit P
    tiles and the row-sum."""
    with (
        tc.tile_pool(name="pexp", bufs=3) as p_pool,
        tc.tile_pool(name="pexp32", bufs=3) as p32_pool,
        tc.tile_pool(name="ptr", bufs=2) as pt_pool,
        tc.tile_pool(name="red", bufs=3) as red_pool,
        tc.tile_pool(name="ob", bufs=3) as o_pool,
        tc.tile_pool(name="ps_s", bufs=4, space="PSUM") as ps_s,
        tc.tile_pool(name="ps_o", bufs=2, space="PSUM") as ps_o,
        tc.tile_pool(name="ps_pt", bufs=2, space="PSUM") as ps_pt,
    ):

        def emit_scores(qb):
            L = (qb + 1) * P
            ktiles = [(k0, min(MMW, L - k0)) for k0 in range(0, L, MMW)]
            nkt = len(ktiles)

            qt_b = qt_src(qb)

            red = red_pool.tile([P, 4 * nkt + 3], F32, tag="red")
            mx = red[:, 0:nkt]
            negm = red[:, nkt : 2 * nkt]
            sm = red[:, 2 * nkt : 3 * nkt]
            scl = red[:, 3 * nkt : 4 * nkt]
            negm_all = red[:, 4 * nkt : 4 * nkt + 1]
            rsum = red[:, 4 * nkt + 1 : 4 * nkt + 2]
            rinv = red[:, 4 * nkt + 2 : 4 * nkt + 3]

            p_sb = p_pool.tile([P, CTX], dt, tag="p")
            p32_sb = p32_pool.tile([P, CTX], F32, tag="p32")
            for idx, (k0, w) in enumerate(ktiles):
                ps = ps_s.tile([P, MMW], F32, tag="ps_s")
                for oc in range(OC):
                    nc.tensor.matmul(
                        ps[:, :w],
                        qt_b[:, oc, :],
                        kt_sb[:, oc, k0 : k0 + w],
                        start=(oc == 0),
                        stop=(oc == OC - 1),
                    )
                if idx == nkt - 1:
                    nc.vector.tensor_add(
                        ps[:, w - P : w], ps[:, w - P : w], negmask[:]
                    )
                nc.vector.reduce_max(
                    mx[:, idx : idx + 1], ps[:, :w], axis=mybir.AxisListType.X
                )
                nc.scalar.mul(
                    negm[:, idx : idx + 1], mx[:, idx : idx + 1], -0.03125
                )
                # exp(s - m_tile) immediately; row-sums into sm[idx]
                nc.scalar.activation(
                    p32_sb[:, k0 : k0 + w],
                    ps[:, :w],
                    mybir.ActivationFunctionType.Exp,
                    bias=negm[:, idx : idx + 1],
                    scale=0.03125,
                    accum_out=sm[:, idx : idx + 1],
                )
            # combine: negm_all = min_idx(-m_idx/32) = -m_all/32;
            # scl_idx = exp((m_idx - m_all)/32)
            nc.vector.tensor_reduce(
                negm_all[:], negm[:], axis=mybir.AxisListType.X,
                op=mybir.AluOpType.min,
            )
            nc.scalar.activation(
                scl[:],
                mx[:],
                mybir.ActivationFunctionType.Exp,
                bias=negm_all[:, 0:1],
                scale=0.03125,
            )
            nc.vector.tensor_mul(sm[:], sm[:], scl[:])
            nc.vector.reduce_sum(rsum[:], sm[:], axis=mybir.AxisListType.X)
            nc.vector.reciprocal(rinv[:], rsum[:])
            for idx, (k0, w) in enumerate(ktiles):
                nc.vector.tensor_scalar_mul(
                    p_sb[:, k0 : k0 + w],
                    p32_sb[:, k0 : k0 + w],
                    scl[:, idx : idx + 1],
                )
            return {"qb": qb, "p_sb": p_sb, "rinv": rinv}

        def emit_pv(stt):
            qb = stt["qb"]
            p_sb = stt["p_sb"]
            rinv = stt["rinv"]
            L = (qb + 1) * P
            pt_sb = pt_pool.tile([P, L], dt, tag="pt")
            for c0 in range(0, qb + 1, 4):
                cn = min(4, qb + 1 - c0)
                ptp = ps_pt.tile([P, 4 * P], dt, tag="ptp")
                for j in range(cn):
                    kc = c0 + j
                    nc.tensor.transpose(
                        ptp[:, j * P : (j + 1) * P],
                        p_sb[:, kc * P : (kc + 1) * P],
                        consts_ident[:],
                    )
                nc.vector.tensor_copy(
                    pt_sb[:, c0 * P : c0 * P + cn * P], ptp[:, : cn * P]
                )

            o_sb = o_pool.tile([P, D_OUT], F32, tag="o")
            for o0 in range(0, D_OUT, MMW):
                pso = ps_o.tile([P, MMW], F32, tag="ps_o")
                for kc in range(qb + 1):
                    nc.tensor.matmul(
                        pso[:],
                        pt_sb[:, kc * P : (kc + 1) * P],
                        v_sb[:, kc, o0 : o0 + MMW],
                        start=(kc == 0),
                        stop=(kc == qb),
                    )
                nc.vector.tensor_scalar_mul(
                    o_sb[:, o0 : o0 + MMW], pso[:], rinv[:, 0:1]
                )
            nc.sync.dma_start(out_d[qb * P : (qb + 1) * P, :], o_sb[:])

        # two-block software pipeline: PV of the previous block hides the
        # softmax latency of the current one.  The four smallest blocks run
        # first (their PV is too short to hide a softmax), then largest-
        # first, so the exposed tail block still has a few-us PV.
        order = [3, 2, 1, 0] + list(range(QB - 1, 3, -1))
        prev = None
        for qb in order:
            stt = emit_scores(qb)
            if prev is not None:
                emit_pv(prev)
            prev = stt
        emit_pv(prev)


def build_program_fp32():
    nc = bass.Bass()
    x_d, wq_d, wk_d, wv_d, negmask_d, out_d = _declare_io(nc)
    qt_d = nc.dram_tensor("qt_scratch", [D_OUT, CTX], F32)
    kt_d = nc.dram_tensor("kt_scratch", [D_OUT, CTX], F32)
    v_d = nc.dram_tensor("v_scratch", [CTX, D_OUT], F32)

    with tile.TileContext(nc) as tc:
        with tc.tile_pool(name="consts", bufs=1) as consts:
            ident = consts.tile([P, P], F32)
            make_identity(nc, ident[:])
            negmask = consts.tile([P, P], F32)
            nc.sync.dma_start(negmask[:], negmask_d[:])

            with (
                tc.tile_pool(name="xt", bufs=1) as xt_pool,
                tc.tile_pool(name="w", bufs=2) as w_pool,
                tc.tile_pool(name="xs", bufs=3) as xs_pool,
                tc.tile_pool(name="stage", bufs=4) as stage_pool,
                tc.tile_pool(name="ps_proj", bufs=4, space="PSUM") as ps_proj,
                tc.tile_pool(name="ps_tr", bufs=4, space="PSUM") as ps_tr,
            ):
                xt = xt_pool.tile([P, IC, CTX], F32)
                for st in range(ST):
                    xs = xs_pool.tile([P, D_IN], F32, tag="xs")
                    nc.sync.dma_start(xs[:], x_d[st * P : (st + 1) * P, :])
                    for ic in range(IC):
                        pt = ps_tr.tile([P, P], F32, tag="pt")
                        nc.tensor.transpose(
                            pt[:], xs[:, ic * P : (ic + 1) * P], ident[:]
                        )
                        nc.vector.tensor_copy(xt[:, ic, st * P : (st + 1) * P], pt[:])

                for w_d, dst in ((wq_d, qt_d), (wk_d, kt_d)):
                    w_sb = w_pool.tile([P, IC, D_OUT], F32, tag="w")
                    nc.sync.dma_start(
                        w_sb[:], w_d[:].rearrange("(c p) o -> p c o", p=P)
                    )
                    for s0 in range(0, CTX, MMW):
                        for oc in range(OC):
                            ps = ps_proj.tile([P, MMW], F32, tag="ps")
                            for ic in range(IC):
                                nc.tensor.matmul(
                                    ps[:],
                                    w_sb[:, ic, oc * P : (oc + 1) * P],
                                    xt[:, ic, s0 : s0 + MMW],
                                    start=(ic == 0),
                                    stop=(ic == IC - 1),
                                )
                            sg = stage_pool.tile([P, MMW], F32, tag="sg")
                            nc.vector.tensor_copy(sg[:], ps[:])
                            nc.sync.dma_start(
                                dst[oc * P : (oc + 1) * P, s0 : s0 + MMW], sg[:]
                            )

                wv_sb = w_pool.tile([P, IC, D_OUT], F32, tag="w")
                nc.sync.dma_start(
                    wv_sb[:], wv_d[:].rearrange("(c p) o -> p c o", p=P)
                )
                for st in range(ST):
                    for o0 in range(0, D_OUT, MMW):
                        ps = ps_proj.tile([P, MMW], F32, tag="ps")
                        for ic in range(IC):
                            nc.tensor.matmul(
                                ps[:],
                                xt[:, ic, st * P : (st + 1) * P],
                                wv_sb[:, ic, o0 : o0 + MMW],
                                start=(ic == 0),
                                stop=(ic == IC - 1),
                            )
                        sg = stage_pool.tile([P, MMW], F32, tag="sg")
                        nc.vector.tensor_copy(sg[:], ps[:])
                        nc.sync.dma_start(
                            v_d[st * P : (st + 1) * P, o0 : o0 + MMW], sg[:]
                        )

            with (
                tc.tile_pool(name="kt", bufs=1) as kt_pool,
                tc.tile_pool(name="v", bufs=1) as v_pool,
                tc.tile_pool(name="qtb", bufs=2) as qtb_pool,
            ):
                kt_sb = kt_pool.tile([P, OC, CTX], F32)
                for c in range(OC):
                    nc.sync.dma_start(kt_sb[:, c, :], kt_d[c * P : (c + 1) * P, :])
                v_sb = v_pool.tile([P, ST, D_OUT], F32)
                for c in range(ST):
                    nc.sync.dma_start(v_sb[:, c, :], v_d[c * P : (c + 1) * P, :])

                def qt_src(qb):
                    qt_b = qtb_pool.tile([P, OC, P], F32, tag="qtb")
                    for oc in range(OC):
                        nc.sync.dma_start(
                            qt_b[:, oc, :],
                            qt_d[oc * P : (oc + 1) * P, qb * P : (qb + 1) * P],
                        )
                    return qt_b

                _attention_phase(
                    nc, tc, ident, negmask, qt_src, kt_sb, v_sb, out_d, F32
                )

    _split_multi_waits(nc)
    return nc


def build_program_fp16():
    """fp16 build: x^T and the weights are pre-cast/pre-transposed to fp16 on
    the HOST (pure layout prep; identical round-to-nearest as a DVE cast), so
    the device only does matmuls, softmax and the P transposes."""
    nc = bass.Bass()
    xt_d = nc.declare_dram_parameter("xT16", [D_IN, CTX], F16, isOutput=False)
    wq_d = nc.declare_dram_parameter("Wq16", [D_IN, D_OUT], F16, isOutput=False)
    wk_d = nc.declare_dram_parameter("Wk16", [D_IN, D_OUT], F16, isOutput=False)
    wv_d = nc.declare_dram_parameter("Wv16", [D_IN, D_OUT], F16, isOutput=False)
    negmask_d = nc.declare_dram_parameter("negmask", [P, P], F32, isOutput=False)
    out_d = nc.declare_dram_parameter("out", [CTX, D_OUT], F32, isOutput=True)

    with tile.TileContext(nc) as tc:
        with tc.tile_pool(name="consts", bufs=1) as consts:
            ident16 = consts.tile([P, P], F16)
            make_identity(nc, ident16[:])
            negmask = consts.tile([P, P], F32)
            nc.sync.dma_start(negmask[:], negmask_d[:])

            with (
                tc.tile_pool(name="qt", bufs=1) as qt_pool,
                tc.tile_pool(name="kt", bufs=1) as kt_pool,
                tc.tile_pool(name="v", bufs=1) as v_pool,
            ):
                qt_sb = qt_pool.tile([P, OC, CTX], F16)
                kt_sb = kt_pool.tile([P, OC, CTX], F16)
                v_sb = v_pool.tile([P, ST, D_OUT], F16)

                with (
                    tc.tile_pool(name="xt", bufs=1) as xt_pool,
                    tc.tile_pool(name="w", bufs=1) as w_pool,
                    tc.tile_pool(name="ps_proj", bufs=8, space="PSUM") as ps_proj,
                ):
                    # x^T arrives per (i-chunk, 512-col s-group); group-0
                    # chunks are queued BEFORE the weight loads so the first
                    # projection group only waits for ~1MB of x^T + 2MB of Wq.
                    xt = xt_pool.tile([P, IC, CTX], F16)
                    SG = MMW // P  # stripes per s-group

                    def load_xt_group(g):
                        s0 = g * MMW
                        for ic in range(IC):
                            nc.sync.dma_start(
                                xt[:, ic, s0 : s0 + MMW],
                                xt_d[ic * P : (ic + 1) * P, s0 : s0 + MMW],
                            )

                    wq_sb = w_pool.tile([P, IC, D_OUT], F16, tag="wq")
                    nc.sync.dma_start(wq_sb[:, 0, :], wq_d[0:P, :])
                    load_xt_group(0)
                    wk_sb = w_pool.tile([P, IC, D_OUT], F16, tag="wk")
                    wv_sb = w_pool.tile([P, IC, D_OUT], F16, tag="wv")
                    for ic in range(1, IC):
                        nc.sync.dma_start(
                            wq_sb[:, ic, :], wq_d[ic * P : (ic + 1) * P, :]
                        )
                    for w_d, w_sb in ((wk_d, wk_sb), (wv_d, wv_sb)):
                        for ic in range(IC):
                            nc.sync.dma_start(
                                w_sb[:, ic, :], w_d[ic * P : (ic + 1) * P, :]
                            )

                    for g in range(ST // SG):
                        s0 = g * MMW
                        if g > 0:
                            load_xt_group(g)
                        for dst, w_sb in ((qt_sb, wq_sb), (kt_sb, wk_sb)):
                            for oc in range(OC):
                                ps = ps_proj.tile([P, MMW], F32, tag="ps")
                                for ic in range(IC):
                                    nc.tensor.matmul(
                                        ps[:],
                                        w_sb[:, ic, oc * P : (oc + 1) * P],
                                        xt[:, ic, s0 : s0 + MMW],
                                        start=(ic == 0),
                                        stop=(ic == IC - 1),
                                    )
                                nc.vector.tensor_copy(
                                    dst[:, oc, s0 : s0 + MMW], ps[:]
                                )
                        for st in range(g * SG, (g + 1) * SG):
                            for o0 in range(0, D_OUT, MMW):
                                ps = ps_proj.tile([P, MMW], F32, tag="ps")
                                for ic in range(IC):
                                    nc.tensor.matmul(
                                        ps[:],
                                        xt[:, ic, st * P : (st + 1) * P],
                                        wv_sb[:, ic, o0 : o0 + MMW],
                                        start=(ic == 0),
                                        stop=(ic == IC - 1),
                                    )
                                nc.vector.tensor_copy(
                                    v_sb[:, st, o0 : o0 + MMW], ps[:]
                                )

                def qt_src(qb):
                    return qt_sb[:, :, qb * P : (qb + 1) * P]

                _attention_phase(
                    nc, tc, ident16, negmask, qt_src, kt_sb, v_sb, out_d, F16
                )

    _split_multi_waits(nc)
    return nc


def _attention_phase_hybrid(
    nc, tc, ident16, negmask, qt16, kt16, v16, qt8, kt8, v8, out_d, dbg=None
):
    """Causal attention with a per-q-block dtype split: blocks < R16 run the
    fp16 path (qt16/kt16/v16), blocks >= R16 run fp8 DoubleRow matmuls
    (qt8/kt8/v8, 0.5 cyc/row).  Softmax is fp32 either way; P is built fp16,
    transposed fp16 on the PE, and cast to fp8 on the PSUM->SBUF copy for the
    fp8 blocks."""
    with (
        tc.tile_pool(name="pexp", bufs=3) as p_pool,
        tc.tile_pool(name="pexp32", bufs=3) as p32_pool,
        tc.tile_pool(name="ptr16", bufs=2) as pt16_pool,
        tc.tile_pool(name="ptr8", bufs=2) as pt8_pool,
        tc.tile_pool(name="red", bufs=3) as red_pool,
        tc.tile_pool(name="ob", bufs=3) as o_pool,
        tc.tile_pool(name="ps_s", bufs=4, space="PSUM") as ps_s,
        tc.tile_pool(name="ps_o", bufs=2, space="PSUM") as ps_o,
        tc.tile_pool(name="ps_pt", bufs=2, space="PSUM") as ps_pt,
    ):

        def emit_scores(qb):
            L = (qb + 1) * P
            ktiles = [(k0, min(MMW, L - k0)) for k0 in range(0, L, MMW)]
            nkt = len(ktiles)
            fp8 = qb >= R16
            sc = SC8 if fp8 else SC16

            red = red_pool.tile([P, 4 * nkt + 3], F32, tag="red")
            mx = red[:, 0:nkt]
            negm = red[:, nkt : 2 * nkt]
            sm = red[:, 2 * nkt : 3 * nkt]
            scl = red[:, 3 * nkt : 4 * nkt]
            negm_all = red[:, 4 * nkt : 4 * nkt + 1]
            rsum = red[:, 4 * nkt + 1 : 4 * nkt + 2]
            rinv = red[:, 4 * nkt + 2 : 4 * nkt + 3]

            p_sb = p_pool.tile([P, CTX], F16, tag="p")
            p32_sb = p32_pool.tile([P, CTX], F32, tag="p32")
            for idx, (k0, w) in enumerate(ktiles):
                ps = ps_s.tile([P, MMW], F32, tag="ps_s")
                if not fp8:
                    for oc in range(OC):
                        nc.tensor.matmul(
                            ps[:, :w],
                            qt16[:, oc, qb * P : (qb + 1) * P],
                            kt16[:, oc, k0 : k0 + w],
                            start=(oc == 0),
                            stop=(oc == OC - 1),
                        )
                else:
                    for c in range(OC // 2):
                        nc.tensor.matmul(
                            ps[:, :w],
                            qt8[:, 2 * c : 2 * c + 2, qb * P : (qb + 1) * P],
                            kt8[:, 2 * c : 2 * c + 2, k0 : k0 + w],
                            start=(c == 0),
                            stop=(c == OC // 2 - 1),
                            perf_mode=DR,
                        )
                if idx == nkt - 1:
                    nc.vector.tensor_add(
                        ps[:, w - P : w], ps[:, w - P : w], negmask[:]
                    )
                nc.vector.reduce_max(
                    mx[:, idx : idx + 1], ps[:, :w], axis=mybir.AxisListType.X
                )
                nc.scalar.mul(
                    negm[:, idx : idx + 1], mx[:, idx : idx + 1], -sc
                )
                nc.scalar.activation(
                    p32_sb[:, k0 : k0 + w],
                    ps[:, :w],
                    mybir.ActivationFunctionType.Exp,
                    bias=negm[:, idx : idx + 1],
                    scale=sc,
                    accum_out=sm[:, idx : idx + 1],
                )
            nc.vector.tensor_reduce(
                negm_all[:], negm[:], axis=mybir.AxisListType.X,
                op=mybir.AluOpType.min,
            )
            nc.scalar.activation(
                scl[:],
                mx[:],
                mybir.ActivationFunctionType.Exp,
                bias=negm_all[:, 0:1],
                scale=sc,
            )
            if fp8:
                # P is built x64 so its mass clears e4m3's subnormal range;
                # the row sum picks up the same factor, and with v arriving
                # x32 the output normalization needs a further 1/32.
                nc.scalar.mul(scl[:], scl[:], PSCALE)
            nc.vector.tensor_mul(sm[:], sm[:], scl[:])
            nc.vector.reduce_sum(rsum[:], sm[:], axis=mybir.AxisListType.X)
            nc.vector.reciprocal(rinv[:], rsum[:])
            if fp8:
                nc.scalar.mul(rinv[:], rinv[:], 1.0 / WSCALE)
            for idx, (k0, w) in enumerate(ktiles):
                nc.vector.tensor_scalar_mul(
                    p_sb[:, k0 : k0 + w],
                    p32_sb[:, k0 : k0 + w],
                    scl[:, idx : idx + 1],
                )
            if dbg is not None and qb == DEBUG_QB:
                nc.sync.dma_start(dbg["p32"][:, :L], p32_sb[:, :L])
                nc.sync.dma_start(dbg["p16"][:, :L], p_sb[:, :L])
                nc.sync.dma_start(dbg["red"][:], red[:])
            return {"qb": qb, "p_sb": p_sb, "rinv": rinv}

        def emit_pv(stt):
            qb = stt["qb"]
            p_sb = stt["p_sb"]
            rinv = stt["rinv"]
            nch = qb + 1
            fp8 = qb >= R16
            if fp8:
                pt_sb = pt8_pool.tile([P, ST, P], F8, tag="pt8")
            else:
                pt_sb = pt16_pool.tile([P, R16, P], F16, tag="pt16")
            for c0 in range(0, nch, 4):
                cn = min(4, nch - c0)
                ptp = ps_pt.tile([P, 4 * P], F16, tag="ptp")
                for j in range(cn):
                    kc = c0 + j
                    nc.tensor.transpose(
                        ptp[:, j * P : (j + 1) * P],
                        p_sb[:, kc * P : (kc + 1) * P],
                        ident16[:],
                    )
                if (c0 // 4) % 2 == 0:
                    nc.vector.tensor_copy(
                        pt_sb[:, c0 : c0 + cn, :], ptp[:, : cn * P]
                    )
                else:
                    nc.scalar.mul(
                        pt_sb[:, c0 : c0 + cn, :], ptp[:, : cn * P], 1.0
                    )

            if dbg is not None and qb == DEBUG_QB:
                nc.sync.dma_start(
                    dbg["pt8"][:, : nch * P],
                    pt_sb[:, 0:nch, :],
                )
            o_sb = o_pool.tile([P, D_OUT], F32, tag="o")
            for o0 in range(0, D_OUT, MMW):
                pso = ps_o.tile([P, MMW], F32, tag="ps_o")
                if fp8:
                    npair = nch // 2
                    for c in range(npair):
                        nc.tensor.matmul(
                            pso[:],
                            pt_sb[:, 2 * c : 2 * c + 2, :],
                            v8[:, 2 * c : 2 * c + 2, o0 : o0 + MMW],
                            start=(c == 0),
                            stop=(c == npair - 1 and nch % 2 == 0),
                            perf_mode=DR,
                        )
                    if nch % 2 == 1:
                        nc.tensor.matmul(
                            pso[:],
                            pt_sb[:, nch - 1, :],
                            v8[:, nch - 1, o0 : o0 + MMW],
                            start=False,
                            stop=True,
                        )
                else:
                    for kc in range(nch):
                        nc.tensor.matmul(
                            pso[:],
                            pt_sb[:, kc, :],
                            v16[:, kc, o0 : o0 + MMW],
                            start=(kc == 0),
                            stop=(kc == nch - 1),
                        )
                nc.vector.tensor_scalar_mul(
                    o_sb[:, o0 : o0 + MMW], pso[:], rinv[:, 0:1]
                )
            nc.sync.dma_start(out_d[qb * P : (qb + 1) * P, :], o_sb[:])

        # three-stage software pipeline: while block n's softmax chain runs
        # on DVE/ACT, the PE executes PV(n-2) and the scores of n+1.
        order = [3, 2, 1, 0] + list(range(QB - 1, 3, -1))
        pend = []
        for qb in order:
            pend.append(emit_scores(qb))
            if len(pend) > 2:
                emit_pv(pend.pop(0))
        for stt in pend:
            emit_pv(stt)


def build_program_fp8():
    """Hybrid fp16/fp8 build.  Rows < RS go through the fp16 pipeline
    (projections and attention), rows >= RS through fp8 DoubleRow matmuls
    (2x PE throughput).  Early K/V are cast fp16->fp8 on the DVE so late
    blocks can consume them in DoubleRow mode.  Host pre-casts x^T and the
    weights to fp16 and fp8 (pure dtype/layout prep, same round-to-nearest
    as a DVE cast)."""
    nc = bass.Bass()
    xt16_d = nc.declare_dram_parameter("xT16pre", [D_IN, RS], F16, isOutput=False)
    xt8_d = nc.declare_dram_parameter("xT8post", [D_IN, CTX - RS], F8, isOutput=False)
    wq16_d = nc.declare_dram_parameter("Wq16", [D_IN, D_OUT], F16, isOutput=False)
    wk16_d = nc.declare_dram_parameter("Wk16", [D_IN, D_OUT], F16, isOutput=False)
    wv16_d = nc.declare_dram_parameter("Wv16", [D_IN, D_OUT], F16, isOutput=False)
    wq8_d = nc.declare_dram_parameter("Wq8", [D_IN, D_OUT], F8, isOutput=False)
    wk8_d = nc.declare_dram_parameter("Wk8", [D_IN, D_OUT], F8, isOutput=False)
    wv8_d = nc.declare_dram_parameter("Wv8", [D_IN, D_OUT], F8, isOutput=False)
    negmask_d = nc.declare_dram_parameter("negmask", [P, P], F32, isOutput=False)
    out_d = nc.declare_dram_parameter("out", [CTX, D_OUT], F32, isOutput=True)
    dbg = None
    if DEBUG_DUMP:
        LD = (DEBUG_QB + 1) * P
        dbg = {
            "qt8": nc.declare_dram_parameter(
                "dbg_qt8", [D_OUT, CTX - RS], F8, isOutput=True
            ),
            "kt8": nc.declare_dram_parameter(
                "dbg_kt8", [D_OUT, CTX], F8, isOutput=True
            ),
            "v8": nc.declare_dram_parameter(
                "dbg_v8", [CTX, D_OUT], F8, isOutput=True
            ),
            "p32": nc.declare_dram_parameter(
                "dbg_p32", [P, LD], F32, isOutput=True
            ),
            "p16": nc.declare_dram_parameter(
                "dbg_p16", [P, LD], F16, isOutput=True
            ),
            "pt8": nc.declare_dram_parameter(
                "dbg_pt8", [P, LD], F8, isOutput=True
            ),
            "red": nc.declare_dram_parameter(
                "dbg_red", [P, 4 * 2 + 3], F32, isOutput=True
            ),
        }

    with tile.TileContext(nc) as tc:
        with tc.tile_pool(name="consts", bufs=1) as consts:
            ident16 = consts.tile([P, P], F16)
            make_identity(nc, ident16[:])
            negmask = consts.tile([P, P], F32)
            nc.sync.dma_start(negmask[:], negmask_d[:])

            with (
                tc.tile_pool(name="qt16", bufs=1) as qt16_pool,
                tc.tile_pool(name="kt16", bufs=1) as kt16_pool,
                tc.tile_pool(name="v16", bufs=1) as v16_pool,
                tc.tile_pool(name="qt8", bufs=1) as qt8_pool,
                tc.tile_pool(name="kt8", bufs=1) as kt8_pool,
                tc.tile_pool(name="v8", bufs=1) as v8_pool,
                tc.tile_pool(name="w8", bufs=1) as w8_pool,
                tc.tile_pool(name="xt8", bufs=1) as xt8_pool,
            ):
                qt16 = qt16_pool.tile([P, OC, RS], F16)
                kt16 = kt16_pool.tile([P, OC, RS], F16)
                v16 = v16_pool.tile([P, R16, D_OUT], F16)
                qt8 = qt8_pool.tile([P, OC, CTX], F8)
                kt8 = kt8_pool.tile([P, OC, CTX], F8)
                v8 = v8_pool.tile([P, ST, D_OUT], F8)
                wq8 = w8_pool.tile([P, IC, D_OUT], F8, tag="wq8")
                wk8 = w8_pool.tile([P, IC, D_OUT], F8, tag="wk8")
                wv8 = w8_pool.tile([P, IC, D_OUT], F8, tag="wv8")
                xt8 = xt8_pool.tile([P, IC, CTX], F8)

                # ---- fp16 projections for rows < RS ----
                with (
                    tc.tile_pool(name="w16", bufs=1) as w16_pool,
                    tc.tile_pool(name="xt16", bufs=1) as xt16_pool,
                    tc.tile_pool(name="ps_p16", bufs=8, space="PSUM") as ps_p16,
                ):
                    wq16 = w16_pool.tile([P, IC, D_OUT], F16, tag="wq16")
                    wk16 = w16_pool.tile([P, IC, D_OUT], F16, tag="wk16")
                    wv16 = w16_pool.tile([P, IC, D_OUT], F16, tag="wv16")
                    xt16 = xt16_pool.tile([P, IC, RS], F16)
                    # DMA order: what the first matmuls need comes first;
                    # the fp8-phase tensors stream in behind.
                    for ic in range(IC):
                        nc.sync.dma_start(
                            xt16[:, ic, :], xt16_d[ic * P : (ic + 1) * P, :]
                        )
                    for w_d, w_sb in (
                        (wq16_d, wq16),
                        (wk16_d, wk16),
                        (wv16_d, wv16),
                    ):
                        for ic in range(IC):
                            nc.sync.dma_start(
                                w_sb[:, ic, :], w_d[ic * P : (ic + 1) * P, :]
                            )
                    for w_d, w_sb in ((wq8_d, wq8), (wk8_d, wk8), (wv8_d, wv8)):
                        for ic in range(IC):
                            nc.sync.dma_start(
                                w_sb[:, ic, :], w_d[ic * P : (ic + 1) * P, :]
                            )
                    for ic in range(IC):
                        nc.sync.dma_start(
                            xt8[:, ic, RS:CTX], xt8_d[ic * P : (ic + 1) * P, :]
                        )

                    for dst, w_sb in ((qt16, wq16), (kt16, wk16)):
                        for s0, w in ((0, MMW), (MMW, RS - MMW)):
                            for oc in range(OC):
                                ps = ps_p16.tile([P, MMW], F32, tag="ps")
                                for ic in range(IC):
                                    nc.tensor.matmul(
                                        ps[:, :w],
                                        w_sb[:, ic, oc * P : (oc + 1) * P],
                                        xt16[:, ic, s0 : s0 + w],
                                        start=(ic == 0),
                                        stop=(ic == IC - 1),
                                    )
                                nc.vector.tensor_copy(
                                    dst[:, oc, s0 : s0 + w], ps[:, :w]
                                )
                    for st in range(R16):
                        for o0 in range(0, D_OUT, MMW):
                            ps = ps_p16.tile([P, MMW], F32, tag="ps")
                            for ic in range(IC):
                                nc.tensor.matmul(
                                    ps[:],
                                    xt16[:, ic, st * P : (st + 1) * P],
                                    wv16[:, ic, o0 : o0 + MMW],
                                    start=(ic == 0),
                                    stop=(ic == IC - 1),
                                )
                            nc.vector.tensor_copy(
                                v16[:, st, o0 : o0 + MMW], ps[:]
                            )
                    # early K/V cast to fp8 (x32, matching the scaled fp8
                    # projections) for the late fp8 blocks
                    for oc in range(OC):
                        nc.scalar.mul(kt8[:, oc, 0:RS], kt16[:, oc, :], WSCALE)
                    for st in range(R16):
                        nc.scalar.mul(v8[:, st, :], v16[:, st, :], WSCALE)

                # ---- fp8 DoubleRow projections for rows >= RS ----
                with tc.tile_pool(name="ps_p8", bufs=8, space="PSUM") as ps_p8:
                    sgroups = []
                    s0 = RS
                    while s0 < CTX:
                        w = min(MMW, CTX - s0)
                        sgroups.append((s0, w))
                        s0 += w
                    # PSUM->SBUF casts alternate between DVE and ACT: a
                    # [128,512] cast (~0.6us) costs more than the 4 paired
                    # DR matmuls feeding it (~0.43us), so a single engine
                    # would gate the PE here.
                    for s0, w in sgroups:
                        for dst, w_sb in ((qt8, wq8), (kt8, wk8)):
                            for oc in range(OC):
                                ps = ps_p8.tile([P, MMW], F32, tag="ps")
                                for c in range(IC // 2):
                                    nc.tensor.matmul(
                                        ps[:, :w],
                                        w_sb[
                                            :,
                                            2 * c : 2 * c + 2,
                                            oc * P : (oc + 1) * P,
                                        ],
                                        xt8[:, 2 * c : 2 * c + 2, s0 : s0 + w],
                                        start=(c == 0),
                                        stop=(c == IC // 2 - 1),
                                        perf_mode=DR,
                                    )
                                if oc % 2 == 0:
                                    nc.vector.tensor_copy(
                                        dst[:, oc, s0 : s0 + w], ps[:, :w]
                                    )
                                else:
                                    nc.scalar.mul(
                                        dst[:, oc, s0 : s0 + w], ps[:, :w], 1.0
                                    )
                        for st in range(s0 // P, (s0 + w) // P):
                            for o0 in range(0, D_OUT, MMW):
                                ps = ps_p8.tile([P, MMW], F32, tag="ps")
                                for c in range(IC // 2):
                                    nc.tensor.matmul(
                                        ps[:],
                                        xt8[
                                            :,
                                            2 * c : 2 * c + 2,
                                            st * P : (st + 1) * P,
                                        ],
                                        wv8[:, 2 * c : 2 * c + 2, o0 : o0 + MMW],
                                        start=(c == 0),
                                        stop=(c == IC // 2 - 1),
                                        perf_mode=DR,
                                    )
                                if (st + o0 // MMW) % 2 == 0:
                                    nc.vector.tensor_copy(
                                        v8[:, st, o0 : o0 + MMW], ps[:]
                                    )
                                else:
                                    nc.scalar.mul(
                                        v8[:, st, o0 : o0 + MMW], ps[:], 1.0
                                    )

                if dbg is not None:
                    for oc in range(OC):
                        nc.sync.dma_start(
                            dbg["qt8"][oc * P : (oc + 1) * P, :],
                            qt8[:, oc, RS:CTX],
                        )
                        nc.sync.dma_start(
                            dbg["kt8"][oc * P : (oc + 1) * P, :], kt8[:, oc, :]
                        )
                    for st in range(ST):
                        nc.sync.dma_start(
                            dbg["v8"][st * P : (st + 1) * P, :], v8[:, st, :]
                        )

                _attention_phase_hybrid(
                    nc, tc, ident16, negmask, qt16, kt16, v16, qt8, kt8, v8,
                    out_d, dbg,
                )

    _split_multi_waits(nc)
    return nc


_program_cache = {}


def build_program(mode=None):
    mode = mode or MODE
    if mode == "fp32":
        return build_program_fp32()
    elif mode == "fp16":
        return build_program_fp16()
    elif mode == "fp8":
        return build_program_fp8()
    raise ValueError(mode)


def make_in_maps(x, Wq, Wk, Wv):
    x = np.ascontiguousarray(np.asarray(x), dtype=np.float32)
    Wq = np.ascontiguousarray(np.asarray(Wq), dtype=np.float32)
    Wk = np.ascontiguousarray(np.asarray(Wk), dtype=np.float32)
    Wv = np.ascontiguousarray(np.asarray(Wv), dtype=np.float32)

    iu = np.triu(np.ones((P, P), dtype=np.float32), k=1)
    negmask = (iu * NEG_BIG).astype(np.float32)

    if MODE == "fp8":
        import ml_dtypes

        F8NP = ml_dtypes.float8_e4m3
        xT = np.transpose(x, (0, 2, 1))  # [b, d_in, ctx]
        xT16pre = np.ascontiguousarray(xT[:, :, :RS].astype(np.float16))
        xT8post = np.ascontiguousarray(xT[:, :, RS:].astype(F8NP))
        wq16 = np.ascontiguousarray(Wq.astype(np.float16))
        wk16 = np.ascontiguousarray(Wk.astype(np.float16))
        wv16 = np.ascontiguousarray(Wv.astype(np.float16))
        ws = np.float32(WSCALE)
        wq8 = np.ascontiguousarray((Wq * ws).astype(F8NP))
        wk8 = np.ascontiguousarray((Wk * ws).astype(F8NP))
        wv8 = np.ascontiguousarray((Wv * ws).astype(F8NP))
        in_maps = [
            {
                "xT16pre": xT16pre[b],
                "xT8post": xT8post[b],
                "Wq16": wq16,
                "Wk16": wk16,
                "Wv16": wv16,
                "Wq8": wq8,
                "Wk8": wk8,
                "Wv8": wv8,
                "negmask": negmask,
            }
            for b in range(BATCH)
        ]
    elif MODE == "fp16":
        # host-side layout prep: fp16 round-to-nearest (same as a DVE cast)
        # and the x transpose the device would otherwise do on the PE
        xT16 = np.ascontiguousarray(
            np.transpose(x.astype(np.float16), (0, 2, 1))
        )
        wq16 = np.ascontiguousarray(Wq.astype(np.float16))
        wk16 = np.ascontiguousarray(Wk.astype(np.float16))
        wv16 = np.ascontiguousarray(Wv.astype(np.float16))
        in_maps = [
            {
                "xT16": xT16[b],
                "Wq16": wq16,
                "Wk16": wk16,
                "Wv16": wv16,
                "negmask": negmask,
            }
            for b in range(BATCH)
        ]
    else:
        in_maps = [
            {"x": x[b], "Wq": Wq, "Wk": Wk, "Wv": Wv, "negmask": negmask}
            for b in range(BATCH)
        ]
    return in_maps


def kernel(x, Wq, Wk, Wv):
    from concourse.bass_utils import run_bass_kernel_spmd

    if MODE not in _program_cache:
        _program_cache[MODE] = build_program(MODE)
    nc = _program_cache[MODE]

    in_maps = make_in_maps(x, Wq, Wk, Wv)
    res = run_bass_kernel_spmd(nc, in_maps, list(range(N_CORES)))
    return np.stack([res.results[b]["out"] for b in range(BATCH)], axis=0)



# revision 22
# speedup vs baseline: 1.1210x; 1.0624x over previous
"""Causal single-head attention (batch=8, ctx=2048, d=1024) on 8 trn2 cores.

Strategy: pure data-parallel over batch — core b computes attention for
batch element b with no cross-core communication.

Per-core pipeline:
  phase 1: Q^T, K^T (o-major) and V (s-major) projections accumulated in
           PSUM (fp32), consumed per 512-column s-group of x^T.
  phase 2: flash-style causal attention per 128-row q-block:
           S = Q^T.T @ K^T, additive causal mask on the diagonal
           128x128 sub-tile, one-pass softmax (per-tile exp(s - m_tile)
           on ACT with row-sum accumulators, exp(m_tile - m_all)
           correction folded into P), P transposed per tile on the PE,
           O = P @ V accumulated in PSUM, deferred normalization by the
           reciprocal row sum, DMA out (fp32); two-block software
           pipeline so PV of one block hides the next one's softmax.

MODE selects the matmul input dtype:
  "fp32": all matmul inputs fp32 (4 cyc/row); x^T built on-device via PE
          transposes; Q^T/K^T/V staged through DRAM scratch (SBUF can't
          hold x^T + all three in fp32).  ~1.25ms, rel err ~6e-6.
  "fp16": matmul inputs fp16 (1 cyc/row), fp32 PSUM accumulation and
          softmax; x^T and the weights are pre-cast/pre-transposed on the
          host (pure layout prep, bit-identical to a DVE cast) and
          everything stays resident in SBUF.  ~332us, rel err ~5e-4.
"""

import sys

sys.path.insert(0, "/opt/trn_rl_repo")

import numpy as np

import concourse.bass as bass
import concourse.mybir as mybir
import concourse.tile as tile
from concourse.masks import make_identity
from concourse.vector_clock import ScopedClock

MODE = "fp8"

BATCH = 8
CTX = 2048
D_IN = 1024
D_OUT = 1024
N_CORES = 8
P = 128
F32 = mybir.dt.float32
F16 = mybir.dt.float16
F8 = mybir.dt.float8e4
DR = mybir.MatmulPerfMode.DoubleRow
NEG_BIG = -1.0e30
R16 = 5  # q/seq blocks 0..R16-1 use the fp16 path; the rest fp8 DoubleRow
RS = R16 * P
# e4m3's normal range starts at 2^-6; the weights (std 1/32) and softmax
# probs sit mostly below it, where quantization is coarse (and the PE
# appears to flush subnormals).  Scale W by 32 on the host (so q,k,v land
# in PSUM pre-scaled by 32) and P by 64 on the device; the inverse scales
# fold into the exp logit scale and the output normalization for free.
WSCALE = 32.0  # host: W8 = e4m3(W * 32) -> q,k,v arrive x32
PSCALE = 64.0  # device: P8 = e4m3(P * 64)
SC16 = 0.03125  # logit scale for the fp16 path: 1/sqrt(1024)
SC8 = 0.03125 / (WSCALE * WSCALE)  # fp8 path: logits arrive x1024
DEBUG_DUMP = False  # extra DRAM outputs for per-stage error attribution
DEBUG_QB = 6

# ---------------------------------------------------------------------------
# Workarounds for the walrus build on this stack: it rejects any instruction
# carrying more than ONE sync wait.  (1) Patch the TileContext tail drain to
# spread its waits over preceding sync-engine nops; (2) post-pass that hoists
# extra waits from any instruction onto same-engine nops inserted right
# before it (sequencers execute per-engine streams in order, so this is
# semantics-preserving).
# ---------------------------------------------------------------------------


def _patched_drain_and_barrier(self, tick_clock, wait_clock):
    nc = self.nc
    nops = [nc.sync.nop(nofuse=True) for _ in range(27)]
    drain_inst = nc.sync.drain()
    wait_clock.add_sem_waits(
        drain_inst.ins, ScopedClock({None: tick_clock.global_clock})
    )
    si = drain_inst.ins.sync_info
    if si is not None and si.on_wait is not None and len(si.on_wait) > 1:
        waits = list(si.on_wait)
        si.on_wait = waits[:1]
        rest = waits[1:]
        for i, nop in enumerate(nops):
            chunk = rest[i : i + 1]
            if not chunk:
                break
            nsi = nop.ins.sync_info
            if nsi is None:
                nop.ins.sync_info = mybir.SyncInfo(on_wait=chunk, on_update=[])
            else:
                nsi.on_wait = chunk

    nc.all_engine_barrier()
    assert self.sems is not None
    popped = nc._tile_sem_poison_stack.pop()
    assert popped is self._sem_poison
    nc.clear_and_free_semaphores(list(self.sems.allocated().values()))
    nc.all_engine_barrier()


tile.TileContext._drain_and_barrier = _patched_drain_and_barrier


def _split_multi_waits(nc):
    n_split = 0
    for f in nc.m.functions:
        for bb in f.blocks:
            il = bb.instructions
            if not any(
                inst.sync_info is not None
                and inst.sync_info.on_wait
                and len(inst.sync_info.on_wait) > 1
                for inst in il
            ):
                continue
            new = []
            for inst in il:
                si = inst.sync_info
                if si is not None and si.on_wait and len(si.on_wait) > 1:
                    waits = list(si.on_wait)
                    for w in waits[:-1]:
                        nop = mybir.InstNoOp(
                            name=nc.get_next_instruction_name(), ins=[], outs=[]
                        )
                        nop.engine = inst.engine
                        nop.sync_info = mybir.SyncInfo(on_wait=[w], on_update=[])
                        new.append(nop)
                        n_split += 1
                    si.on_wait = [waits[-1]]
                new.append(inst)
            il[:] = new
    return n_split


# ---------------------------------------------------------------------------
# Program builders
# ---------------------------------------------------------------------------

IC = D_IN // P  # 8 input-dim chunks
OC = D_OUT // P  # 8 output-dim chunks
ST = CTX // P  # 16 seq chunks
QB = CTX // P  # 16 q blocks
MMW = 512  # moving width (psum bank = 512 fp32)


def _declare_io(nc):
    x_d = nc.declare_dram_parameter("x", [CTX, D_IN], F32, isOutput=False)
    wq_d = nc.declare_dram_parameter("Wq", [D_IN, D_OUT], F32, isOutput=False)
    wk_d = nc.declare_dram_parameter("Wk", [D_IN, D_OUT], F32, isOutput=False)
    wv_d = nc.declare_dram_parameter("Wv", [D_IN, D_OUT], F32, isOutput=False)
    negmask_d = nc.declare_dram_parameter("negmask", [P, P], F32, isOutput=False)
    out_d = nc.declare_dram_parameter("out", [CTX, D_OUT], F32, isOutput=True)
    return x_d, wq_d, wk_d, wv_d, negmask_d, out_d


def _attention_phase(nc, tc, consts_ident, negmask, qt_src, kt_sb, v_sb, out_d, dt):
    """qt_src(qb) -> [P, OC, P] tile of Q^T for that block.

    One-pass softmax: each score tile gets exp(s - m_tile) immediately
    (ACT, off the PE critical path); after the block's tiles are done the
    per-tile correction exp(m_tile - m_all) is folded into the 16-bit P
    tiles and the row-sum."""
    with (
        tc.tile_pool(name="pexp", bufs=3) as p_pool,
        tc.tile_pool(name="pexp32", bufs=3) as p32_pool,
        tc.tile_pool(name="ptr", bufs=2) as pt_pool,
        tc.tile_pool(name="red", bufs=3) as red_pool,
        tc.tile_pool(name="ob", bufs=3) as o_pool,
        tc.tile_pool(name="ps_s", bufs=4, space="PSUM") as ps_s,
        tc.tile_pool(name="ps_o", bufs=2, space="PSUM") as ps_o,
        tc.tile_pool(name="ps_pt", bufs=2, space="PSUM") as ps_pt,
    ):

        def emit_scores(qb):
            L = (qb + 1) * P
            ktiles = [(k0, min(MMW, L - k0)) for k0 in range(0, L, MMW)]
            nkt = len(ktiles)

            qt_b = qt_src(qb)

            red = red_pool.tile([P, 4 * nkt + 3], F32, tag="red")
            mx = red[:, 0:nkt]
            negm = red[:, nkt : 2 * nkt]
            sm = red[:, 2 * nkt : 3 * nkt]
            scl = red[:, 3 * nkt : 4 * nkt]
            negm_all = red[:, 4 * nkt : 4 * nkt + 1]
            rsum = red[:, 4 * nkt + 1 : 4 * nkt + 2]
            rinv = red[:, 4 * nkt + 2 : 4 * nkt + 3]

            p_sb = p_pool.tile([P, CTX], dt, tag="p")
            p32_sb = p32_pool.tile([P, CTX], F32, tag="p32")
            for idx, (k0, w) in enumerate(ktiles):
                ps = ps_s.tile([P, MMW], F32, tag="ps_s")
                for oc in range(OC):
                    nc.tensor.matmul(
                        ps[:, :w],
                        qt_b[:, oc, :],
                        kt_sb[:, oc, k0 : k0 + w],
                        start=(oc == 0),
                        stop=(oc == OC - 1),
                    )
                if idx == nkt - 1:
                    nc.vector.tensor_add(
                        ps[:, w - P : w], ps[:, w - P : w], negmask[:]
                    )
                nc.vector.reduce_max(
                    mx[:, idx : idx + 1], ps[:, :w], axis=mybir.AxisListType.X
                )
                nc.scalar.mul(
                    negm[:, idx : idx + 1], mx[:, idx : idx + 1], -0.03125
                )
                # exp(s - m_tile) immediately; row-sums into sm[idx]
                nc.scalar.activation(
                    p32_sb[:, k0 : k0 + w],
                    ps[:, :w],
                    mybir.ActivationFunctionType.Exp,
                    bias=negm[:, idx : idx + 1],
                    scale=0.03125,
                    accum_out=sm[:, idx : idx + 1],
                )
            # combine: negm_all = min_idx(-m_idx/32) = -m_all/32;
            # scl_idx = exp((m_idx - m_all)/32)
            nc.vector.tensor_reduce(
                negm_all[:], negm[:], axis=mybir.AxisListType.X,
                op=mybir.AluOpType.min,
            )
            nc.scalar.activation(
                scl[:],
                mx[:],
                mybir.ActivationFunctionType.Exp,
                bias=negm_all[:, 0:1],
                scale=0.03125,
            )
            nc.vector.tensor_mul(sm[:], sm[:], scl[:])
            nc.vector.reduce_sum(rsum[:], sm[:], axis=mybir.AxisListType.X)
            nc.vector.reciprocal(rinv[:], rsum[:])
            for idx, (k0, w) in enumerate(ktiles):
                nc.vector.tensor_scalar_mul(
                    p_sb[:, k0 : k0 + w],
                    p32_sb[:, k0 : k0 + w],
                    scl[:, idx : idx + 1],
                )
            return {"qb": qb, "p_sb": p_sb, "rinv": rinv}

        def emit_pv(stt):
            qb = stt["qb"]
            p_sb = stt["p_sb"]
            rinv = stt["rinv"]
            L = (qb + 1) * P
            pt_sb = pt_pool.tile([P, L], dt, tag="pt")
            for c0 in range(0, qb + 1, 4):
                cn = min(4, qb + 1 - c0)
                ptp = ps_pt.tile([P, 4 * P], dt, tag="ptp")
                for j in range(cn):
                    kc = c0 + j
                    nc.tensor.transpose(
                        ptp[:, j * P : (j + 1) * P],
                        p_sb[:, kc * P : (kc + 1) * P],
                        consts_ident[:],
                    )
                nc.vector.tensor_copy(
                    pt_sb[:, c0 * P : c0 * P + cn * P], ptp[:, : cn * P]
                )

            o_sb = o_pool.tile([P, D_OUT], F32, tag="o")
            for o0 in range(0, D_OUT, MMW):
                pso = ps_o.tile([P, MMW], F32, tag="ps_o")
                for kc in range(qb + 1):
                    nc.tensor.matmul(
                        pso[:],
                        pt_sb[:, kc * P : (kc + 1) * P],
                        v_sb[:, kc, o0 : o0 + MMW],
                        start=(kc == 0),
                        stop=(kc == qb),
                    )
                nc.vector.tensor_scalar_mul(
                    o_sb[:, o0 : o0 + MMW], pso[:], rinv[:, 0:1]
                )
            nc.sync.dma_start(out_d[qb * P : (qb + 1) * P, :], o_sb[:])

        # two-block software pipeline: PV of the previous block hides the
        # softmax latency of the current one.  The four smallest blocks run
        # first (their PV is too short to hide a softmax), then largest-
        # first, so the exposed tail block still has a few-us PV.
        order = [3, 2, 1, 0] + list(range(QB - 1, 3, -1))
        prev = None
        for qb in order:
            stt = emit_scores(qb)
            if prev is not None:
                emit_pv(prev)
            prev = stt
        emit_pv(prev)


def build_program_fp32():
    nc = bass.Bass()
    x_d, wq_d, wk_d, wv_d, negmask_d, out_d = _declare_io(nc)
    qt_d = nc.dram_tensor("qt_scratch", [D_OUT, CTX], F32)
    kt_d = nc.dram_tensor("kt_scratch", [D_OUT, CTX], F32)
    v_d = nc.dram_tensor("v_scratch", [CTX, D_OUT], F32)

    with tile.TileContext(nc) as tc:
        with tc.tile_pool(name="consts", bufs=1) as consts:
            ident = consts.tile([P, P], F32)
            make_identity(nc, ident[:])
            negmask = consts.tile([P, P], F32)
            nc.sync.dma_start(negmask[:], negmask_d[:])

            with (
                tc.tile_pool(name="xt", bufs=1) as xt_pool,
                tc.tile_pool(name="w", bufs=2) as w_pool,
                tc.tile_pool(name="xs", bufs=3) as xs_pool,
                tc.tile_pool(name="stage", bufs=4) as stage_pool,
                tc.tile_pool(name="ps_proj", bufs=4, space="PSUM") as ps_proj,
                tc.tile_pool(name="ps_tr", bufs=4, space="PSUM") as ps_tr,
            ):
                xt = xt_pool.tile([P, IC, CTX], F32)
                for st in range(ST):
                    xs = xs_pool.tile([P, D_IN], F32, tag="xs")
                    nc.sync.dma_start(xs[:], x_d[st * P : (st + 1) * P, :])
                    for ic in range(IC):
                        pt = ps_tr.tile([P, P], F32, tag="pt")
                        nc.tensor.transpose(
                            pt[:], xs[:, ic * P : (ic + 1) * P], ident[:]
                        )
                        nc.vector.tensor_copy(xt[:, ic, st * P : (st + 1) * P], pt[:])

                for w_d, dst in ((wq_d, qt_d), (wk_d, kt_d)):
                    w_sb = w_pool.tile([P, IC, D_OUT], F32, tag="w")
                    nc.sync.dma_start(
                        w_sb[:], w_d[:].rearrange("(c p) o -> p c o", p=P)
                    )
                    for s0 in range(0, CTX, MMW):
                        for oc in range(OC):
                            ps = ps_proj.tile([P, MMW], F32, tag="ps")
                            for ic in range(IC):
                                nc.tensor.matmul(
                                    ps[:],
                                    w_sb[:, ic, oc * P : (oc + 1) * P],
                                    xt[:, ic, s0 : s0 + MMW],
                                    start=(ic == 0),
                                    stop=(ic == IC - 1),
                                )
                            sg = stage_pool.tile([P, MMW], F32, tag="sg")
                            nc.vector.tensor_copy(sg[:], ps[:])
                            nc.sync.dma_start(
                                dst[oc * P : (oc + 1) * P, s0 : s0 + MMW], sg[:]
                            )

                wv_sb = w_pool.tile([P, IC, D_OUT], F32, tag="w")
                nc.sync.dma_start(
                    wv_sb[:], wv_d[:].rearrange("(c p) o -> p c o", p=P)
                )
                for st in range(ST):
                    for o0 in range(0, D_OUT, MMW):
                        ps = ps_proj.tile([P, MMW], F32, tag="ps")
                        for ic in range(IC):
                            nc.tensor.matmul(
                                ps[:],
                                xt[:, ic, st * P : (st + 1) * P],
                                wv_sb[:, ic, o0 : o0 + MMW],
                                start=(ic == 0),
                                stop=(ic == IC - 1),
                            )
                        sg = stage_pool.tile([P, MMW], F32, tag="sg")
                        nc.vector.tensor_copy(sg[:], ps[:])
                        nc.sync.dma_start(
                            v_d[st * P : (st + 1) * P, o0 : o0 + MMW], sg[:]
                        )

            with (
                tc.tile_pool(name="kt", bufs=1) as kt_pool,
                tc.tile_pool(name="v", bufs=1) as v_pool,
                tc.tile_pool(name="qtb", bufs=2) as qtb_pool,
            ):
                kt_sb = kt_pool.tile([P, OC, CTX], F32)
                for c in range(OC):
                    nc.sync.dma_start(kt_sb[:, c, :], kt_d[c * P : (c + 1) * P, :])
                v_sb = v_pool.tile([P, ST, D_OUT], F32)
                for c in range(ST):
                    nc.sync.dma_start(v_sb[:, c, :], v_d[c * P : (c + 1) * P, :])

                def qt_src(qb):
                    qt_b = qtb_pool.tile([P, OC, P], F32, tag="qtb")
                    for oc in range(OC):
                        nc.sync.dma_start(
                            qt_b[:, oc, :],
                            qt_d[oc * P : (oc + 1) * P, qb * P : (qb + 1) * P],
                        )
                    return qt_b

                _attention_phase(
                    nc, tc, ident, negmask, qt_src, kt_sb, v_sb, out_d, F32
                )

    _split_multi_waits(nc)
    return nc


def build_program_fp16():
    """fp16 build: x^T and the weights are pre-cast/pre-transposed to fp16 on
    the HOST (pure layout prep; identical round-to-nearest as a DVE cast), so
    the device only does matmuls, softmax and the P transposes."""
    nc = bass.Bass()
    xt_d = nc.declare_dram_parameter("xT16", [D_IN, CTX], F16, isOutput=False)
    wq_d = nc.declare_dram_parameter("Wq16", [D_IN, D_OUT], F16, isOutput=False)
    wk_d = nc.declare_dram_parameter("Wk16", [D_IN, D_OUT], F16, isOutput=False)
    wv_d = nc.declare_dram_parameter("Wv16", [D_IN, D_OUT], F16, isOutput=False)
    negmask_d = nc.declare_dram_parameter("negmask", [P, P], F32, isOutput=False)
    out_d = nc.declare_dram_parameter("out", [CTX, D_OUT], F32, isOutput=True)

    with tile.TileContext(nc) as tc:
        with tc.tile_pool(name="consts", bufs=1) as consts:
            ident16 = consts.tile([P, P], F16)
            make_identity(nc, ident16[:])
            negmask = consts.tile([P, P], F32)
            nc.sync.dma_start(negmask[:], negmask_d[:])

            with (
                tc.tile_pool(name="qt", bufs=1) as qt_pool,
                tc.tile_pool(name="kt", bufs=1) as kt_pool,
                tc.tile_pool(name="v", bufs=1) as v_pool,
            ):
                qt_sb = qt_pool.tile([P, OC, CTX], F16)
                kt_sb = kt_pool.tile([P, OC, CTX], F16)
                v_sb = v_pool.tile([P, ST, D_OUT], F16)

                with (
                    tc.tile_pool(name="xt", bufs=1) as xt_pool,
                    tc.tile_pool(name="w", bufs=1) as w_pool,
                    tc.tile_pool(name="ps_proj", bufs=8, space="PSUM") as ps_proj,
                ):
                    # x^T arrives per (i-chunk, 512-col s-group); group-0
                    # chunks are queued BEFORE the weight loads so the first
                    # projection group only waits for ~1MB of x^T + 2MB of Wq.
                    xt = xt_pool.tile([P, IC, CTX], F16)
                    SG = MMW // P  # stripes per s-group

                    def load_xt_group(g):
                        s0 = g * MMW
                        for ic in range(IC):
                            nc.sync.dma_start(
                                xt[:, ic, s0 : s0 + MMW],
                                xt_d[ic * P : (ic + 1) * P, s0 : s0 + MMW],
                            )

                    wq_sb = w_pool.tile([P, IC, D_OUT], F16, tag="wq")
                    nc.sync.dma_start(wq_sb[:, 0, :], wq_d[0:P, :])
                    load_xt_group(0)
                    wk_sb = w_pool.tile([P, IC, D_OUT], F16, tag="wk")
                    wv_sb = w_pool.tile([P, IC, D_OUT], F16, tag="wv")
                    for ic in range(1, IC):
                        nc.sync.dma_start(
                            wq_sb[:, ic, :], wq_d[ic * P : (ic + 1) * P, :]
                        )
                    for w_d, w_sb in ((wk_d, wk_sb), (wv_d, wv_sb)):
                        for ic in range(IC):
                            nc.sync.dma_start(
                                w_sb[:, ic, :], w_d[ic * P : (ic + 1) * P, :]
                            )

                    for g in range(ST // SG):
                        s0 = g * MMW
                        if g > 0:
                            load_xt_group(g)
                        for dst, w_sb in ((qt_sb, wq_sb), (kt_sb, wk_sb)):
                            for oc in range(OC):
                                ps = ps_proj.tile([P, MMW], F32, tag="ps")
                                for ic in range(IC):
                                    nc.tensor.matmul(
                                        ps[:],
                                        w_sb[:, ic, oc * P : (oc + 1) * P],
                                        xt[:, ic, s0 : s0 + MMW],
                                        start=(ic == 0),
                                        stop=(ic == IC - 1),
                                    )
                                nc.vector.tensor_copy(
                                    dst[:, oc, s0 : s0 + MMW], ps[:]
                                )
                        for st in range(g * SG, (g + 1) * SG):
                            for o0 in range(0, D_OUT, MMW):
                                ps = ps_proj.tile([P, MMW], F32, tag="ps")
                                for ic in range(IC):
                                    nc.tensor.matmul(
                                        ps[:],
                                        xt[:, ic, st * P : (st + 1) * P],
                                        wv_sb[:, ic, o0 : o0 + MMW],
                                        start=(ic == 0),
                                        stop=(ic == IC - 1),
                                    )
                                nc.vector.tensor_copy(
                                    v_sb[:, st, o0 : o0 + MMW], ps[:]
                                )

                def qt_src(qb):
                    return qt_sb[:, :, qb * P : (qb + 1) * P]

                _attention_phase(
                    nc, tc, ident16, negmask, qt_src, kt_sb, v_sb, out_d, F16
                )

    _split_multi_waits(nc)
    return nc


def _attention_phase_hybrid(
    nc, tc, ident16, negmask, qt16, kt16, v16, qt8, kt8, v8, out_d, dbg=None
):
    """Causal attention with a per-q-block dtype split: blocks < R16 run the
    fp16 path (qt16/kt16/v16), blocks >= R16 run fp8 DoubleRow matmuls
    (qt8/kt8/v8, 0.5 cyc/row).  Softmax is fp32 either way; P is built fp16,
    transposed fp16 on the PE, and cast to fp8 on the PSUM->SBUF copy for the
    fp8 blocks."""
    with (
        tc.tile_pool(name="pexp", bufs=3) as p_pool,
        tc.tile_pool(name="ptr16", bufs=2) as pt16_pool,
        tc.tile_pool(name="ptr8", bufs=2) as pt8_pool,
        tc.tile_pool(name="red", bufs=3) as red_pool,
        tc.tile_pool(name="ob", bufs=3) as o_pool,
        tc.tile_pool(name="ps_s", bufs=4, space="PSUM") as ps_s,
        tc.tile_pool(name="ps_o", bufs=2, space="PSUM") as ps_o,
        tc.tile_pool(name="ps_pt", bufs=2, space="PSUM") as ps_pt,
    ):

        def emit_scores(qb):
            """Single-pass softmax: score tiles stay in PSUM until the
            block max is known, then one exp pass per tile writes fp16 P
            (x PSCALE on the fp8 path) with accum_out row sums."""
            L = (qb + 1) * P
            ktiles = [(k0, min(MMW, L - k0)) for k0 in range(0, L, MMW)]
            nkt = len(ktiles)
            fp8 = qb >= R16
            sc = SC8 if fp8 else SC16

            red = red_pool.tile([P, 2 * nkt + 4], F32, tag="red")
            mx = red[:, 0:nkt]
            sm = red[:, nkt : 2 * nkt]
            mx_all = red[:, 2 * nkt : 2 * nkt + 1]
            negm_all = red[:, 2 * nkt + 1 : 2 * nkt + 2]
            rsum = red[:, 2 * nkt + 2 : 2 * nkt + 3]
            rinv = red[:, 2 * nkt + 3 : 2 * nkt + 4]

            p_sb = p_pool.tile([P, CTX], F16, tag="p")
            pss = []
            for idx, (k0, w) in enumerate(ktiles):
                ps = ps_s.tile([P, MMW], F32, tag="ps_s")
                pss.append(ps)
                if not fp8:
                    for oc in range(OC):
                        nc.tensor.matmul(
                            ps[:, :w],
                            qt16[:, oc, qb * P : (qb + 1) * P],
                            kt16[:, oc, k0 : k0 + w],
                            start=(oc == 0),
                            stop=(oc == OC - 1),
                        )
                else:
                    for c in range(OC // 2):
                        nc.tensor.matmul(
                            ps[:, :w],
                            qt8[:, 2 * c : 2 * c + 2, qb * P : (qb + 1) * P],
                            kt8[:, 2 * c : 2 * c + 2, k0 : k0 + w],
                            start=(c == 0),
                            stop=(c == OC // 2 - 1),
                            perf_mode=DR,
                        )
                if idx == nkt - 1:
                    nc.vector.tensor_add(
                        ps[:, w - P : w], ps[:, w - P : w], negmask[:]
                    )
                nc.vector.reduce_max(
                    mx[:, idx : idx + 1], ps[:, :w], axis=mybir.AxisListType.X
                )
            if nkt > 1:
                nc.vector.tensor_reduce(
                    mx_all[:], mx[:], axis=mybir.AxisListType.X,
                    op=mybir.AluOpType.max,
                )
            else:
                mx_all = mx
            nc.scalar.mul(negm_all[:], mx_all[:, 0:1], -sc)
            if fp8:
                # fold ln(PSCALE) into the exp bias: P comes out x64, past
                # e4m3's subnormal range; rsum picks up the same factor and
                # with v arriving x32 the normalization needs a further 1/32.
                nc.vector.tensor_scalar_add(
                    negm_all[:], negm_all[:], float(np.log(PSCALE))
                )
            for idx, (k0, w) in enumerate(ktiles):
                nc.scalar.activation(
                    p_sb[:, k0 : k0 + w],
                    pss[idx][:, :w],
                    mybir.ActivationFunctionType.Exp,
                    bias=negm_all[:, 0:1],
                    scale=sc,
                    accum_out=sm[:, idx : idx + 1],
                )
            if nkt > 1:
                nc.vector.reduce_sum(
                    rsum[:], sm[:], axis=mybir.AxisListType.X
                )
            else:
                rsum = sm
            nc.vector.reciprocal(rinv[:], rsum[:])
            if fp8:
                nc.scalar.mul(rinv[:], rinv[:], 1.0 / WSCALE)
            if dbg is not None and qb == DEBUG_QB:
                nc.sync.dma_start(dbg["p16"][:, :L], p_sb[:, :L])
                nc.sync.dma_start(dbg["red"][:], red[:])
            return {"qb": qb, "p_sb": p_sb, "rinv": rinv}

        def emit_pv(stt):
            qb = stt["qb"]
            p_sb = stt["p_sb"]
            rinv = stt["rinv"]
            nch = qb + 1
            fp8 = qb >= R16
            if fp8:
                pt_sb = pt8_pool.tile([P, ST, P], F8, tag="pt8")
            else:
                pt_sb = pt16_pool.tile([P, R16, P], F16, tag="pt16")
            for c0 in range(0, nch, 4):
                cn = min(4, nch - c0)
                ptp = ps_pt.tile([P, 4 * P], F16, tag="ptp")
                for j in range(cn):
                    kc = c0 + j
                    nc.tensor.transpose(
                        ptp[:, j * P : (j + 1) * P],
                        p_sb[:, kc * P : (kc + 1) * P],
                        ident16[:],
                    )
                if (c0 // 4) % 2 == 0:
                    nc.vector.tensor_copy(
                        pt_sb[:, c0 : c0 + cn, :], ptp[:, : cn * P]
                    )
                else:
                    nc.scalar.mul(
                        pt_sb[:, c0 : c0 + cn, :], ptp[:, : cn * P], 1.0
                    )

            if dbg is not None and qb == DEBUG_QB:
                nc.sync.dma_start(
                    dbg["pt8"][:, : nch * P],
                    pt_sb[:, 0:nch, :],
                )
            o_sb = o_pool.tile([P, D_OUT], F32, tag="o")
            for o0 in range(0, D_OUT, MMW):
                pso = ps_o.tile([P, MMW], F32, tag="ps_o")
                if fp8:
                    npair = nch // 2
                    for c in range(npair):
                        nc.tensor.matmul(
                            pso[:],
                            pt_sb[:, 2 * c : 2 * c + 2, :],
                            v8[:, 2 * c : 2 * c + 2, o0 : o0 + MMW],
                            start=(c == 0),
                            stop=(c == npair - 1 and nch % 2 == 0),
                            perf_mode=DR,
                        )
                    if nch % 2 == 1:
                        nc.tensor.matmul(
                            pso[:],
                            pt_sb[:, nch - 1, :],
                            v8[:, nch - 1, o0 : o0 + MMW],
                            start=False,
                            stop=True,
                        )
                else:
                    for kc in range(nch):
                        nc.tensor.matmul(
                            pso[:],
                            pt_sb[:, kc, :],
                            v16[:, kc, o0 : o0 + MMW],
                            start=(kc == 0),
                            stop=(kc == nch - 1),
                        )
                nc.vector.tensor_scalar_mul(
                    o_sb[:, o0 : o0 + MMW], pso[:], rinv[:, 0:1]
                )
            nc.sync.dma_start(out_d[qb * P : (qb + 1) * P, :], o_sb[:])

        # three-stage software pipeline: while block n's softmax chain runs
        # on DVE/ACT, the PE executes PV(n-2) and the scores of n+1.  Tiny
        # blocks bracket the pipeline so its exposed fill/drain is cheap.
        order = [3, 2] + list(range(QB - 1, 3, -1)) + [1, 0]
        pend = []
        for qb in order:
            pend.append(emit_scores(qb))
            if len(pend) > 2:
                emit_pv(pend.pop(0))
        for stt in pend:
            emit_pv(stt)


def build_program_fp8():
    """Hybrid fp16/fp8 build.  Rows < RS go through the fp16 pipeline
    (projections and attention), rows >= RS through fp8 DoubleRow matmuls
    (2x PE throughput).  Early K/V are cast fp16->fp8 on the DVE so late
    blocks can consume them in DoubleRow mode.  Host pre-casts x^T and the
    weights to fp16 and fp8 (pure dtype/layout prep, same round-to-nearest
    as a DVE cast)."""
    nc = bass.Bass()
    xt16_d = nc.declare_dram_parameter("xT16pre", [D_IN, RS], F16, isOutput=False)
    xt8_d = nc.declare_dram_parameter("xT8post", [D_IN, CTX - RS], F8, isOutput=False)
    wq16_d = nc.declare_dram_parameter("Wq16", [D_IN, D_OUT], F16, isOutput=False)
    wk16_d = nc.declare_dram_parameter("Wk16", [D_IN, D_OUT], F16, isOutput=False)
    wv16_d = nc.declare_dram_parameter("Wv16", [D_IN, D_OUT], F16, isOutput=False)
    wq8_d = nc.declare_dram_parameter("Wq8", [D_IN, D_OUT], F8, isOutput=False)
    wk8_d = nc.declare_dram_parameter("Wk8", [D_IN, D_OUT], F8, isOutput=False)
    wv8_d = nc.declare_dram_parameter("Wv8", [D_IN, D_OUT], F8, isOutput=False)
    negmask_d = nc.declare_dram_parameter("negmask", [P, P], F32, isOutput=False)
    out_d = nc.declare_dram_parameter("out", [CTX, D_OUT], F32, isOutput=True)
    dbg = None
    if DEBUG_DUMP:
        LD = (DEBUG_QB + 1) * P
        dbg = {
            "qt8": nc.declare_dram_parameter(
                "dbg_qt8", [D_OUT, CTX - RS], F8, isOutput=True
            ),
            "kt8": nc.declare_dram_parameter(
                "dbg_kt8", [D_OUT, CTX], F8, isOutput=True
            ),
            "v8": nc.declare_dram_parameter(
                "dbg_v8", [CTX, D_OUT], F8, isOutput=True
            ),
            "p32": nc.declare_dram_parameter(
                "dbg_p32", [P, LD], F32, isOutput=True
            ),
            "p16": nc.declare_dram_parameter(
                "dbg_p16", [P, LD], F16, isOutput=True
            ),
            "pt8": nc.declare_dram_parameter(
                "dbg_pt8", [P, LD], F8, isOutput=True
            ),
            "red": nc.declare_dram_parameter(
                "dbg_red", [P, 4 * 2 + 3], F32, isOutput=True
            ),
        }

    with tile.TileContext(nc) as tc:
        with tc.tile_pool(name="consts", bufs=1) as consts:
            ident16 = consts.tile([P, P], F16)
            make_identity(nc, ident16[:])
            negmask = consts.tile([P, P], F32)
            nc.sync.dma_start(negmask[:], negmask_d[:])

            with (
                tc.tile_pool(name="qt16", bufs=1) as qt16_pool,
                tc.tile_pool(name="kt16", bufs=1) as kt16_pool,
                tc.tile_pool(name="v16", bufs=1) as v16_pool,
                tc.tile_pool(name="qt8", bufs=1) as qt8_pool,
                tc.tile_pool(name="kt8", bufs=1) as kt8_pool,
                tc.tile_pool(name="v8", bufs=1) as v8_pool,
                tc.tile_pool(name="w8", bufs=1) as w8_pool,
                tc.tile_pool(name="xt8", bufs=1) as xt8_pool,
            ):
                qt16 = qt16_pool.tile([P, OC, RS], F16)
                kt16 = kt16_pool.tile([P, OC, RS], F16)
                v16 = v16_pool.tile([P, R16, D_OUT], F16)
                qt8 = qt8_pool.tile([P, OC, CTX], F8)
                kt8 = kt8_pool.tile([P, OC, CTX], F8)
                v8 = v8_pool.tile([P, ST, D_OUT], F8)
                wq8 = w8_pool.tile([P, IC, D_OUT], F8, tag="wq8")
                wk8 = w8_pool.tile([P, IC, D_OUT], F8, tag="wk8")
                wv8 = w8_pool.tile([P, IC, D_OUT], F8, tag="wv8")
                xt8 = xt8_pool.tile([P, IC, CTX], F8)

                # ---- fp8 DoubleRow projections for rows >= RS (first: they
                # need only ~4.4MB of DMA, so the PE starts almost
                # immediately; the 7.3MB fp16-phase tensors stream behind)
                sgroups = []
                s0 = RS
                while s0 < CTX:
                    w = min(MMW, CTX - s0)
                    sgroups.append((s0, w))
                    s0 += w
                with tc.tile_pool(name="ps_p8", bufs=8, space="PSUM") as ps_p8:
                    for ic in range(IC):
                        nc.sync.dma_start(
                            wq8[:, ic, :], wq8_d[ic * P : (ic + 1) * P, :]
                        )
                    s0, w = sgroups[0]
                    for ic in range(IC):
                        nc.sync.dma_start(
                            xt8[:, ic, s0 : s0 + w],
                            xt8_d[ic * P : (ic + 1) * P, : s0 + w - RS],
                        )
                    for w_d, w_sb in ((wk8_d, wk8), (wv8_d, wv8)):
                        for ic in range(IC):
                            nc.sync.dma_start(
                                w_sb[:, ic, :], w_d[ic * P : (ic + 1) * P, :]
                            )
                    for s0, w in sgroups[1:]:
                        for ic in range(IC):
                            nc.sync.dma_start(
                                xt8[:, ic, s0 : s0 + w],
                                xt8_d[
                                    ic * P : (ic + 1) * P,
                                    s0 - RS : s0 + w - RS,
                                ],
                            )
                    # PSUM->SBUF casts alternate between DVE and ACT: a
                    # [128,512] cast (~0.6us) costs more than the 4 paired
                    # DR matmuls feeding it (~0.43us), so a single engine
                    # would gate the PE here.
                    for s0, w in sgroups:
                        for dst, w_sb in ((qt8, wq8), (kt8, wk8)):
                            for oc in range(OC):
                                ps = ps_p8.tile([P, MMW], F32, tag="ps")
                                for c in range(IC // 2):
                                    nc.tensor.matmul(
                                        ps[:, :w],
                                        w_sb[
                                            :,
                                            2 * c : 2 * c + 2,
                                            oc * P : (oc + 1) * P,
                                        ],
                                        xt8[:, 2 * c : 2 * c + 2, s0 : s0 + w],
                                        start=(c == 0),
                                        stop=(c == IC // 2 - 1),
                                        perf_mode=DR,
                                    )
                                if oc % 2 == 0:
                                    nc.vector.tensor_copy(
                                        dst[:, oc, s0 : s0 + w], ps[:, :w]
                                    )
                                else:
                                    nc.scalar.mul(
                                        dst[:, oc, s0 : s0 + w], ps[:, :w], 1.0
                                    )
                        for st in range(s0 // P, (s0 + w) // P):
                            for o0 in range(0, D_OUT, MMW):
                                ps = ps_p8.tile([P, MMW], F32, tag="ps")
                                for c in range(IC // 2):
                                    nc.tensor.matmul(
                                        ps[:],
                                        xt8[
                                            :,
                                            2 * c : 2 * c + 2,
                                            st * P : (st + 1) * P,
                                        ],
                                        wv8[:, 2 * c : 2 * c + 2, o0 : o0 + MMW],
                                        start=(c == 0),
                                        stop=(c == IC // 2 - 1),
                                        perf_mode=DR,
                                    )
                                if (st + o0 // MMW) % 2 == 0:
                                    nc.vector.tensor_copy(
                                        v8[:, st, o0 : o0 + MMW], ps[:]
                                    )
                                else:
                                    nc.scalar.mul(
                                        v8[:, st, o0 : o0 + MMW], ps[:], 1.0
                                    )

                # ---- fp16 projections for rows < RS ----
                with (
                    tc.tile_pool(name="w16", bufs=1) as w16_pool,
                    tc.tile_pool(name="xt16", bufs=1) as xt16_pool,
                    tc.tile_pool(name="ps_p16", bufs=8, space="PSUM") as ps_p16,
                ):
                    wq16 = w16_pool.tile([P, IC, D_OUT], F16, tag="wq16")
                    wk16 = w16_pool.tile([P, IC, D_OUT], F16, tag="wk16")
                    wv16 = w16_pool.tile([P, IC, D_OUT], F16, tag="wv16")
                    xt16 = xt16_pool.tile([P, IC, RS], F16)
                    for ic in range(IC):
                        nc.sync.dma_start(
                            xt16[:, ic, :], xt16_d[ic * P : (ic + 1) * P, :]
                        )
                    for w_d, w_sb in (
                        (wq16_d, wq16),
                        (wk16_d, wk16),
                        (wv16_d, wv16),
                    ):
                        for ic in range(IC):
                            nc.sync.dma_start(
                                w_sb[:, ic, :], w_d[ic * P : (ic + 1) * P, :]
                            )

                    for dst, w_sb in ((qt16, wq16), (kt16, wk16)):
                        for s0, w in ((0, MMW), (MMW, RS - MMW)):
                            for oc in range(OC):
                                ps = ps_p16.tile([P, MMW], F32, tag="ps")
                                for ic in range(IC):
                                    nc.tensor.matmul(
                                        ps[:, :w],
                                        w_sb[:, ic, oc * P : (oc + 1) * P],
                                        xt16[:, ic, s0 : s0 + w],
                                        start=(ic == 0),
                                        stop=(ic == IC - 1),
                                    )
                                nc.vector.tensor_copy(
                                    dst[:, oc, s0 : s0 + w], ps[:, :w]
                                )
                    for st in range(R16):
                        for o0 in range(0, D_OUT, MMW):
                            ps = ps_p16.tile([P, MMW], F32, tag="ps")
                            for ic in range(IC):
                                nc.tensor.matmul(
                                    ps[:],
                                    xt16[:, ic, st * P : (st + 1) * P],
                                    wv16[:, ic, o0 : o0 + MMW],
                                    start=(ic == 0),
                                    stop=(ic == IC - 1),
                                )
                            nc.vector.tensor_copy(
                                v16[:, st, o0 : o0 + MMW], ps[:]
                            )
                    # early K/V cast to fp8 (x32, matching the scaled fp8
                    # projections) for the late fp8 blocks
                    for oc in range(OC):
                        nc.scalar.mul(kt8[:, oc, 0:RS], kt16[:, oc, :], WSCALE)
                    for st in range(R16):
                        nc.scalar.mul(v8[:, st, :], v16[:, st, :], WSCALE)

                if dbg is not None:
                    for oc in range(OC):
                        nc.sync.dma_start(
                            dbg["qt8"][oc * P : (oc + 1) * P, :],
                            qt8[:, oc, RS:CTX],
                        )
                        nc.sync.dma_start(
                            dbg["kt8"][oc * P : (oc + 1) * P, :], kt8[:, oc, :]
                        )
                    for st in range(ST):
                        nc.sync.dma_start(
                            dbg["v8"][st * P : (st + 1) * P, :], v8[:, st, :]
                        )

                _attention_phase_hybrid(
                    nc, tc, ident16, negmask, qt16, kt16, v16, qt8, kt8, v8,
                    out_d, dbg,
                )

    _split_multi_waits(nc)
    return nc


_program_cache = {}


def build_program(mode=None):
    mode = mode or MODE
    if mode == "fp32":
        return build_program_fp32()
    elif mode == "fp16":
        return build_program_fp16()
    elif mode == "fp8":
        return build_program_fp8()
    raise ValueError(mode)


def make_in_maps(x, Wq, Wk, Wv):
    x = np.ascontiguousarray(np.asarray(x), dtype=np.float32)
    Wq = np.ascontiguousarray(np.asarray(Wq), dtype=np.float32)
    Wk = np.ascontiguousarray(np.asarray(Wk), dtype=np.float32)
    Wv = np.ascontiguousarray(np.asarray(Wv), dtype=np.float32)

    iu = np.triu(np.ones((P, P), dtype=np.float32), k=1)
    negmask = (iu * NEG_BIG).astype(np.float32)

    if MODE == "fp8":
        import ml_dtypes

        F8NP = ml_dtypes.float8_e4m3
        xT = np.transpose(x, (0, 2, 1))  # [b, d_in, ctx]
        xT16pre = np.ascontiguousarray(xT[:, :, :RS].astype(np.float16))
        xT8post = np.ascontiguousarray(xT[:, :, RS:].astype(F8NP))
        wq16 = np.ascontiguousarray(Wq.astype(np.float16))
        wk16 = np.ascontiguousarray(Wk.astype(np.float16))
        wv16 = np.ascontiguousarray(Wv.astype(np.float16))
        ws = np.float32(WSCALE)
        wq8 = np.ascontiguousarray((Wq * ws).astype(F8NP))
        wk8 = np.ascontiguousarray((Wk * ws).astype(F8NP))
        wv8 = np.ascontiguousarray((Wv * ws).astype(F8NP))
        in_maps = [
            {
                "xT16pre": xT16pre[b],
                "xT8post": xT8post[b],
                "Wq16": wq16,
                "Wk16": wk16,
                "Wv16": wv16,
                "Wq8": wq8,
                "Wk8": wk8,
                "Wv8": wv8,
                "negmask": negmask,
            }
            for b in range(BATCH)
        ]
    elif MODE == "fp16":
        # host-side layout prep: fp16 round-to-nearest (same as a DVE cast)
        # and the x transpose the device would otherwise do on the PE
        xT16 = np.ascontiguousarray(
            np.transpose(x.astype(np.float16), (0, 2, 1))
        )
        wq16 = np.ascontiguousarray(Wq.astype(np.float16))
        wk16 = np.ascontiguousarray(Wk.astype(np.float16))
        wv16 = np.ascontiguousarray(Wv.astype(np.float16))
        in_maps = [
            {
                "xT16": xT16[b],
                "Wq16": wq16,
                "Wk16": wk16,
                "Wv16": wv16,
                "negmask": negmask,
            }
            for b in range(BATCH)
        ]
    else:
        in_maps = [
            {"x": x[b], "Wq": Wq, "Wk": Wk, "Wv": Wv, "negmask": negmask}
            for b in range(BATCH)
        ]
    return in_maps


def kernel(x, Wq, Wk, Wv):
    from concourse.bass_utils import run_bass_kernel_spmd

    if MODE not in _program_cache:
        _program_cache[MODE] = build_program(MODE)
    nc = _program_cache[MODE]

    in_maps = make_in_maps(x, Wq, Wk, Wv)
    res = run_bass_kernel_spmd(nc, in_maps, list(range(N_CORES)))
    return np.stack([res.results[b]["out"] for b in range(BATCH)], axis=0)



# revision 27
# speedup vs baseline: 1.1708x; 1.0444x over previous
"""Causal single-head attention (batch=8, ctx=2048, d=1024) on 8 trn2 cores.

Strategy: pure data-parallel over batch — core b computes attention for
batch element b with no cross-core communication.

Per-core pipeline:
  phase 1: Q^T, K^T (o-major) and V (s-major) projections accumulated in
           PSUM (fp32), consumed per 512-column s-group of x^T.
  phase 2: flash-style causal attention per 128-row q-block:
           S = Q^T.T @ K^T, additive causal mask on the diagonal
           128x128 sub-tile, one-pass softmax (per-tile exp(s - m_tile)
           on ACT with row-sum accumulators, exp(m_tile - m_all)
           correction folded into P), P transposed per tile on the PE,
           O = P @ V accumulated in PSUM, deferred normalization by the
           reciprocal row sum, DMA out (fp32); two-block software
           pipeline so PV of one block hides the next one's softmax.

MODE selects the matmul input dtype:
  "fp32": all matmul inputs fp32 (4 cyc/row); x^T built on-device via PE
          transposes; Q^T/K^T/V staged through DRAM scratch (SBUF can't
          hold x^T + all three in fp32).  ~1.25ms, rel err ~6e-6.
  "fp16": matmul inputs fp16 (1 cyc/row), fp32 PSUM accumulation and
          softmax; x^T and the weights are pre-cast/pre-transposed on the
          host (pure layout prep, bit-identical to a DVE cast) and
          everything stays resident in SBUF.  ~332us, rel err ~5e-4.
"""

import sys

sys.path.insert(0, "/opt/trn_rl_repo")

import numpy as np

import concourse.bass as bass
import concourse.mybir as mybir
import concourse.tile as tile
from concourse.masks import make_identity
from concourse.vector_clock import ScopedClock

MODE = "fp8"

BATCH = 8
CTX = 2048
D_IN = 1024
D_OUT = 1024
N_CORES = 8
P = 128
F32 = mybir.dt.float32
F16 = mybir.dt.float16
F8 = mybir.dt.float8e4
DR = mybir.MatmulPerfMode.DoubleRow
NEG_BIG = -1.0e30
R16 = 5  # q/seq blocks 0..R16-1 use the fp16 path; the rest fp8 DoubleRow
RS = R16 * P
# e4m3's normal range starts at 2^-6; the weights (std 1/32) and softmax
# probs sit mostly below it, where quantization is coarse (and the PE
# appears to flush subnormals).  Scale W by 32 on the host (so q,k,v land
# in PSUM pre-scaled by 32) and P by 64 on the device; the inverse scales
# fold into the exp logit scale and the output normalization for free.
WSCALE = 32.0  # host: W8 = e4m3(W * 32) -> q,k,v arrive x32
PSCALE = 64.0  # device: P8 = e4m3(P * 64)
SC16 = 0.03125  # logit scale for the fp16 path: 1/sqrt(1024)
SC8 = 0.03125 / (WSCALE * WSCALE)  # fp8 path: logits arrive x1024
DEBUG_DUMP = False  # extra DRAM outputs for per-stage error attribution
DEBUG_QB = 6

# ---------------------------------------------------------------------------
# Workarounds for the walrus build on this stack: it rejects any instruction
# carrying more than ONE sync wait.  (1) Patch the TileContext tail drain to
# spread its waits over preceding sync-engine nops; (2) post-pass that hoists
# extra waits from any instruction onto same-engine nops inserted right
# before it (sequencers execute per-engine streams in order, so this is
# semantics-preserving).
# ---------------------------------------------------------------------------


def _patched_drain_and_barrier(self, tick_clock, wait_clock):
    nc = self.nc
    nops = [nc.sync.nop(nofuse=True) for _ in range(27)]
    drain_inst = nc.sync.drain()
    wait_clock.add_sem_waits(
        drain_inst.ins, ScopedClock({None: tick_clock.global_clock})
    )
    si = drain_inst.ins.sync_info
    if si is not None and si.on_wait is not None and len(si.on_wait) > 1:
        waits = list(si.on_wait)
        si.on_wait = waits[:1]
        rest = waits[1:]
        for i, nop in enumerate(nops):
            chunk = rest[i : i + 1]
            if not chunk:
                break
            nsi = nop.ins.sync_info
            if nsi is None:
                nop.ins.sync_info = mybir.SyncInfo(on_wait=chunk, on_update=[])
            else:
                nsi.on_wait = chunk

    nc.all_engine_barrier()
    assert self.sems is not None
    popped = nc._tile_sem_poison_stack.pop()
    assert popped is self._sem_poison
    nc.clear_and_free_semaphores(list(self.sems.allocated().values()))
    nc.all_engine_barrier()


tile.TileContext._drain_and_barrier = _patched_drain_and_barrier


def _split_multi_waits(nc):
    n_split = 0
    for f in nc.m.functions:
        for bb in f.blocks:
            il = bb.instructions
            if not any(
                inst.sync_info is not None
                and inst.sync_info.on_wait
                and len(inst.sync_info.on_wait) > 1
                for inst in il
            ):
                continue
            new = []
            for inst in il:
                si = inst.sync_info
                if si is not None and si.on_wait and len(si.on_wait) > 1:
                    waits = list(si.on_wait)
                    for w in waits[:-1]:
                        nop = mybir.InstNoOp(
                            name=nc.get_next_instruction_name(), ins=[], outs=[]
                        )
                        nop.engine = inst.engine
                        nop.sync_info = mybir.SyncInfo(on_wait=[w], on_update=[])
                        new.append(nop)
                        n_split += 1
                    si.on_wait = [waits[-1]]
                new.append(inst)
            il[:] = new
    return n_split


# ---------------------------------------------------------------------------
# Program builders
# ---------------------------------------------------------------------------

IC = D_IN // P  # 8 input-dim chunks
OC = D_OUT // P  # 8 output-dim chunks
ST = CTX // P  # 16 seq chunks
QB = CTX // P  # 16 q blocks
MMW = 512  # moving width (psum bank = 512 fp32)


def _declare_io(nc):
    x_d = nc.declare_dram_parameter("x", [CTX, D_IN], F32, isOutput=False)
    wq_d = nc.declare_dram_parameter("Wq", [D_IN, D_OUT], F32, isOutput=False)
    wk_d = nc.declare_dram_parameter("Wk", [D_IN, D_OUT], F32, isOutput=False)
    wv_d = nc.declare_dram_parameter("Wv", [D_IN, D_OUT], F32, isOutput=False)
    negmask_d = nc.declare_dram_parameter("negmask", [P, P], F32, isOutput=False)
    out_d = nc.declare_dram_parameter("out", [CTX, D_OUT], F32, isOutput=True)
    return x_d, wq_d, wk_d, wv_d, negmask_d, out_d


def _attention_phase(nc, tc, consts_ident, negmask, qt_src, kt_sb, v_sb, out_d, dt):
    """qt_src(qb) -> [P, OC, P] tile of Q^T for that block.

    One-pass softmax: each score tile gets exp(s - m_tile) immediately
    (ACT, off the PE critical path); after the block's tiles are done the
    per-tile correction exp(m_tile - m_all) is folded into the 16-bit P
    tiles and the row-sum."""
    with (
        tc.tile_pool(name="pexp", bufs=3) as p_pool,
        tc.tile_pool(name="pexp32", bufs=3) as p32_pool,
        tc.tile_pool(name="ptr", bufs=2) as pt_pool,
        tc.tile_pool(name="red", bufs=3) as red_pool,
        tc.tile_pool(name="ob", bufs=3) as o_pool,
        tc.tile_pool(name="ps_s", bufs=4, space="PSUM") as ps_s,
        tc.tile_pool(name="ps_o", bufs=2, space="PSUM") as ps_o,
        tc.tile_pool(name="ps_pt", bufs=2, space="PSUM") as ps_pt,
    ):

        def emit_scores(qb):
            L = (qb + 1) * P
            ktiles = [(k0, min(MMW, L - k0)) for k0 in range(0, L, MMW)]
            nkt = len(ktiles)

            qt_b = qt_src(qb)

            red = red_pool.tile([P, 4 * nkt + 3], F32, tag="red")
            mx = red[:, 0:nkt]
            negm = red[:, nkt : 2 * nkt]
            sm = red[:, 2 * nkt : 3 * nkt]
            scl = red[:, 3 * nkt : 4 * nkt]
            negm_all = red[:, 4 * nkt : 4 * nkt + 1]
            rsum = red[:, 4 * nkt + 1 : 4 * nkt + 2]
            rinv = red[:, 4 * nkt + 2 : 4 * nkt + 3]

            p_sb = p_pool.tile([P, CTX], dt, tag="p")
            p32_sb = p32_pool.tile([P, CTX], F32, tag="p32")
            for idx, (k0, w) in enumerate(ktiles):
                ps = ps_s.tile([P, MMW], F32, tag="ps_s")
                for oc in range(OC):
                    nc.tensor.matmul(
                        ps[:, :w],
                        qt_b[:, oc, :],
                        kt_sb[:, oc, k0 : k0 + w],
                        start=(oc == 0),
                        stop=(oc == OC - 1),
                    )
                if idx == nkt - 1:
                    nc.vector.tensor_add(
                        ps[:, w - P : w], ps[:, w - P : w], negmask[:]
                    )
                nc.vector.reduce_max(
                    mx[:, idx : idx + 1], ps[:, :w], axis=mybir.AxisListType.X
                )
                nc.scalar.mul(
                    negm[:, idx : idx + 1], mx[:, idx : idx + 1], -0.03125
                )
                # exp(s - m_tile) immediately; row-sums into sm[idx]
                nc.scalar.activation(
                    p32_sb[:, k0 : k0 + w],
                    ps[:, :w],
                    mybir.ActivationFunctionType.Exp,
                    bias=negm[:, idx : idx + 1],
                    scale=0.03125,
                    accum_out=sm[:, idx : idx + 1],
                )
            # combine: negm_all = min_idx(-m_idx/32) = -m_all/32;
            # scl_idx = exp((m_idx - m_all)/32)
            nc.vector.tensor_reduce(
                negm_all[:], negm[:], axis=mybir.AxisListType.X,
                op=mybir.AluOpType.min,
            )
            nc.scalar.activation(
                scl[:],
                mx[:],
                mybir.ActivationFunctionType.Exp,
                bias=negm_all[:, 0:1],
                scale=0.03125,
            )
            nc.vector.tensor_mul(sm[:], sm[:], scl[:])
            nc.vector.reduce_sum(rsum[:], sm[:], axis=mybir.AxisListType.X)
            nc.vector.reciprocal(rinv[:], rsum[:])
            for idx, (k0, w) in enumerate(ktiles):
                nc.vector.tensor_scalar_mul(
                    p_sb[:, k0 : k0 + w],
                    p32_sb[:, k0 : k0 + w],
                    scl[:, idx : idx + 1],
                )
            return {"qb": qb, "p_sb": p_sb, "rinv": rinv}

        def emit_pv(stt):
            qb = stt["qb"]
            p_sb = stt["p_sb"]
            rinv = stt["rinv"]
            L = (qb + 1) * P
            pt_sb = pt_pool.tile([P, L], dt, tag="pt")
            for c0 in range(0, qb + 1, 4):
                cn = min(4, qb + 1 - c0)
                ptp = ps_pt.tile([P, 4 * P], dt, tag="ptp")
                for j in range(cn):
                    kc = c0 + j
                    nc.tensor.transpose(
                        ptp[:, j * P : (j + 1) * P],
                        p_sb[:, kc * P : (kc + 1) * P],
                        consts_ident[:],
                    )
                nc.vector.tensor_copy(
                    pt_sb[:, c0 * P : c0 * P + cn * P], ptp[:, : cn * P]
                )

            o_sb = o_pool.tile([P, D_OUT], F32, tag="o")
            for o0 in range(0, D_OUT, MMW):
                pso = ps_o.tile([P, MMW], F32, tag="ps_o")
                for kc in range(qb + 1):
                    nc.tensor.matmul(
                        pso[:],
                        pt_sb[:, kc * P : (kc + 1) * P],
                        v_sb[:, kc, o0 : o0 + MMW],
                        start=(kc == 0),
                        stop=(kc == qb),
                    )
                nc.vector.tensor_scalar_mul(
                    o_sb[:, o0 : o0 + MMW], pso[:], rinv[:, 0:1]
                )
            nc.sync.dma_start(out_d[qb * P : (qb + 1) * P, :], o_sb[:])

        # two-block software pipeline: PV of the previous block hides the
        # softmax latency of the current one.  The four smallest blocks run
        # first (their PV is too short to hide a softmax), then largest-
        # first, so the exposed tail block still has a few-us PV.
        order = [3, 2, 1, 0] + list(range(QB - 1, 3, -1))
        prev = None
        for qb in order:
            stt = emit_scores(qb)
            if prev is not None:
                emit_pv(prev)
            prev = stt
        emit_pv(prev)


def build_program_fp32():
    nc = bass.Bass()
    x_d, wq_d, wk_d, wv_d, negmask_d, out_d = _declare_io(nc)
    qt_d = nc.dram_tensor("qt_scratch", [D_OUT, CTX], F32)
    kt_d = nc.dram_tensor("kt_scratch", [D_OUT, CTX], F32)
    v_d = nc.dram_tensor("v_scratch", [CTX, D_OUT], F32)

    with tile.TileContext(nc) as tc:
        with tc.tile_pool(name="consts", bufs=1) as consts:
            ident = consts.tile([P, P], F32)
            make_identity(nc, ident[:])
            negmask = consts.tile([P, P], F32)
            nc.sync.dma_start(negmask[:], negmask_d[:])

            with (
                tc.tile_pool(name="xt", bufs=1) as xt_pool,
                tc.tile_pool(name="w", bufs=2) as w_pool,
                tc.tile_pool(name="xs", bufs=3) as xs_pool,
                tc.tile_pool(name="stage", bufs=4) as stage_pool,
                tc.tile_pool(name="ps_proj", bufs=4, space="PSUM") as ps_proj,
                tc.tile_pool(name="ps_tr", bufs=4, space="PSUM") as ps_tr,
            ):
                xt = xt_pool.tile([P, IC, CTX], F32)
                for st in range(ST):
                    xs = xs_pool.tile([P, D_IN], F32, tag="xs")
                    nc.sync.dma_start(xs[:], x_d[st * P : (st + 1) * P, :])
                    for ic in range(IC):
                        pt = ps_tr.tile([P, P], F32, tag="pt")
                        nc.tensor.transpose(
                            pt[:], xs[:, ic * P : (ic + 1) * P], ident[:]
                        )
                        nc.vector.tensor_copy(xt[:, ic, st * P : (st + 1) * P], pt[:])

                for w_d, dst in ((wq_d, qt_d), (wk_d, kt_d)):
                    w_sb = w_pool.tile([P, IC, D_OUT], F32, tag="w")
                    nc.sync.dma_start(
                        w_sb[:], w_d[:].rearrange("(c p) o -> p c o", p=P)
                    )
                    for s0 in range(0, CTX, MMW):
                        for oc in range(OC):
                            ps = ps_proj.tile([P, MMW], F32, tag="ps")
                            for ic in range(IC):
                                nc.tensor.matmul(
                                    ps[:],
                                    w_sb[:, ic, oc * P : (oc + 1) * P],
                                    xt[:, ic, s0 : s0 + MMW],
                                    start=(ic == 0),
                                    stop=(ic == IC - 1),
                                )
                            sg = stage_pool.tile([P, MMW], F32, tag="sg")
                            nc.vector.tensor_copy(sg[:], ps[:])
                            nc.sync.dma_start(
                                dst[oc * P : (oc + 1) * P, s0 : s0 + MMW], sg[:]
                            )

                wv_sb = w_pool.tile([P, IC, D_OUT], F32, tag="w")
                nc.sync.dma_start(
                    wv_sb[:], wv_d[:].rearrange("(c p) o -> p c o", p=P)
                )
                for st in range(ST):
                    for o0 in range(0, D_OUT, MMW):
                        ps = ps_proj.tile([P, MMW], F32, tag="ps")
                        for ic in range(IC):
                            nc.tensor.matmul(
                                ps[:],
                                xt[:, ic, st * P : (st + 1) * P],
                                wv_sb[:, ic, o0 : o0 + MMW],
                                start=(ic == 0),
                                stop=(ic == IC - 1),
                            )
                        sg = stage_pool.tile([P, MMW], F32, tag="sg")
                        nc.vector.tensor_copy(sg[:], ps[:])
                        nc.sync.dma_start(
                            v_d[st * P : (st + 1) * P, o0 : o0 + MMW], sg[:]
                        )

            with (
                tc.tile_pool(name="kt", bufs=1) as kt_pool,
                tc.tile_pool(name="v", bufs=1) as v_pool,
                tc.tile_pool(name="qtb", bufs=2) as qtb_pool,
            ):
                kt_sb = kt_pool.tile([P, OC, CTX], F32)
                for c in range(OC):
                    nc.sync.dma_start(kt_sb[:, c, :], kt_d[c * P : (c + 1) * P, :])
                v_sb = v_pool.tile([P, ST, D_OUT], F32)
                for c in range(ST):
                    nc.sync.dma_start(v_sb[:, c, :], v_d[c * P : (c + 1) * P, :])

                def qt_src(qb):
                    qt_b = qtb_pool.tile([P, OC, P], F32, tag="qtb")
                    for oc in range(OC):
                        nc.sync.dma_start(
                            qt_b[:, oc, :],
                            qt_d[oc * P : (oc + 1) * P, qb * P : (qb + 1) * P],
                        )
                    return qt_b

                _attention_phase(
                    nc, tc, ident, negmask, qt_src, kt_sb, v_sb, out_d, F32
                )

    _split_multi_waits(nc)
    return nc


def build_program_fp16():
    """fp16 build: x^T and the weights are pre-cast/pre-transposed to fp16 on
    the HOST (pure layout prep; identical round-to-nearest as a DVE cast), so
    the device only does matmuls, softmax and the P transposes."""
    nc = bass.Bass()
    xt_d = nc.declare_dram_parameter("xT16", [D_IN, CTX], F16, isOutput=False)
    wq_d = nc.declare_dram_parameter("Wq16", [D_IN, D_OUT], F16, isOutput=False)
    wk_d = nc.declare_dram_parameter("Wk16", [D_IN, D_OUT], F16, isOutput=False)
    wv_d = nc.declare_dram_parameter("Wv16", [D_IN, D_OUT], F16, isOutput=False)
    negmask_d = nc.declare_dram_parameter("negmask", [P, P], F32, isOutput=False)
    out_d = nc.declare_dram_parameter("out", [CTX, D_OUT], F32, isOutput=True)

    with tile.TileContext(nc) as tc:
        with tc.tile_pool(name="consts", bufs=1) as consts:
            ident16 = consts.tile([P, P], F16)
            make_identity(nc, ident16[:])
            negmask = consts.tile([P, P], F32)
            nc.sync.dma_start(negmask[:], negmask_d[:])

            with (
                tc.tile_pool(name="qt", bufs=1) as qt_pool,
                tc.tile_pool(name="kt", bufs=1) as kt_pool,
                tc.tile_pool(name="v", bufs=1) as v_pool,
            ):
                qt_sb = qt_pool.tile([P, OC, CTX], F16)
                kt_sb = kt_pool.tile([P, OC, CTX], F16)
                v_sb = v_pool.tile([P, ST, D_OUT], F16)

                with (
                    tc.tile_pool(name="xt", bufs=1) as xt_pool,
                    tc.tile_pool(name="w", bufs=1) as w_pool,
                    tc.tile_pool(name="ps_proj", bufs=8, space="PSUM") as ps_proj,
                ):
                    # x^T arrives per (i-chunk, 512-col s-group); group-0
                    # chunks are queued BEFORE the weight loads so the first
                    # projection group only waits for ~1MB of x^T + 2MB of Wq.
                    xt = xt_pool.tile([P, IC, CTX], F16)
                    SG = MMW // P  # stripes per s-group

                    def load_xt_group(g):
                        s0 = g * MMW
                        for ic in range(IC):
                            nc.sync.dma_start(
                                xt[:, ic, s0 : s0 + MMW],
                                xt_d[ic * P : (ic + 1) * P, s0 : s0 + MMW],
                            )

                    wq_sb = w_pool.tile([P, IC, D_OUT], F16, tag="wq")
                    nc.sync.dma_start(wq_sb[:, 0, :], wq_d[0:P, :])
                    load_xt_group(0)
                    wk_sb = w_pool.tile([P, IC, D_OUT], F16, tag="wk")
                    wv_sb = w_pool.tile([P, IC, D_OUT], F16, tag="wv")
                    for ic in range(1, IC):
                        nc.sync.dma_start(
                            wq_sb[:, ic, :], wq_d[ic * P : (ic + 1) * P, :]
                        )
                    for w_d, w_sb in ((wk_d, wk_sb), (wv_d, wv_sb)):
                        for ic in range(IC):
                            nc.sync.dma_start(
                                w_sb[:, ic, :], w_d[ic * P : (ic + 1) * P, :]
                            )

                    for g in range(ST // SG):
                        s0 = g * MMW
                        if g > 0:
                            load_xt_group(g)
                        for dst, w_sb in ((qt_sb, wq_sb), (kt_sb, wk_sb)):
                            for oc in range(OC):
                                ps = ps_proj.tile([P, MMW], F32, tag="ps")
                                for ic in range(IC):
                                    nc.tensor.matmul(
                                        ps[:],
                                        w_sb[:, ic, oc * P : (oc + 1) * P],
                                        xt[:, ic, s0 : s0 + MMW],
                                        start=(ic == 0),
                                        stop=(ic == IC - 1),
                                    )
                                nc.vector.tensor_copy(
                                    dst[:, oc, s0 : s0 + MMW], ps[:]
                                )
                        for st in range(g * SG, (g + 1) * SG):
                            for o0 in range(0, D_OUT, MMW):
                                ps = ps_proj.tile([P, MMW], F32, tag="ps")
                                for ic in range(IC):
                                    nc.tensor.matmul(
                                        ps[:],
                                        xt[:, ic, st * P : (st + 1) * P],
                                        wv_sb[:, ic, o0 : o0 + MMW],
                                        start=(ic == 0),
                                        stop=(ic == IC - 1),
                                    )
                                nc.vector.tensor_copy(
                                    v_sb[:, st, o0 : o0 + MMW], ps[:]
                                )

                def qt_src(qb):
                    return qt_sb[:, :, qb * P : (qb + 1) * P]

                _attention_phase(
                    nc, tc, ident16, negmask, qt_src, kt_sb, v_sb, out_d, F16
                )

    _split_multi_waits(nc)
    return nc


def _attention_phase_hybrid(
    nc, tc, ident16, negmask, qt16, kt16, v16, qt8, kt8, v8, out_d, dbg=None
):
    """Causal attention with a per-q-block dtype split: blocks < R16 run the
    fp16 path (qt16/kt16/v16), blocks >= R16 run fp8 DoubleRow matmuls
    (qt8/kt8/v8, 0.5 cyc/row).  Softmax is fp32 either way; P is built fp16,
    transposed fp16 on the PE, and cast to fp8 on the PSUM->SBUF copy for the
    fp8 blocks."""
    with (
        tc.tile_pool(name="pexp", bufs=3) as p_pool,
        tc.tile_pool(name="ptr16", bufs=2) as pt16_pool,
        tc.tile_pool(name="ptr8", bufs=2) as pt8_pool,
        tc.tile_pool(name="red", bufs=3) as red_pool,
        tc.tile_pool(name="ob", bufs=3) as o_pool,
        tc.tile_pool(name="ps_s", bufs=4, space="PSUM") as ps_s,
        tc.tile_pool(name="ps_o", bufs=2, space="PSUM") as ps_o,
        tc.tile_pool(name="ps_pt", bufs=2, space="PSUM") as ps_pt,
    ):

        def emit_scores(qb):
            """Single-pass softmax: score tiles stay in PSUM until the
            block max is known, then one exp pass per tile writes fp16 P
            (x PSCALE on the fp8 path) with accum_out row sums."""
            L = (qb + 1) * P
            ktiles = [(k0, min(MMW, L - k0)) for k0 in range(0, L, MMW)]
            nkt = len(ktiles)
            fp8 = qb >= R16
            sc = SC8 if fp8 else SC16

            red = red_pool.tile([P, 2 * nkt + 4], F32, tag="red")
            mx = red[:, 0:nkt]
            sm = red[:, nkt : 2 * nkt]
            mx_all = red[:, 2 * nkt : 2 * nkt + 1]
            negm_all = red[:, 2 * nkt + 1 : 2 * nkt + 2]
            rsum = red[:, 2 * nkt + 2 : 2 * nkt + 3]
            rinv = red[:, 2 * nkt + 3 : 2 * nkt + 4]

            p_sb = p_pool.tile([P, CTX], F16, tag="p")
            pss = []
            for idx, (k0, w) in enumerate(ktiles):
                ps = ps_s.tile([P, MMW], F32, tag="ps_s")
                pss.append(ps)
                if not fp8:
                    for oc in range(OC):
                        nc.tensor.matmul(
                            ps[:, :w],
                            qt16[:, oc, qb * P : (qb + 1) * P],
                            kt16[:, oc, k0 : k0 + w],
                            start=(oc == 0),
                            stop=(oc == OC - 1),
                        )
                else:
                    for c in range(OC // 2):
                        nc.tensor.matmul(
                            ps[:, :w],
                            qt8[:, 2 * c : 2 * c + 2, qb * P : (qb + 1) * P],
                            kt8[:, 2 * c : 2 * c + 2, k0 : k0 + w],
                            start=(c == 0),
                            stop=(c == OC // 2 - 1),
                            perf_mode=DR,
                        )
                if idx == nkt - 1:
                    nc.vector.tensor_add(
                        ps[:, w - P : w], ps[:, w - P : w], negmask[:]
                    )
                nc.vector.reduce_max(
                    mx[:, idx : idx + 1], ps[:, :w], axis=mybir.AxisListType.X
                )
            if nkt > 1:
                nc.vector.tensor_reduce(
                    mx_all[:], mx[:], axis=mybir.AxisListType.X,
                    op=mybir.AluOpType.max,
                )
            else:
                mx_all = mx
            nc.scalar.mul(negm_all[:], mx_all[:, 0:1], -sc)
            if fp8:
                # fold ln(PSCALE) into the exp bias: P comes out x64, past
                # e4m3's subnormal range; rsum picks up the same factor and
                # with v arriving x32 the normalization needs a further 1/32.
                nc.vector.tensor_scalar_add(
                    negm_all[:], negm_all[:], float(np.log(PSCALE))
                )
            for idx, (k0, w) in enumerate(ktiles):
                nc.scalar.activation(
                    p_sb[:, k0 : k0 + w],
                    pss[idx][:, :w],
                    mybir.ActivationFunctionType.Exp,
                    bias=negm_all[:, 0:1],
                    scale=sc,
                    accum_out=sm[:, idx : idx + 1],
                )
            if nkt > 1:
                nc.vector.reduce_sum(
                    rsum[:], sm[:], axis=mybir.AxisListType.X
                )
            else:
                rsum = sm
            nc.vector.reciprocal(rinv[:], rsum[:])
            if fp8:
                nc.scalar.mul(rinv[:], rinv[:], 1.0 / WSCALE)
            if dbg is not None and qb == DEBUG_QB:
                nc.sync.dma_start(dbg["p16"][:, :L], p_sb[:, :L])
                nc.sync.dma_start(dbg["red"][:], red[:])
            return {"qb": qb, "p_sb": p_sb, "rinv": rinv}

        def emit_pv(stt):
            qb = stt["qb"]
            p_sb = stt["p_sb"]
            rinv = stt["rinv"]
            nch = qb + 1
            fp8 = qb >= R16
            if fp8:
                pt_sb = pt8_pool.tile([P, ST, P], F8, tag="pt8")
            else:
                pt_sb = pt16_pool.tile([P, R16, P], F16, tag="pt16")
            for c0 in range(0, nch, 4):
                cn = min(4, nch - c0)
                ptp = ps_pt.tile([P, 4 * P], F16, tag="ptp")
                for j in range(cn):
                    kc = c0 + j
                    nc.tensor.transpose(
                        ptp[:, j * P : (j + 1) * P],
                        p_sb[:, kc * P : (kc + 1) * P],
                        ident16[:],
                    )
                nc.scalar.mul(
                    pt_sb[:, c0 : c0 + cn, :], ptp[:, : cn * P], 1.0
                )

            if dbg is not None and qb == DEBUG_QB:
                nc.sync.dma_start(
                    dbg["pt8"][:, : nch * P],
                    pt_sb[:, 0:nch, :],
                )
            o_sb = o_pool.tile([P, D_OUT], F32, tag="o")
            for o0 in range(0, D_OUT, MMW):
                pso = ps_o.tile([P, MMW], F32, tag="ps_o")
                if fp8:
                    npair = nch // 2
                    for c in range(npair):
                        nc.tensor.matmul(
                            pso[:],
                            pt_sb[:, 2 * c : 2 * c + 2, :],
                            v8[:, 2 * c : 2 * c + 2, o0 : o0 + MMW],
                            start=(c == 0),
                            stop=(c == npair - 1 and nch % 2 == 0),
                            perf_mode=DR,
                        )
                    if nch % 2 == 1:
                        nc.tensor.matmul(
                            pso[:],
                            pt_sb[:, nch - 1, :],
                            v8[:, nch - 1, o0 : o0 + MMW],
                            start=False,
                            stop=True,
                        )
                else:
                    for kc in range(nch):
                        nc.tensor.matmul(
                            pso[:],
                            pt_sb[:, kc, :],
                            v16[:, kc, o0 : o0 + MMW],
                            start=(kc == 0),
                            stop=(kc == nch - 1),
                        )
                nc.vector.tensor_scalar_mul(
                    o_sb[:, o0 : o0 + MMW], pso[:], rinv[:, 0:1]
                )
            nc.sync.dma_start(out_d[qb * P : (qb + 1) * P, :], o_sb[:])

        # three-stage software pipeline: while block n's softmax chain runs
        # on DVE/ACT, the PE executes PV(n-2) and the scores of n+1.  Tiny
        # blocks bracket the pipeline so its exposed fill/drain is cheap.
        order = [3, 2] + list(range(QB - 1, 3, -1)) + [1, 0]
        pend = []
        for qb in order:
            pend.append(emit_scores(qb))
            if len(pend) > 2:
                emit_pv(pend.pop(0))
        for stt in pend:
            emit_pv(stt)


def build_program_fp8():
    """Hybrid fp16/fp8 build.  Rows < RS go through the fp16 pipeline
    (projections and attention), rows >= RS through fp8 DoubleRow matmuls
    (2x PE throughput).  Early K/V are cast fp16->fp8 on the DVE so late
    blocks can consume them in DoubleRow mode.  Host pre-casts x^T and the
    weights to fp16 and fp8 (pure dtype/layout prep, same round-to-nearest
    as a DVE cast)."""
    nc = bass.Bass()
    xt16_d = nc.declare_dram_parameter("xT16pre", [D_IN, RS], F16, isOutput=False)
    xt8_d = nc.declare_dram_parameter("xT8post", [D_IN, CTX - RS], F8, isOutput=False)
    wq16_d = nc.declare_dram_parameter("Wq16", [D_IN, D_OUT], F16, isOutput=False)
    wk16_d = nc.declare_dram_parameter("Wk16", [D_IN, D_OUT], F16, isOutput=False)
    wv16_d = nc.declare_dram_parameter("Wv16", [D_IN, D_OUT], F16, isOutput=False)
    wq8_d = nc.declare_dram_parameter("Wq8", [D_IN, D_OUT], F8, isOutput=False)
    wk8_d = nc.declare_dram_parameter("Wk8", [D_IN, D_OUT], F8, isOutput=False)
    wv8_d = nc.declare_dram_parameter("Wv8", [D_IN, D_OUT], F8, isOutput=False)
    negmask_d = nc.declare_dram_parameter("negmask", [P, P], F32, isOutput=False)
    out_d = nc.declare_dram_parameter("out", [CTX, D_OUT], F32, isOutput=True)
    dbg = None
    if DEBUG_DUMP:
        LD = (DEBUG_QB + 1) * P
        dbg = {
            "qt8": nc.declare_dram_parameter(
                "dbg_qt8", [D_OUT, CTX - RS], F8, isOutput=True
            ),
            "kt8": nc.declare_dram_parameter(
                "dbg_kt8", [D_OUT, CTX], F8, isOutput=True
            ),
            "v8": nc.declare_dram_parameter(
                "dbg_v8", [CTX, D_OUT], F8, isOutput=True
            ),
            "p32": nc.declare_dram_parameter(
                "dbg_p32", [P, LD], F32, isOutput=True
            ),
            "p16": nc.declare_dram_parameter(
                "dbg_p16", [P, LD], F16, isOutput=True
            ),
            "pt8": nc.declare_dram_parameter(
                "dbg_pt8", [P, LD], F8, isOutput=True
            ),
            "red": nc.declare_dram_parameter(
                "dbg_red", [P, 4 * 2 + 3], F32, isOutput=True
            ),
        }

    with tile.TileContext(nc) as tc:
        with tc.tile_pool(name="consts", bufs=1) as consts:
            ident16 = consts.tile([P, P], F16)
            make_identity(nc, ident16[:])
            negmask = consts.tile([P, P], F32)
            nc.sync.dma_start(negmask[:], negmask_d[:])

            with (
                tc.tile_pool(name="qt16", bufs=1) as qt16_pool,
                tc.tile_pool(name="kt16", bufs=1) as kt16_pool,
                tc.tile_pool(name="v16", bufs=1) as v16_pool,
                tc.tile_pool(name="qt8", bufs=1) as qt8_pool,
                tc.tile_pool(name="kt8", bufs=1) as kt8_pool,
                tc.tile_pool(name="v8", bufs=1) as v8_pool,
                tc.tile_pool(name="w8", bufs=1) as w8_pool,
                tc.tile_pool(name="xt8", bufs=1) as xt8_pool,
            ):
                qt16 = qt16_pool.tile([P, OC, RS], F16)
                kt16 = kt16_pool.tile([P, OC, RS], F16)
                v16 = v16_pool.tile([P, R16, D_OUT], F16)
                qt8 = qt8_pool.tile([P, OC, CTX], F8)
                kt8 = kt8_pool.tile([P, OC, CTX], F8)
                v8 = v8_pool.tile([P, ST, D_OUT], F8)
                wq8 = w8_pool.tile([P, IC, D_OUT], F8, tag="wq8")
                wk8 = w8_pool.tile([P, IC, D_OUT], F8, tag="wk8")
                wv8 = w8_pool.tile([P, IC, D_OUT], F8, tag="wv8")
                xt8 = xt8_pool.tile([P, IC, CTX], F8)

                # ---- fp8 DoubleRow projections for rows >= RS (first: they
                # need only ~4.4MB of DMA, so the PE starts almost
                # immediately; the 7.3MB fp16-phase tensors stream behind)
                sgroups = []
                s0 = RS
                while s0 < CTX:
                    w = min(MMW, CTX - s0)
                    sgroups.append((s0, w))
                    s0 += w
                with tc.tile_pool(name="ps_p8", bufs=8, space="PSUM") as ps_p8:
                    # one DMA descriptor per tensor/group: each dma_start
                    # costs ~0.6us of sync-queue issue time, so per-chunk
                    # descriptors would stall the PE at startup
                    nc.sync.dma_start(
                        wq8[:], wq8_d[:].rearrange("(c p) o -> p c o", p=P)
                    )
                    s0, w = sgroups[0]
                    nc.sync.dma_start(
                        xt8[:, :, s0 : s0 + w],
                        xt8_d[:, : s0 + w - RS].rearrange(
                            "(c p) s -> p c s", p=P
                        ),
                    )
                    for w_d, w_sb in ((wk8_d, wk8), (wv8_d, wv8)):
                        nc.sync.dma_start(
                            w_sb[:], w_d[:].rearrange("(c p) o -> p c o", p=P)
                        )
                    for s0, w in sgroups[1:]:
                        nc.sync.dma_start(
                            xt8[:, :, s0 : s0 + w],
                            xt8_d[:, s0 - RS : s0 + w - RS].rearrange(
                                "(c p) s -> p c s", p=P
                            ),
                        )
                    # PSUM->SBUF casts alternate between DVE and ACT: a
                    # [128,512] cast (~0.6us) costs more than the 4 paired
                    # DR matmuls feeding it (~0.43us), so a single engine
                    # would gate the PE here.
                    for s0, w in sgroups:
                        for dst, w_sb in ((qt8, wq8), (kt8, wk8)):
                            for oc in range(OC):
                                ps = ps_p8.tile([P, MMW], F32, tag="ps")
                                for c in range(IC // 2):
                                    nc.tensor.matmul(
                                        ps[:, :w],
                                        w_sb[
                                            :,
                                            2 * c : 2 * c + 2,
                                            oc * P : (oc + 1) * P,
                                        ],
                                        xt8[:, 2 * c : 2 * c + 2, s0 : s0 + w],
                                        start=(c == 0),
                                        stop=(c == IC // 2 - 1),
                                        perf_mode=DR,
                                    )
                                if oc % 2 == 0:
                                    nc.vector.tensor_copy(
                                        dst[:, oc, s0 : s0 + w], ps[:, :w]
                                    )
                                else:
                                    nc.scalar.mul(
                                        dst[:, oc, s0 : s0 + w], ps[:, :w], 1.0
                                    )
                        for st in range(s0 // P, (s0 + w) // P):
                            for o0 in range(0, D_OUT, MMW):
                                ps = ps_p8.tile([P, MMW], F32, tag="ps")
                                for c in range(IC // 2):
                                    nc.tensor.matmul(
                                        ps[:],
                                        xt8[
                                            :,
                                            2 * c : 2 * c + 2,
                                            st * P : (st + 1) * P,
                                        ],
                                        wv8[:, 2 * c : 2 * c + 2, o0 : o0 + MMW],
                                        start=(c == 0),
                                        stop=(c == IC // 2 - 1),
                                        perf_mode=DR,
                                    )
                                if (st + o0 // MMW) % 2 == 0:
                                    nc.vector.tensor_copy(
                                        v8[:, st, o0 : o0 + MMW], ps[:]
                                    )
                                else:
                                    nc.scalar.mul(
                                        v8[:, st, o0 : o0 + MMW], ps[:], 1.0
                                    )

                # ---- fp16 projections for rows < RS ----
                with (
                    tc.tile_pool(name="w16", bufs=1) as w16_pool,
                    tc.tile_pool(name="xt16", bufs=1) as xt16_pool,
                    tc.tile_pool(name="ps_p16", bufs=8, space="PSUM") as ps_p16,
                ):
                    wq16 = w16_pool.tile([P, IC, D_OUT], F16, tag="wq16")
                    wk16 = w16_pool.tile([P, IC, D_OUT], F16, tag="wk16")
                    wv16 = w16_pool.tile([P, IC, D_OUT], F16, tag="wv16")
                    xt16 = xt16_pool.tile([P, IC, RS], F16)
                    nc.sync.dma_start(
                        xt16[:], xt16_d[:].rearrange("(c p) s -> p c s", p=P)
                    )
                    for w_d, w_sb in (
                        (wq16_d, wq16),
                        (wk16_d, wk16),
                        (wv16_d, wv16),
                    ):
                        nc.sync.dma_start(
                            w_sb[:], w_d[:].rearrange("(c p) o -> p c o", p=P)
                        )

                    for dst, w_sb in ((qt16, wq16), (kt16, wk16)):
                        for s0, w in ((0, RS // 2), (RS // 2, RS // 2)):
                            for oc in range(OC):
                                ps = ps_p16.tile([P, MMW], F32, tag="ps")
                                for ic in range(IC):
                                    nc.tensor.matmul(
                                        ps[:, :w],
                                        w_sb[:, ic, oc * P : (oc + 1) * P],
                                        xt16[:, ic, s0 : s0 + w],
                                        start=(ic == 0),
                                        stop=(ic == IC - 1),
                                    )
                                nc.vector.tensor_copy(
                                    dst[:, oc, s0 : s0 + w], ps[:, :w]
                                )
                    for st in range(R16):
                        for o0 in range(0, D_OUT, MMW):
                            ps = ps_p16.tile([P, MMW], F32, tag="ps")
                            for ic in range(IC):
                                nc.tensor.matmul(
                                    ps[:],
                                    xt16[:, ic, st * P : (st + 1) * P],
                                    wv16[:, ic, o0 : o0 + MMW],
                                    start=(ic == 0),
                                    stop=(ic == IC - 1),
                                )
                            nc.vector.tensor_copy(
                                v16[:, st, o0 : o0 + MMW], ps[:]
                            )
                    # early K/V cast to fp8 (x32, matching the scaled fp8
                    # projections) for the late fp8 blocks
                    for oc in range(OC):
                        nc.scalar.mul(kt8[:, oc, 0:RS], kt16[:, oc, :], WSCALE)
                    for st in range(R16):
                        nc.scalar.mul(v8[:, st, :], v16[:, st, :], WSCALE)

                if dbg is not None:
                    for oc in range(OC):
                        nc.sync.dma_start(
                            dbg["qt8"][oc * P : (oc + 1) * P, :],
                            qt8[:, oc, RS:CTX],
                        )
                        nc.sync.dma_start(
                            dbg["kt8"][oc * P : (oc + 1) * P, :], kt8[:, oc, :]
                        )
                    for st in range(ST):
                        nc.sync.dma_start(
                            dbg["v8"][st * P : (st + 1) * P, :], v8[:, st, :]
                        )

                _attention_phase_hybrid(
                    nc, tc, ident16, negmask, qt16, kt16, v16, qt8, kt8, v8,
                    out_d, dbg,
                )

    _split_multi_waits(nc)
    return nc


_program_cache = {}


def build_program(mode=None):
    mode = mode or MODE
    if mode == "fp32":
        return build_program_fp32()
    elif mode == "fp16":
        return build_program_fp16()
    elif mode == "fp8":
        return build_program_fp8()
    raise ValueError(mode)


def make_in_maps(x, Wq, Wk, Wv):
    x = np.ascontiguousarray(np.asarray(x), dtype=np.float32)
    Wq = np.ascontiguousarray(np.asarray(Wq), dtype=np.float32)
    Wk = np.ascontiguousarray(np.asarray(Wk), dtype=np.float32)
    Wv = np.ascontiguousarray(np.asarray(Wv), dtype=np.float32)

    iu = np.triu(np.ones((P, P), dtype=np.float32), k=1)
    negmask = (iu * NEG_BIG).astype(np.float32)

    if MODE == "fp8":
        import ml_dtypes

        F8NP = ml_dtypes.float8_e4m3
        xT = np.transpose(x, (0, 2, 1))  # [b, d_in, ctx]
        xT16pre = np.ascontiguousarray(xT[:, :, :RS].astype(np.float16))
        xT8post = np.ascontiguousarray(xT[:, :, RS:].astype(F8NP))
        wq16 = np.ascontiguousarray(Wq.astype(np.float16))
        wk16 = np.ascontiguousarray(Wk.astype(np.float16))
        wv16 = np.ascontiguousarray(Wv.astype(np.float16))
        ws = np.float32(WSCALE)
        wq8 = np.ascontiguousarray((Wq * ws).astype(F8NP))
        wk8 = np.ascontiguousarray((Wk * ws).astype(F8NP))
        wv8 = np.ascontiguousarray((Wv * ws).astype(F8NP))
        in_maps = [
            {
                "xT16pre": xT16pre[b],
                "xT8post": xT8post[b],
                "Wq16": wq16,
                "Wk16": wk16,
                "Wv16": wv16,
                "Wq8": wq8,
                "Wk8": wk8,
                "Wv8": wv8,
                "negmask": negmask,
            }
            for b in range(BATCH)
        ]
    elif MODE == "fp16":
        # host-side layout prep: fp16 round-to-nearest (same as a DVE cast)
        # and the x transpose the device would otherwise do on the PE
        xT16 = np.ascontiguousarray(
            np.transpose(x.astype(np.float16), (0, 2, 1))
        )
        wq16 = np.ascontiguousarray(Wq.astype(np.float16))
        wk16 = np.ascontiguousarray(Wk.astype(np.float16))
        wv16 = np.ascontiguousarray(Wv.astype(np.float16))
        in_maps = [
            {
                "xT16": xT16[b],
                "Wq16": wq16,
                "Wk16": wk16,
                "Wv16": wv16,
                "negmask": negmask,
            }
            for b in range(BATCH)
        ]
    else:
        in_maps = [
            {"x": x[b], "Wq": Wq, "Wk": Wk, "Wv": Wv, "negmask": negmask}
            for b in range(BATCH)
        ]
    return in_maps


def kernel(x, Wq, Wk, Wv):
    from concourse.bass_utils import run_bass_kernel_spmd

    if MODE not in _program_cache:
        _program_cache[MODE] = build_program(MODE)
    nc = _program_cache[MODE]

    in_maps = make_in_maps(x, Wq, Wk, Wv)
    res = run_bass_kernel_spmd(nc, in_maps, list(range(N_CORES)))
    return np.stack([res.results[b]["out"] for b in range(BATCH)], axis=0)



# revision 28
# speedup vs baseline: 1.1813x; 1.0089x over previous
"""Causal single-head attention (batch=8, ctx=2048, d=1024) on 8 trn2 cores.

Strategy: pure data-parallel over batch — core b computes attention for
batch element b with no cross-core communication.

Per-core pipeline:
  phase 1: Q^T, K^T (o-major) and V (s-major) projections accumulated in
           PSUM (fp32), consumed per 512-column s-group of x^T.
  phase 2: flash-style causal attention per 128-row q-block:
           S = Q^T.T @ K^T, additive causal mask on the diagonal
           128x128 sub-tile, one-pass softmax (per-tile exp(s - m_tile)
           on ACT with row-sum accumulators, exp(m_tile - m_all)
           correction folded into P), P transposed per tile on the PE,
           O = P @ V accumulated in PSUM, deferred normalization by the
           reciprocal row sum, DMA out (fp32); two-block software
           pipeline so PV of one block hides the next one's softmax.

MODE selects the matmul input dtype:
  "fp32": all matmul inputs fp32 (4 cyc/row); x^T built on-device via PE
          transposes; Q^T/K^T/V staged through DRAM scratch (SBUF can't
          hold x^T + all three in fp32).  ~1.25ms, rel err ~6e-6.
  "fp16": matmul inputs fp16 (1 cyc/row), fp32 PSUM accumulation and
          softmax; x^T and the weights are pre-cast/pre-transposed on the
          host (pure layout prep, bit-identical to a DVE cast) and
          everything stays resident in SBUF.  ~332us, rel err ~5e-4.
"""

import sys

sys.path.insert(0, "/opt/trn_rl_repo")

import numpy as np

import concourse.bass as bass
import concourse.mybir as mybir
import concourse.tile as tile
from concourse.masks import make_identity
from concourse.vector_clock import ScopedClock

MODE = "fp8"

BATCH = 8
CTX = 2048
D_IN = 1024
D_OUT = 1024
N_CORES = 8
P = 128
F32 = mybir.dt.float32
F16 = mybir.dt.float16
F8 = mybir.dt.float8e4
DR = mybir.MatmulPerfMode.DoubleRow
NEG_BIG = -1.0e30
R16 = 5  # q/seq blocks 0..R16-1 use the fp16 path; the rest fp8 DoubleRow
RS = R16 * P
# e4m3's normal range starts at 2^-6; the weights (std 1/32) and softmax
# probs sit mostly below it, where quantization is coarse (and the PE
# appears to flush subnormals).  Scale W by 32 on the host (so q,k,v land
# in PSUM pre-scaled by 32) and P by 64 on the device; the inverse scales
# fold into the exp logit scale and the output normalization for free.
WSCALE = 32.0  # host: W8 = e4m3(W * 32) -> q,k,v arrive x32
PSCALE = 64.0  # device: P8 = e4m3(P * 64)
SC16 = 0.03125  # logit scale for the fp16 path: 1/sqrt(1024)
SC8 = 0.03125 / (WSCALE * WSCALE)  # fp8 path: logits arrive x1024
DEBUG_DUMP = False  # extra DRAM outputs for per-stage error attribution
DEBUG_QB = 6

# ---------------------------------------------------------------------------
# Workarounds for the walrus build on this stack: it rejects any instruction
# carrying more than ONE sync wait.  (1) Patch the TileContext tail drain to
# spread its waits over preceding sync-engine nops; (2) post-pass that hoists
# extra waits from any instruction onto same-engine nops inserted right
# before it (sequencers execute per-engine streams in order, so this is
# semantics-preserving).
# ---------------------------------------------------------------------------


def _patched_drain_and_barrier(self, tick_clock, wait_clock):
    nc = self.nc
    nops = [nc.sync.nop(nofuse=True) for _ in range(27)]
    drain_inst = nc.sync.drain()
    wait_clock.add_sem_waits(
        drain_inst.ins, ScopedClock({None: tick_clock.global_clock})
    )
    si = drain_inst.ins.sync_info
    if si is not None and si.on_wait is not None and len(si.on_wait) > 1:
        waits = list(si.on_wait)
        si.on_wait = waits[:1]
        rest = waits[1:]
        for i, nop in enumerate(nops):
            chunk = rest[i : i + 1]
            if not chunk:
                break
            nsi = nop.ins.sync_info
            if nsi is None:
                nop.ins.sync_info = mybir.SyncInfo(on_wait=chunk, on_update=[])
            else:
                nsi.on_wait = chunk

    nc.all_engine_barrier()
    assert self.sems is not None
    popped = nc._tile_sem_poison_stack.pop()
    assert popped is self._sem_poison
    nc.clear_and_free_semaphores(list(self.sems.allocated().values()))
    nc.all_engine_barrier()


tile.TileContext._drain_and_barrier = _patched_drain_and_barrier


def _split_multi_waits(nc):
    n_split = 0
    for f in nc.m.functions:
        for bb in f.blocks:
            il = bb.instructions
            if not any(
                inst.sync_info is not None
                and inst.sync_info.on_wait
                and len(inst.sync_info.on_wait) > 1
                for inst in il
            ):
                continue
            new = []
            for inst in il:
                si = inst.sync_info
                if si is not None and si.on_wait and len(si.on_wait) > 1:
                    waits = list(si.on_wait)
                    for w in waits[:-1]:
                        nop = mybir.InstNoOp(
                            name=nc.get_next_instruction_name(), ins=[], outs=[]
                        )
                        nop.engine = inst.engine
                        nop.sync_info = mybir.SyncInfo(on_wait=[w], on_update=[])
                        new.append(nop)
                        n_split += 1
                    si.on_wait = [waits[-1]]
                new.append(inst)
            il[:] = new
    return n_split


# ---------------------------------------------------------------------------
# Program builders
# ---------------------------------------------------------------------------

IC = D_IN // P  # 8 input-dim chunks
OC = D_OUT // P  # 8 output-dim chunks
ST = CTX // P  # 16 seq chunks
QB = CTX // P  # 16 q blocks
MMW = 512  # moving width (psum bank = 512 fp32)


def _declare_io(nc):
    x_d = nc.declare_dram_parameter("x", [CTX, D_IN], F32, isOutput=False)
    wq_d = nc.declare_dram_parameter("Wq", [D_IN, D_OUT], F32, isOutput=False)
    wk_d = nc.declare_dram_parameter("Wk", [D_IN, D_OUT], F32, isOutput=False)
    wv_d = nc.declare_dram_parameter("Wv", [D_IN, D_OUT], F32, isOutput=False)
    negmask_d = nc.declare_dram_parameter("negmask", [P, P], F32, isOutput=False)
    out_d = nc.declare_dram_parameter("out", [CTX, D_OUT], F32, isOutput=True)
    return x_d, wq_d, wk_d, wv_d, negmask_d, out_d


def _attention_phase(nc, tc, consts_ident, negmask, qt_src, kt_sb, v_sb, out_d, dt):
    """qt_src(qb) -> [P, OC, P] tile of Q^T for that block.

    One-pass softmax: each score tile gets exp(s - m_tile) immediately
    (ACT, off the PE critical path); after the block's tiles are done the
    per-tile correction exp(m_tile - m_all) is folded into the 16-bit P
    tiles and the row-sum."""
    with (
        tc.tile_pool(name="pexp", bufs=3) as p_pool,
        tc.tile_pool(name="pexp32", bufs=3) as p32_pool,
        tc.tile_pool(name="ptr", bufs=2) as pt_pool,
        tc.tile_pool(name="red", bufs=3) as red_pool,
        tc.tile_pool(name="ob", bufs=3) as o_pool,
        tc.tile_pool(name="ps_s", bufs=4, space="PSUM") as ps_s,
        tc.tile_pool(name="ps_o", bufs=2, space="PSUM") as ps_o,
        tc.tile_pool(name="ps_pt", bufs=2, space="PSUM") as ps_pt,
    ):

        def emit_scores(qb):
            L = (qb + 1) * P
            ktiles = [(k0, min(MMW, L - k0)) for k0 in range(0, L, MMW)]
            nkt = len(ktiles)

            qt_b = qt_src(qb)

            red = red_pool.tile([P, 4 * nkt + 3], F32, tag="red")
            mx = red[:, 0:nkt]
            negm = red[:, nkt : 2 * nkt]
            sm = red[:, 2 * nkt : 3 * nkt]
            scl = red[:, 3 * nkt : 4 * nkt]
            negm_all = red[:, 4 * nkt : 4 * nkt + 1]
            rsum = red[:, 4 * nkt + 1 : 4 * nkt + 2]
            rinv = red[:, 4 * nkt + 2 : 4 * nkt + 3]

            p_sb = p_pool.tile([P, CTX], dt, tag="p")
            p32_sb = p32_pool.tile([P, CTX], F32, tag="p32")
            for idx, (k0, w) in enumerate(ktiles):
                ps = ps_s.tile([P, MMW], F32, tag="ps_s")
                for oc in range(OC):
                    nc.tensor.matmul(
                        ps[:, :w],
                        qt_b[:, oc, :],
                        kt_sb[:, oc, k0 : k0 + w],
                        start=(oc == 0),
                        stop=(oc == OC - 1),
                    )
                if idx == nkt - 1:
                    nc.vector.tensor_add(
                        ps[:, w - P : w], ps[:, w - P : w], negmask[:]
                    )
                nc.vector.reduce_max(
                    mx[:, idx : idx + 1], ps[:, :w], axis=mybir.AxisListType.X
                )
                nc.scalar.mul(
                    negm[:, idx : idx + 1], mx[:, idx : idx + 1], -0.03125
                )
                # exp(s - m_tile) immediately; row-sums into sm[idx]
                nc.scalar.activation(
                    p32_sb[:, k0 : k0 + w],
                    ps[:, :w],
                    mybir.ActivationFunctionType.Exp,
                    bias=negm[:, idx : idx + 1],
                    scale=0.03125,
                    accum_out=sm[:, idx : idx + 1],
                )
            # combine: negm_all = min_idx(-m_idx/32) = -m_all/32;
            # scl_idx = exp((m_idx - m_all)/32)
            nc.vector.tensor_reduce(
                negm_all[:], negm[:], axis=mybir.AxisListType.X,
                op=mybir.AluOpType.min,
            )
            nc.scalar.activation(
                scl[:],
                mx[:],
                mybir.ActivationFunctionType.Exp,
                bias=negm_all[:, 0:1],
                scale=0.03125,
            )
            nc.vector.tensor_mul(sm[:], sm[:], scl[:])
            nc.vector.reduce_sum(rsum[:], sm[:], axis=mybir.AxisListType.X)
            nc.vector.reciprocal(rinv[:], rsum[:])
            for idx, (k0, w) in enumerate(ktiles):
                nc.vector.tensor_scalar_mul(
                    p_sb[:, k0 : k0 + w],
                    p32_sb[:, k0 : k0 + w],
                    scl[:, idx : idx + 1],
                )
            return {"qb": qb, "p_sb": p_sb, "rinv": rinv}

        def emit_pv(stt):
            qb = stt["qb"]
            p_sb = stt["p_sb"]
            rinv = stt["rinv"]
            L = (qb + 1) * P
            pt_sb = pt_pool.tile([P, L], dt, tag="pt")
            for c0 in range(0, qb + 1, 4):
                cn = min(4, qb + 1 - c0)
                ptp = ps_pt.tile([P, 4 * P], dt, tag="ptp")
                for j in range(cn):
                    kc = c0 + j
                    nc.tensor.transpose(
                        ptp[:, j * P : (j + 1) * P],
                        p_sb[:, kc * P : (kc + 1) * P],
                        consts_ident[:],
                    )
                nc.vector.tensor_copy(
                    pt_sb[:, c0 * P : c0 * P + cn * P], ptp[:, : cn * P]
                )

            o_sb = o_pool.tile([P, D_OUT], F32, tag="o")
            for o0 in range(0, D_OUT, MMW):
                pso = ps_o.tile([P, MMW], F32, tag="ps_o")
                for kc in range(qb + 1):
                    nc.tensor.matmul(
                        pso[:],
                        pt_sb[:, kc * P : (kc + 1) * P],
                        v_sb[:, kc, o0 : o0 + MMW],
                        start=(kc == 0),
                        stop=(kc == qb),
                    )
                nc.vector.tensor_scalar_mul(
                    o_sb[:, o0 : o0 + MMW], pso[:], rinv[:, 0:1]
                )
            nc.sync.dma_start(out_d[qb * P : (qb + 1) * P, :], o_sb[:])

        # two-block software pipeline: PV of the previous block hides the
        # softmax latency of the current one.  The four smallest blocks run
        # first (their PV is too short to hide a softmax), then largest-
        # first, so the exposed tail block still has a few-us PV.
        order = [3, 2, 1, 0] + list(range(QB - 1, 3, -1))
        prev = None
        for qb in order:
            stt = emit_scores(qb)
            if prev is not None:
                emit_pv(prev)
            prev = stt
        emit_pv(prev)


def build_program_fp32():
    nc = bass.Bass()
    x_d, wq_d, wk_d, wv_d, negmask_d, out_d = _declare_io(nc)
    qt_d = nc.dram_tensor("qt_scratch", [D_OUT, CTX], F32)
    kt_d = nc.dram_tensor("kt_scratch", [D_OUT, CTX], F32)
    v_d = nc.dram_tensor("v_scratch", [CTX, D_OUT], F32)

    with tile.TileContext(nc) as tc:
        with tc.tile_pool(name="consts", bufs=1) as consts:
            ident = consts.tile([P, P], F32)
            make_identity(nc, ident[:])
            negmask = consts.tile([P, P], F32)
            nc.sync.dma_start(negmask[:], negmask_d[:])

            with (
                tc.tile_pool(name="xt", bufs=1) as xt_pool,
                tc.tile_pool(name="w", bufs=2) as w_pool,
                tc.tile_pool(name="xs", bufs=3) as xs_pool,
                tc.tile_pool(name="stage", bufs=4) as stage_pool,
                tc.tile_pool(name="ps_proj", bufs=4, space="PSUM") as ps_proj,
                tc.tile_pool(name="ps_tr", bufs=4, space="PSUM") as ps_tr,
            ):
                xt = xt_pool.tile([P, IC, CTX], F32)
                for st in range(ST):
                    xs = xs_pool.tile([P, D_IN], F32, tag="xs")
                    nc.sync.dma_start(xs[:], x_d[st * P : (st + 1) * P, :])
                    for ic in range(IC):
                        pt = ps_tr.tile([P, P], F32, tag="pt")
                        nc.tensor.transpose(
                            pt[:], xs[:, ic * P : (ic + 1) * P], ident[:]
                        )
                        nc.vector.tensor_copy(xt[:, ic, st * P : (st + 1) * P], pt[:])

                for w_d, dst in ((wq_d, qt_d), (wk_d, kt_d)):
                    w_sb = w_pool.tile([P, IC, D_OUT], F32, tag="w")
                    nc.sync.dma_start(
                        w_sb[:], w_d[:].rearrange("(c p) o -> p c o", p=P)
                    )
                    for s0 in range(0, CTX, MMW):
                        for oc in range(OC):
                            ps = ps_proj.tile([P, MMW], F32, tag="ps")
                            for ic in range(IC):
                                nc.tensor.matmul(
                                    ps[:],
                                    w_sb[:, ic, oc * P : (oc + 1) * P],
                                    xt[:, ic, s0 : s0 + MMW],
                                    start=(ic == 0),
                                    stop=(ic == IC - 1),
                                )
                            sg = stage_pool.tile([P, MMW], F32, tag="sg")
                            nc.vector.tensor_copy(sg[:], ps[:])
                            nc.sync.dma_start(
                                dst[oc * P : (oc + 1) * P, s0 : s0 + MMW], sg[:]
                            )

                wv_sb = w_pool.tile([P, IC, D_OUT], F32, tag="w")
                nc.sync.dma_start(
                    wv_sb[:], wv_d[:].rearrange("(c p) o -> p c o", p=P)
                )
                for st in range(ST):
                    for o0 in range(0, D_OUT, MMW):
                        ps = ps_proj.tile([P, MMW], F32, tag="ps")
                        for ic in range(IC):
                            nc.tensor.matmul(
                                ps[:],
                                xt[:, ic, st * P : (st + 1) * P],
                                wv_sb[:, ic, o0 : o0 + MMW],
                                start=(ic == 0),
                                stop=(ic == IC - 1),
                            )
                        sg = stage_pool.tile([P, MMW], F32, tag="sg")
                        nc.vector.tensor_copy(sg[:], ps[:])
                        nc.sync.dma_start(
                            v_d[st * P : (st + 1) * P, o0 : o0 + MMW], sg[:]
                        )

            with (
                tc.tile_pool(name="kt", bufs=1) as kt_pool,
                tc.tile_pool(name="v", bufs=1) as v_pool,
                tc.tile_pool(name="qtb", bufs=2) as qtb_pool,
            ):
                kt_sb = kt_pool.tile([P, OC, CTX], F32)
                for c in range(OC):
                    nc.sync.dma_start(kt_sb[:, c, :], kt_d[c * P : (c + 1) * P, :])
                v_sb = v_pool.tile([P, ST, D_OUT], F32)
                for c in range(ST):
                    nc.sync.dma_start(v_sb[:, c, :], v_d[c * P : (c + 1) * P, :])

                def qt_src(qb):
                    qt_b = qtb_pool.tile([P, OC, P], F32, tag="qtb")
                    for oc in range(OC):
                        nc.sync.dma_start(
                            qt_b[:, oc, :],
                            qt_d[oc * P : (oc + 1) * P, qb * P : (qb + 1) * P],
                        )
                    return qt_b

                _attention_phase(
                    nc, tc, ident, negmask, qt_src, kt_sb, v_sb, out_d, F32
                )

    _split_multi_waits(nc)
    return nc


def build_program_fp16():
    """fp16 build: x^T and the weights are pre-cast/pre-transposed to fp16 on
    the HOST (pure layout prep; identical round-to-nearest as a DVE cast), so
    the device only does matmuls, softmax and the P transposes."""
    nc = bass.Bass()
    xt_d = nc.declare_dram_parameter("xT16", [D_IN, CTX], F16, isOutput=False)
    wq_d = nc.declare_dram_parameter("Wq16", [D_IN, D_OUT], F16, isOutput=False)
    wk_d = nc.declare_dram_parameter("Wk16", [D_IN, D_OUT], F16, isOutput=False)
    wv_d = nc.declare_dram_parameter("Wv16", [D_IN, D_OUT], F16, isOutput=False)
    negmask_d = nc.declare_dram_parameter("negmask", [P, P], F32, isOutput=False)
    out_d = nc.declare_dram_parameter("out", [CTX, D_OUT], F32, isOutput=True)

    with tile.TileContext(nc) as tc:
        with tc.tile_pool(name="consts", bufs=1) as consts:
            ident16 = consts.tile([P, P], F16)
            make_identity(nc, ident16[:])
            negmask = consts.tile([P, P], F32)
            nc.sync.dma_start(negmask[:], negmask_d[:])

            with (
                tc.tile_pool(name="qt", bufs=1) as qt_pool,
                tc.tile_pool(name="kt", bufs=1) as kt_pool,
                tc.tile_pool(name="v", bufs=1) as v_pool,
            ):
                qt_sb = qt_pool.tile([P, OC, CTX], F16)
                kt_sb = kt_pool.tile([P, OC, CTX], F16)
                v_sb = v_pool.tile([P, ST, D_OUT], F16)

                with (
                    tc.tile_pool(name="xt", bufs=1) as xt_pool,
                    tc.tile_pool(name="w", bufs=1) as w_pool,
                    tc.tile_pool(name="ps_proj", bufs=8, space="PSUM") as ps_proj,
                ):
                    # x^T arrives per (i-chunk, 512-col s-group); group-0
                    # chunks are queued BEFORE the weight loads so the first
                    # projection group only waits for ~1MB of x^T + 2MB of Wq.
                    xt = xt_pool.tile([P, IC, CTX], F16)
                    SG = MMW // P  # stripes per s-group

                    def load_xt_group(g):
                        s0 = g * MMW
                        for ic in range(IC):
                            nc.sync.dma_start(
                                xt[:, ic, s0 : s0 + MMW],
                                xt_d[ic * P : (ic + 1) * P, s0 : s0 + MMW],
                            )

                    wq_sb = w_pool.tile([P, IC, D_OUT], F16, tag="wq")
                    nc.sync.dma_start(wq_sb[:, 0, :], wq_d[0:P, :])
                    load_xt_group(0)
                    wk_sb = w_pool.tile([P, IC, D_OUT], F16, tag="wk")
                    wv_sb = w_pool.tile([P, IC, D_OUT], F16, tag="wv")
                    for ic in range(1, IC):
                        nc.sync.dma_start(
                            wq_sb[:, ic, :], wq_d[ic * P : (ic + 1) * P, :]
                        )
                    for w_d, w_sb in ((wk_d, wk_sb), (wv_d, wv_sb)):
                        for ic in range(IC):
                            nc.sync.dma_start(
                                w_sb[:, ic, :], w_d[ic * P : (ic + 1) * P, :]
                            )

                    for g in range(ST // SG):
                        s0 = g * MMW
                        if g > 0:
                            load_xt_group(g)
                        for dst, w_sb in ((qt_sb, wq_sb), (kt_sb, wk_sb)):
                            for oc in range(OC):
                                ps = ps_proj.tile([P, MMW], F32, tag="ps")
                                for ic in range(IC):
                                    nc.tensor.matmul(
                                        ps[:],
                                        w_sb[:, ic, oc * P : (oc + 1) * P],
                                        xt[:, ic, s0 : s0 + MMW],
                                        start=(ic == 0),
                                        stop=(ic == IC - 1),
                                    )
                                nc.vector.tensor_copy(
                                    dst[:, oc, s0 : s0 + MMW], ps[:]
                                )
                        for st in range(g * SG, (g + 1) * SG):
                            for o0 in range(0, D_OUT, MMW):
                                ps = ps_proj.tile([P, MMW], F32, tag="ps")
                                for ic in range(IC):
                                    nc.tensor.matmul(
                                        ps[:],
                                        xt[:, ic, st * P : (st + 1) * P],
                                        wv_sb[:, ic, o0 : o0 + MMW],
                                        start=(ic == 0),
                                        stop=(ic == IC - 1),
                                    )
                                nc.vector.tensor_copy(
                                    v_sb[:, st, o0 : o0 + MMW], ps[:]
                                )

                def qt_src(qb):
                    return qt_sb[:, :, qb * P : (qb + 1) * P]

                _attention_phase(
                    nc, tc, ident16, negmask, qt_src, kt_sb, v_sb, out_d, F16
                )

    _split_multi_waits(nc)
    return nc


def _attention_phase_hybrid(
    nc, tc, ident16, negmask, qt16, kt16, v16, qt8, kt8, v8, out_d, dbg=None
):
    """Causal attention with a per-q-block dtype split: blocks < R16 run the
    fp16 path (qt16/kt16/v16), blocks >= R16 run fp8 DoubleRow matmuls
    (qt8/kt8/v8, 0.5 cyc/row).  Softmax is fp32 either way; P is built fp16,
    transposed fp16 on the PE, and cast to fp8 on the PSUM->SBUF copy for the
    fp8 blocks."""
    with (
        tc.tile_pool(name="pexp", bufs=3) as p_pool,
        tc.tile_pool(name="ptr16", bufs=2) as pt16_pool,
        tc.tile_pool(name="ptr8", bufs=2) as pt8_pool,
        tc.tile_pool(name="red", bufs=3) as red_pool,
        tc.tile_pool(name="ob", bufs=3) as o_pool,
        tc.tile_pool(name="ps_s", bufs=4, space="PSUM") as ps_s,
        tc.tile_pool(name="ps_o", bufs=2, space="PSUM") as ps_o,
        tc.tile_pool(name="ps_pt", bufs=2, space="PSUM") as ps_pt,
    ):

        def emit_scores(qb):
            """Single-pass softmax: score tiles stay in PSUM until the
            block max is known, then one exp pass per tile writes fp16 P
            (x PSCALE on the fp8 path) with accum_out row sums."""
            L = (qb + 1) * P
            ktiles = [(k0, min(MMW, L - k0)) for k0 in range(0, L, MMW)]
            nkt = len(ktiles)
            fp8 = qb >= R16
            sc = SC8 if fp8 else SC16

            red = red_pool.tile([P, 2 * nkt + 4], F32, tag="red")
            mx = red[:, 0:nkt]
            sm = red[:, nkt : 2 * nkt]
            mx_all = red[:, 2 * nkt : 2 * nkt + 1]
            negm_all = red[:, 2 * nkt + 1 : 2 * nkt + 2]
            rsum = red[:, 2 * nkt + 2 : 2 * nkt + 3]
            rinv = red[:, 2 * nkt + 3 : 2 * nkt + 4]

            p_sb = p_pool.tile([P, CTX], F16, tag="p")
            pss = []
            for idx, (k0, w) in enumerate(ktiles):
                ps = ps_s.tile([P, MMW], F32, tag="ps_s")
                pss.append(ps)
                if not fp8:
                    for oc in range(OC):
                        nc.tensor.matmul(
                            ps[:, :w],
                            qt16[:, oc, qb * P : (qb + 1) * P],
                            kt16[:, oc, k0 : k0 + w],
                            start=(oc == 0),
                            stop=(oc == OC - 1),
                        )
                else:
                    for c in range(OC // 2):
                        nc.tensor.matmul(
                            ps[:, :w],
                            qt8[:, 2 * c : 2 * c + 2, qb * P : (qb + 1) * P],
                            kt8[:, 2 * c : 2 * c + 2, k0 : k0 + w],
                            start=(c == 0),
                            stop=(c == OC // 2 - 1),
                            perf_mode=DR,
                        )
                if idx == nkt - 1:
                    nc.vector.tensor_add(
                        ps[:, w - P : w], ps[:, w - P : w], negmask[:]
                    )
                nc.vector.reduce_max(
                    mx[:, idx : idx + 1], ps[:, :w], axis=mybir.AxisListType.X
                )
            if nkt > 1:
                nc.vector.tensor_reduce(
                    mx_all[:], mx[:], axis=mybir.AxisListType.X,
                    op=mybir.AluOpType.max,
                )
            else:
                mx_all = mx
            nc.scalar.mul(negm_all[:], mx_all[:, 0:1], -sc)
            if fp8:
                # fold ln(PSCALE) into the exp bias: P comes out x64, past
                # e4m3's subnormal range; rsum picks up the same factor and
                # with v arriving x32 the normalization needs a further 1/32.
                nc.vector.tensor_scalar_add(
                    negm_all[:], negm_all[:], float(np.log(PSCALE))
                )
            for idx, (k0, w) in enumerate(ktiles):
                nc.scalar.activation(
                    p_sb[:, k0 : k0 + w],
                    pss[idx][:, :w],
                    mybir.ActivationFunctionType.Exp,
                    bias=negm_all[:, 0:1],
                    scale=sc,
                    accum_out=sm[:, idx : idx + 1],
                )
            if nkt > 1:
                nc.vector.reduce_sum(
                    rsum[:], sm[:], axis=mybir.AxisListType.X
                )
            else:
                rsum = sm
            nc.vector.reciprocal(rinv[:], rsum[:])
            if fp8:
                nc.scalar.mul(rinv[:], rinv[:], 1.0 / WSCALE)
            if dbg is not None and qb == DEBUG_QB:
                nc.sync.dma_start(dbg["p16"][:, :L], p_sb[:, :L])
                nc.sync.dma_start(dbg["red"][:], red[:])
            return {"qb": qb, "p_sb": p_sb, "rinv": rinv}

        def emit_pv(stt):
            qb = stt["qb"]
            p_sb = stt["p_sb"]
            rinv = stt["rinv"]
            nch = qb + 1
            fp8 = qb >= R16
            if fp8:
                pt_sb = pt8_pool.tile([P, ST, P], F8, tag="pt8")
            else:
                pt_sb = pt16_pool.tile([P, R16, P], F16, tag="pt16")
            for c0 in range(0, nch, 4):
                cn = min(4, nch - c0)
                ptp = ps_pt.tile([P, 4 * P], F16, tag="ptp")
                for j in range(cn):
                    kc = c0 + j
                    nc.tensor.transpose(
                        ptp[:, j * P : (j + 1) * P],
                        p_sb[:, kc * P : (kc + 1) * P],
                        ident16[:],
                    )
                nc.scalar.mul(
                    pt_sb[:, c0 : c0 + cn, :], ptp[:, : cn * P], 1.0
                )

            if dbg is not None and qb == DEBUG_QB:
                nc.sync.dma_start(
                    dbg["pt8"][:, : nch * P],
                    pt_sb[:, 0:nch, :],
                )
            o_sb = o_pool.tile([P, D_OUT], F32, tag="o")
            for o0 in range(0, D_OUT, MMW):
                pso = ps_o.tile([P, MMW], F32, tag="ps_o")
                if fp8:
                    npair = nch // 2
                    for c in range(npair):
                        nc.tensor.matmul(
                            pso[:],
                            pt_sb[:, 2 * c : 2 * c + 2, :],
                            v8[:, 2 * c : 2 * c + 2, o0 : o0 + MMW],
                            start=(c == 0),
                            stop=(c == npair - 1 and nch % 2 == 0),
                            perf_mode=DR,
                        )
                    if nch % 2 == 1:
                        nc.tensor.matmul(
                            pso[:],
                            pt_sb[:, nch - 1, :],
                            v8[:, nch - 1, o0 : o0 + MMW],
                            start=False,
                            stop=True,
                        )
                else:
                    for kc in range(nch):
                        nc.tensor.matmul(
                            pso[:],
                            pt_sb[:, kc, :],
                            v16[:, kc, o0 : o0 + MMW],
                            start=(kc == 0),
                            stop=(kc == nch - 1),
                        )
                nc.vector.tensor_scalar_mul(
                    o_sb[:, o0 : o0 + MMW], pso[:], rinv[:, 0:1]
                )
            nc.sync.dma_start(out_d[qb * P : (qb + 1) * P, :], o_sb[:])

        # three-stage software pipeline: while block n's softmax chain runs
        # on DVE/ACT, the PE executes PV(n-2) and the scores of n+1.  Tiny
        # blocks bracket the pipeline so its exposed fill/drain is cheap.
        order = [3, 2] + list(range(QB - 1, 3, -1)) + [1, 0]
        pend = []
        for qb in order:
            pend.append(emit_scores(qb))
            if len(pend) > 2:
                emit_pv(pend.pop(0))
        for stt in pend:
            emit_pv(stt)


def build_program_fp8():
    """Hybrid fp16/fp8 build.  Rows < RS go through the fp16 pipeline
    (projections and attention), rows >= RS through fp8 DoubleRow matmuls
    (2x PE throughput).  Early K/V are cast fp16->fp8 on the DVE so late
    blocks can consume them in DoubleRow mode.  Host pre-casts x^T and the
    weights to fp16 and fp8 (pure dtype/layout prep, same round-to-nearest
    as a DVE cast)."""
    nc = bass.Bass()
    xt16_d = nc.declare_dram_parameter("xT16pre", [D_IN, RS], F16, isOutput=False)
    xt8_d = nc.declare_dram_parameter("xT8post", [D_IN, CTX - RS], F8, isOutput=False)
    wq16_d = nc.declare_dram_parameter("Wq16", [D_IN, D_OUT], F16, isOutput=False)
    wk16_d = nc.declare_dram_parameter("Wk16", [D_IN, D_OUT], F16, isOutput=False)
    wv16_d = nc.declare_dram_parameter("Wv16", [D_IN, D_OUT], F16, isOutput=False)
    wq8_d = nc.declare_dram_parameter("Wq8", [D_IN, D_OUT], F8, isOutput=False)
    wk8_d = nc.declare_dram_parameter("Wk8", [D_IN, D_OUT], F8, isOutput=False)
    wv8_d = nc.declare_dram_parameter("Wv8", [D_IN, D_OUT], F8, isOutput=False)
    negmask_d = nc.declare_dram_parameter("negmask", [P, P], F32, isOutput=False)
    out_d = nc.declare_dram_parameter("out", [CTX, D_OUT], F32, isOutput=True)
    dbg = None
    if DEBUG_DUMP:
        LD = (DEBUG_QB + 1) * P
        dbg = {
            "qt8": nc.declare_dram_parameter(
                "dbg_qt8", [D_OUT, CTX - RS], F8, isOutput=True
            ),
            "kt8": nc.declare_dram_parameter(
                "dbg_kt8", [D_OUT, CTX], F8, isOutput=True
            ),
            "v8": nc.declare_dram_parameter(
                "dbg_v8", [CTX, D_OUT], F8, isOutput=True
            ),
            "p32": nc.declare_dram_parameter(
                "dbg_p32", [P, LD], F32, isOutput=True
            ),
            "p16": nc.declare_dram_parameter(
                "dbg_p16", [P, LD], F16, isOutput=True
            ),
            "pt8": nc.declare_dram_parameter(
                "dbg_pt8", [P, LD], F8, isOutput=True
            ),
            "red": nc.declare_dram_parameter(
                "dbg_red", [P, 4 * 2 + 3], F32, isOutput=True
            ),
        }

    with tile.TileContext(nc) as tc:
        with tc.tile_pool(name="consts", bufs=1) as consts:
            ident16 = consts.tile([P, P], F16)
            make_identity(nc, ident16[:])
            negmask = consts.tile([P, P], F32)
            nc.sync.dma_start(negmask[:], negmask_d[:])

            with (
                tc.tile_pool(name="qt16", bufs=1) as qt16_pool,
                tc.tile_pool(name="kt16", bufs=1) as kt16_pool,
                tc.tile_pool(name="v16", bufs=1) as v16_pool,
                tc.tile_pool(name="qt8", bufs=1) as qt8_pool,
                tc.tile_pool(name="kt8", bufs=1) as kt8_pool,
                tc.tile_pool(name="v8", bufs=1) as v8_pool,
                tc.tile_pool(name="w8", bufs=1) as w8_pool,
                tc.tile_pool(name="xt8", bufs=1) as xt8_pool,
            ):
                qt16 = qt16_pool.tile([P, OC, RS], F16)
                kt16 = kt16_pool.tile([P, OC, RS], F16)
                v16 = v16_pool.tile([P, R16, D_OUT], F16)
                qt8 = qt8_pool.tile([P, OC, CTX], F8)
                kt8 = kt8_pool.tile([P, OC, CTX], F8)
                v8 = v8_pool.tile([P, ST, D_OUT], F8)
                wq8 = w8_pool.tile([P, IC, D_OUT], F8, tag="wq8")
                wk8 = w8_pool.tile([P, IC, D_OUT], F8, tag="wk8")
                wv8 = w8_pool.tile([P, IC, D_OUT], F8, tag="wv8")
                xt8 = xt8_pool.tile([P, IC, CTX], F8)

                # ---- fp8 DoubleRow projections for rows >= RS (first: they
                # need only ~4.4MB of DMA, so the PE starts almost
                # immediately; the 7.3MB fp16-phase tensors stream behind)
                sgroups = []
                s0 = RS
                while s0 < CTX:
                    w = min(MMW, CTX - s0)
                    sgroups.append((s0, w))
                    s0 += w
                with tc.tile_pool(name="ps_p8", bufs=8, space="PSUM") as ps_p8:
                    # one DMA descriptor per tensor/group: each dma_start
                    # costs ~0.6us of sync-queue issue time, so per-chunk
                    # descriptors would stall the PE at startup
                    nc.sync.dma_start(
                        wq8[:, :, : D_OUT // 2],
                        wq8_d[:, : D_OUT // 2].rearrange(
                            "(c p) o -> p c o", p=P
                        ),
                    )
                    s0, w = sgroups[0]
                    nc.sync.dma_start(
                        xt8[:, :, s0 : s0 + w],
                        xt8_d[:, : s0 + w - RS].rearrange(
                            "(c p) s -> p c s", p=P
                        ),
                    )
                    nc.sync.dma_start(
                        wq8[:, :, D_OUT // 2 :],
                        wq8_d[:, D_OUT // 2 :].rearrange(
                            "(c p) o -> p c o", p=P
                        ),
                    )
                    for w_d, w_sb in ((wk8_d, wk8), (wv8_d, wv8)):
                        nc.sync.dma_start(
                            w_sb[:], w_d[:].rearrange("(c p) o -> p c o", p=P)
                        )
                    for s0, w in sgroups[1:]:
                        nc.sync.dma_start(
                            xt8[:, :, s0 : s0 + w],
                            xt8_d[:, s0 - RS : s0 + w - RS].rearrange(
                                "(c p) s -> p c s", p=P
                            ),
                        )
                    # PSUM->SBUF casts alternate between DVE and ACT: a
                    # [128,512] cast (~0.6us) costs more than the 4 paired
                    # DR matmuls feeding it (~0.43us), so a single engine
                    # would gate the PE here.
                    for s0, w in sgroups:
                        for dst, w_sb in ((qt8, wq8), (kt8, wk8)):
                            for oc in range(OC):
                                ps = ps_p8.tile([P, MMW], F32, tag="ps")
                                for c in range(IC // 2):
                                    nc.tensor.matmul(
                                        ps[:, :w],
                                        w_sb[
                                            :,
                                            2 * c : 2 * c + 2,
                                            oc * P : (oc + 1) * P,
                                        ],
                                        xt8[:, 2 * c : 2 * c + 2, s0 : s0 + w],
                                        start=(c == 0),
                                        stop=(c == IC // 2 - 1),
                                        perf_mode=DR,
                                    )
                                if oc % 2 == 0:
                                    nc.vector.tensor_copy(
                                        dst[:, oc, s0 : s0 + w], ps[:, :w]
                                    )
                                else:
                                    nc.scalar.mul(
                                        dst[:, oc, s0 : s0 + w], ps[:, :w], 1.0
                                    )
                        for st in range(s0 // P, (s0 + w) // P):
                            for o0 in range(0, D_OUT, MMW):
                                ps = ps_p8.tile([P, MMW], F32, tag="ps")
                                for c in range(IC // 2):
                                    nc.tensor.matmul(
                                        ps[:],
                                        xt8[
                                            :,
                                            2 * c : 2 * c + 2,
                                            st * P : (st + 1) * P,
                                        ],
                                        wv8[:, 2 * c : 2 * c + 2, o0 : o0 + MMW],
                                        start=(c == 0),
                                        stop=(c == IC // 2 - 1),
                                        perf_mode=DR,
                                    )
                                if (st + o0 // MMW) % 2 == 0:
                                    nc.vector.tensor_copy(
                                        v8[:, st, o0 : o0 + MMW], ps[:]
                                    )
                                else:
                                    nc.scalar.mul(
                                        v8[:, st, o0 : o0 + MMW], ps[:], 1.0
                                    )

                # ---- fp16 projections for rows < RS ----
                with (
                    tc.tile_pool(name="w16", bufs=1) as w16_pool,
                    tc.tile_pool(name="xt16", bufs=1) as xt16_pool,
                    tc.tile_pool(name="ps_p16", bufs=8, space="PSUM") as ps_p16,
                ):
                    wq16 = w16_pool.tile([P, IC, D_OUT], F16, tag="wq16")
                    wk16 = w16_pool.tile([P, IC, D_OUT], F16, tag="wk16")
                    wv16 = w16_pool.tile([P, IC, D_OUT], F16, tag="wv16")
                    xt16 = xt16_pool.tile([P, IC, RS], F16)
                    nc.sync.dma_start(
                        xt16[:], xt16_d[:].rearrange("(c p) s -> p c s", p=P)
                    )
                    for w_d, w_sb in (
                        (wq16_d, wq16),
                        (wk16_d, wk16),
                        (wv16_d, wv16),
                    ):
                        nc.sync.dma_start(
                            w_sb[:], w_d[:].rearrange("(c p) o -> p c o", p=P)
                        )

                    for dst, w_sb in ((qt16, wq16), (kt16, wk16)):
                        for s0, w in ((0, RS // 2), (RS // 2, RS // 2)):
                            for oc in range(OC):
                                ps = ps_p16.tile([P, MMW], F32, tag="ps")
                                for ic in range(IC):
                                    nc.tensor.matmul(
                                        ps[:, :w],
                                        w_sb[:, ic, oc * P : (oc + 1) * P],
                                        xt16[:, ic, s0 : s0 + w],
                                        start=(ic == 0),
                                        stop=(ic == IC - 1),
                                    )
                                nc.vector.tensor_copy(
                                    dst[:, oc, s0 : s0 + w], ps[:, :w]
                                )
                    for st in range(R16):
                        for o0 in range(0, D_OUT, MMW):
                            ps = ps_p16.tile([P, MMW], F32, tag="ps")
                            for ic in range(IC):
                                nc.tensor.matmul(
                                    ps[:],
                                    xt16[:, ic, st * P : (st + 1) * P],
                                    wv16[:, ic, o0 : o0 + MMW],
                                    start=(ic == 0),
                                    stop=(ic == IC - 1),
                                )
                            nc.vector.tensor_copy(
                                v16[:, st, o0 : o0 + MMW], ps[:]
                            )
                    # early K/V cast to fp8 (x32, matching the scaled fp8
                    # projections) for the late fp8 blocks
                    for oc in range(OC):
                        nc.scalar.mul(kt8[:, oc, 0:RS], kt16[:, oc, :], WSCALE)
                    for st in range(R16):
                        nc.scalar.mul(v8[:, st, :], v16[:, st, :], WSCALE)

                if dbg is not None:
                    for oc in range(OC):
                        nc.sync.dma_start(
                            dbg["qt8"][oc * P : (oc + 1) * P, :],
                            qt8[:, oc, RS:CTX],
                        )
                        nc.sync.dma_start(
                            dbg["kt8"][oc * P : (oc + 1) * P, :], kt8[:, oc, :]
                        )
                    for st in range(ST):
                        nc.sync.dma_start(
                            dbg["v8"][st * P : (st + 1) * P, :], v8[:, st, :]
                        )

                _attention_phase_hybrid(
                    nc, tc, ident16, negmask, qt16, kt16, v16, qt8, kt8, v8,
                    out_d, dbg,
                )

    _split_multi_waits(nc)
    return nc


_program_cache = {}


def build_program(mode=None):
    mode = mode or MODE
    if mode == "fp32":
        return build_program_fp32()
    elif mode == "fp16":
        return build_program_fp16()
    elif mode == "fp8":
        return build_program_fp8()
    raise ValueError(mode)


def make_in_maps(x, Wq, Wk, Wv):
    x = np.ascontiguousarray(np.asarray(x), dtype=np.float32)
    Wq = np.ascontiguousarray(np.asarray(Wq), dtype=np.float32)
    Wk = np.ascontiguousarray(np.asarray(Wk), dtype=np.float32)
    Wv = np.ascontiguousarray(np.asarray(Wv), dtype=np.float32)

    iu = np.triu(np.ones((P, P), dtype=np.float32), k=1)
    negmask = (iu * NEG_BIG).astype(np.float32)

    if MODE == "fp8":
        import ml_dtypes

        F8NP = ml_dtypes.float8_e4m3
        xT = np.transpose(x, (0, 2, 1))  # [b, d_in, ctx]
        xT16pre = np.ascontiguousarray(xT[:, :, :RS].astype(np.float16))
        xT8post = np.ascontiguousarray(xT[:, :, RS:].astype(F8NP))
        wq16 = np.ascontiguousarray(Wq.astype(np.float16))
        wk16 = np.ascontiguousarray(Wk.astype(np.float16))
        wv16 = np.ascontiguousarray(Wv.astype(np.float16))
        ws = np.float32(WSCALE)
        wq8 = np.ascontiguousarray((Wq * ws).astype(F8NP))
        wk8 = np.ascontiguousarray((Wk * ws).astype(F8NP))
        wv8 = np.ascontiguousarray((Wv * ws).astype(F8NP))
        in_maps = [
            {
                "xT16pre": xT16pre[b],
                "xT8post": xT8post[b],
                "Wq16": wq16,
                "Wk16": wk16,
                "Wv16": wv16,
                "Wq8": wq8,
                "Wk8": wk8,
                "Wv8": wv8,
                "negmask": negmask,
            }
            for b in range(BATCH)
        ]
    elif MODE == "fp16":
        # host-side layout prep: fp16 round-to-nearest (same as a DVE cast)
        # and the x transpose the device would otherwise do on the PE
        xT16 = np.ascontiguousarray(
            np.transpose(x.astype(np.float16), (0, 2, 1))
        )
        wq16 = np.ascontiguousarray(Wq.astype(np.float16))
        wk16 = np.ascontiguousarray(Wk.astype(np.float16))
        wv16 = np.ascontiguousarray(Wv.astype(np.float16))
        in_maps = [
            {
                "xT16": xT16[b],
                "Wq16": wq16,
                "Wk16": wk16,
                "Wv16": wv16,
                "negmask": negmask,
            }
            for b in range(BATCH)
        ]
    else:
        in_maps = [
            {"x": x[b], "Wq": Wq, "Wk": Wk, "Wv": Wv, "negmask": negmask}
            for b in range(BATCH)
        ]
    return in_maps


def kernel(x, Wq, Wk, Wv):
    from concourse.bass_utils import run_bass_kernel_spmd

    if MODE not in _program_cache:
        _program_cache[MODE] = build_program(MODE)
    nc = _program_cache[MODE]

    in_maps = make_in_maps(x, Wq, Wk, Wv)
    res = run_bass_kernel_spmd(nc, in_maps, list(range(N_CORES)))
    return np.stack([res.results[b]["out"] for b in range(BATCH)], axis=0)

